# revision 28
# baseline (speedup 1.0000x reference)
"""CoocGNN Trainium2 kernel: 8-core SPMD, batch-parallel feats + replicated graph.

Contract: kernel(**inputs) takes FULL inputs (as produced by setup_inputs) and
returns the FULL output tuple (W_adj, aw, dlog, cam_vis, refined, total).
"""

import os
import sys

for _p in ("/opt/trn_rl_repo", os.path.expanduser("~/.axon_site/_ro/trn_rl_repo")):
    if os.path.isdir(_p) and _p not in sys.path:
        sys.path.insert(0, _p)

import numpy as np

import concourse.bacc as bacc
import concourse.tile as tile
from concourse import masks, mybir
from concourse.bass_utils import run_bass_kernel_spmd

F32 = mybir.dt.float32
F16 = mybir.dt.float16
AX = mybir.AxisListType
ALU = mybir.AluOpType
ACTF = mybir.ActivationFunctionType

N_CORES = 8
B, C, F, H, W = 32, 80, 2048, 32, 32
PIX = H * W          # 1024
BL = B // N_CORES    # 4 images per core
HID = 256
EDD = 64
IL = C // N_CORES    # 10 edge-rows per core
TEMP = 2.5
BETA_POS = 0.5
GAMMA_NEG = 0.25
KF = F // 128        # 16 f-chunks
EPS = 1e-6


def _build_program(nc, tc):
    ps_cm = tc.tile_pool(name="ps", bufs=3, space="PSUM")
    ps = ps_cm.__enter__()
    psacc_cm = tc.tile_pool(name="psacc", bufs=3, space="PSUM")
    psacc = psacc_cm.__enter__()
    pscam_cm = tc.tile_pool(name="pscam", bufs=2, space="PSUM")
    pscam = pscam_cm.__enter__()
    dram_cm = tc.tile_pool(name="dram", bufs=1, space="DRAM")
    dram = dram_cm.__enter__()
    base_cm = tc.tile_pool(name="base", bufs=1)
    base = base_cm.__enter__()
    wts_cm = tc.tile_pool(name="wts", bufs=1)
    wts = wts_cm.__enter__()

    t_feats = nc.dram_tensor("feats_l", [BL, F, PIX], F32, kind="ExternalInput")
    t_cls = nc.dram_tensor("cls_logits", [B, C], F32, kind="ExternalInput")
    t_tgt = nc.dram_tensor("tgt", [B, C], F32, kind="ExternalInput")
    t_prior = nc.dram_tensor("prior_pmi", [C, C], F32, kind="ExternalInput")
    t_isel = nc.dram_tensor("iselT", [C, IL], F32, kind="ExternalInput")
    t_pp_w1 = nc.dram_tensor("pp_w1", [F, HID], F32, kind="ExternalInput")
    t_pp_w2 = nc.dram_tensor("pp_w2", [HID, HID], F32, kind="ExternalInput")
    t_msg_w1 = nc.dram_tensor("msg_w1", [HID, HID], F32, kind="ExternalInput")
    t_msg_w2 = nc.dram_tensor("msg_w2", [HID, HID], F32, kind="ExternalInput")
    t_ed_w = nc.dram_tensor("ed_w", [HID, EDD], F32, kind="ExternalInput")
    t_em_w1 = nc.dram_tensor("em_w1", [4 * EDD + 3, 128], F32, kind="ExternalInput")
    t_em_w2 = nc.dram_tensor("em_w2", [128, 64], F32, kind="ExternalInput")
    t_em_w3 = nc.dram_tensor("em_w3", [64, 1], F32, kind="ExternalInput")
    t_alpha_w = nc.dram_tensor("alpha_w", [HID, F], F32, kind="ExternalInput")
    t_bias_w = nc.dram_tensor("bias_w", [HID, 1], F32, kind="ExternalInput")
    t_pp_b1 = nc.dram_tensor("pp_b1", [HID], F32, kind="ExternalInput")
    t_pp_b2 = nc.dram_tensor("pp_b2", [HID], F32, kind="ExternalInput")
    t_msg_b1 = nc.dram_tensor("msg_b1", [HID], F32, kind="ExternalInput")
    t_msg_b2 = nc.dram_tensor("msg_b2", [HID], F32, kind="ExternalInput")
    t_ed_b = nc.dram_tensor("ed_b", [EDD], F32, kind="ExternalInput")
    t_em_b1 = nc.dram_tensor("em_b1", [128], F32, kind="ExternalInput")
    t_em_b2 = nc.dram_tensor("em_b2", [64], F32, kind="ExternalInput")
    t_em_b3 = nc.dram_tensor("em_b3", [1], F32, kind="ExternalInput")
    t_alpha_b = nc.dram_tensor("alpha_b", [F], F32, kind="ExternalInput")
    t_bias_b = nc.dram_tensor("bias_b", [1], F32, kind="ExternalInput")

    o_wadj = nc.dram_tensor("o_wadj", [C, C], F32, kind="ExternalOutput")
    o_aw = nc.dram_tensor("o_aw", [C, F], F32, kind="ExternalOutput")
    o_dlog = nc.dram_tensor("o_dlog", [C], F32, kind="ExternalOutput")
    o_refined = nc.dram_tensor("o_refined", [B, C], F32, kind="ExternalOutput")
    o_total = nc.dram_tensor("o_total", [1], F32, kind="ExternalOutput")
    o_cam = nc.dram_tensor("o_cam", [BL, C, PIX], F32, kind="ExternalOutput")

    # ---------------- constants / small inputs ----------------
    id128 = base.tile([128, 128], F32, tag="id128")
    masks.make_identity(nc, id128[:])
    ones_col = base.tile([128, 1], F32, tag="ones_col")
    nc.vector.memset(ones_col[:], 1.0)
    ones_row = base.tile([1, 128], F32, tag="ones_row")
    nc.vector.memset(ones_row[:], 1.0)
    ones_row16 = base.tile([1, 128], F16, tag="ones_row16")
    nc.vector.memset(ones_row16[:], 1.0)
    offd = base.tile([C, C], F32, tag="offd")
    nc.vector.memset(offd[:], 1.0)
    nc.vector.tensor_sub(offd[:], offd[:], id128[:C, :C])

    def load(pool, shape, dram_ap, tag, dt=F32):
        t = pool.tile(shape, dt, tag=tag)
        nc.sync.dma_start(t[:], dram_ap)
        return t

    cls_sb = load(base, [B, C], t_cls[:, :], "cls_sb")
    tgt_sb = load(base, [B, C], t_tgt[:, :], "tgt_sb")
    prior_sb = load(base, [C, C], t_prior[:, :], "prior_sb")
    isel_sb = load(base, [C, IL], t_isel[:, :], "isel_sb")

    pp_w2_sb = load(wts, [128, 2, HID], t_pp_w2.ap().rearrange("(k p) h -> p k h", p=128), "pp_w2_sb")
    msg_w1_sb = load(wts, [128, 2, HID], t_msg_w1.ap().rearrange("(k p) h -> p k h", p=128), "msg_w1_sb")
    msg_w2_sb = load(wts, [128, 2, HID], t_msg_w2.ap().rearrange("(k p) h -> p k h", p=128), "msg_w2_sb")
    ed_w_sb = load(wts, [128, 2, EDD], t_ed_w.ap().rearrange("(k p) h -> p k h", p=128), "ed_w_sb")
    bias_w_sb = load(wts, [128, 2, 1], t_bias_w.ap().rearrange("(k p) h -> p k h", p=128), "bias_w_sb")

    ew1_a = load(wts, [EDD, 128], t_em_w1[0:EDD, :], "ew1_a")
    ew1_b = load(wts, [EDD, 128], t_em_w1[EDD:2 * EDD, :], "ew1_b")
    ew1_fi = load(wts, [1, 128], t_em_w1[4 * EDD + 1:4 * EDD + 2, :], "ew1_fi")
    ew1_fj = load(wts, [1, 128], t_em_w1[4 * EDD + 2:4 * EDD + 3, :], "ew1_fj")
    # fp16 copies for the edge MLP (f32 staging in a short-lived pool)
    ew1_abs16 = wts.tile([EDD, 128], F16, tag="ew1_abs16")
    ew1_prod16 = wts.tile([EDD, 128], F16, tag="ew1_prod16")
    ew1_cos16 = wts.tile([1, 128], F16, tag="ew1_cos16")
    em_w2_16 = wts.tile([128, 64], F16, tag="em_w2_16")
    em_w3_16 = wts.tile([64, 1], F16, tag="em_w3_16")
    b_alpha = wts.tile([1, F], F16, tag="b_alpha")

    b_pp1 = load(wts, [1, HID], t_pp_b1.ap().rearrange("(o h) -> o h", o=1), "b_pp1")
    b_pp2 = load(wts, [1, HID], t_pp_b2.ap().rearrange("(o h) -> o h", o=1), "b_pp2")
    b_m1 = load(wts, [1, HID], t_msg_b1.ap().rearrange("(o h) -> o h", o=1), "b_m1")
    b_m2 = load(wts, [1, HID], t_msg_b2.ap().rearrange("(o h) -> o h", o=1), "b_m2")
    b_ed = load(wts, [1, EDD], t_ed_b.ap().rearrange("(o h) -> o h", o=1), "b_ed")
    b_em1 = load(wts, [128, 1], t_em_b1.ap().rearrange("(p o) -> p o", o=1), "b_em1")
    b_em2 = load(wts, [64, 1], t_em_b2.ap().rearrange("(p o) -> p o", o=1), "b_em2")
    b_em3 = load(wts, [1, 1], t_em_b3.ap().rearrange("(p o) -> p o", o=1), "b_em3")
    b_biasb = load(wts, [1, 1], t_bias_b.ap().rearrange("(p o) -> p o", o=1), "b_biasb")

    # cpm: mid-lived tensors that survive into late phase C
    cpm_cm = tc.tile_pool(name="cpm", bufs=1)
    cpm = cpm_cm.__enter__()

    # pp_w1 lives in its own pool; released after Hn1 so alpha_w can reuse it.
    w1pool_cm = tc.tile_pool(name="w1pool", bufs=1)
    w1pool = w1pool_cm.__enter__()
    pp_w1_sb = w1pool.tile([128, KF, HID], F32, tag="pp_w1_sb")
    nc.sync.dma_start(pp_w1_sb[:], t_pp_w1.ap().rearrange("(k p) h -> p k h", p=128))

    # f32 staging for the fp16 weight copies; freed before phase A pressure
    with tc.tile_pool(name="stg", bufs=1) as stg:
        ew1_abs = load(stg, [EDD, 128], t_em_w1[2 * EDD:3 * EDD, :], "ew1_abs")
        nc.scalar.copy(ew1_abs16[:], ew1_abs[:])
        ew1_prod = load(stg, [EDD, 128], t_em_w1[3 * EDD:4 * EDD, :], "ew1_prod")
        nc.scalar.copy(ew1_prod16[:], ew1_prod[:])
        ew1_cos = load(stg, [1, 128], t_em_w1[4 * EDD:4 * EDD + 1, :], "ew1_cos")
        nc.scalar.copy(ew1_cos16[:], ew1_cos[:])
        em_w2_sb = load(stg, [128, 64], t_em_w2[:, :], "em_w2_sb")
        nc.scalar.copy(em_w2_16[:], em_w2_sb[:])
        em_w3_sb = load(stg, [64, 1], t_em_w3[:, :], "em_w3_sb")
        nc.scalar.copy(em_w3_16[:], em_w3_sb[:])
        b_alpha32 = load(stg, [1, F], t_alpha_b.ap().rearrange("(o h) -> o h", o=1), "b_alpha32")
        nc.scalar.copy(b_alpha[:], b_alpha32[:])

    # ---------------- phase A: stream feats, stash fp16, pool ----------------
    stash = base.tile([128, BL, KF, PIX], F16, tag="stash")
    pooled_sb = base.tile([128, BL, KF], F32, tag="pooled_sb")
    pooled_loc = dram.tile([BL, F], F32)
    pooled_full_d = dram.tile([B, F], F32)

    with tc.tile_pool(name="ina", bufs=3) as ina:
        for b in range(BL):
            for i in range(KF // 2):
                tin = ina.tile([128, 2, PIX], F32, tag="tin")
                nc.sync.dma_start(
                    tin[:],
                    t_feats[b, 256 * i:256 * (i + 1), :].rearrange("(c p) n -> p c n", p=128),
                )
                nc.scalar.copy(stash[:, b, 2 * i:2 * i + 2, :], tin[:])
                nc.vector.tensor_reduce(
                    pooled_sb[:, b, 2 * i:2 * i + 2], tin[:], axis=AX.X, op=ALU.add
                )
            nc.sync.dma_start(
                pooled_loc[:][b].rearrange("(k p) -> p k", p=128), pooled_sb[:, b, :]
            )

    nc.gpsimd.collective_compute(
        "AllGather", ALU.bypass,
        replica_groups=[list(range(N_CORES))],
        ins=[pooled_loc.opt()], outs=[pooled_full_d.opt()],
    )

    # ---------------- phase C (early): proto / cos / Hn1 ----------------
    cpe_cm = tc.tile_pool(name="cpe", bufs=1)
    cpe = cpe_cm.__enter__()

    pooled_full = cpe.tile([B, F], F32, tag="pooled_full")
    nc.sync.dma_start(pooled_full[:], pooled_full_d[:])

    def sigmoid(out_ap, in_ap, scale=1.0):
        # 1 / (1 + exp(-x*scale)) via Exp + DVE reciprocal (single ACT table)
        nc.scalar.activation(out_ap, in_ap, ACTF.Exp, scale=-scale)
        nc.vector.tensor_scalar_add(out_ap, out_ap, 1.0)
        nc.vector.reciprocal(out_ap, out_ap)

    def softplus_neg(out_ap, in_ap):
        # log1p(exp(-x)) for x >= 0
        nc.scalar.activation(out_ap, in_ap, ACTF.Exp, scale=-1.0)
        nc.scalar.activation(out_ap, out_ap, ACTF.Ln, bias=1.0)

    probs = base.tile([B, C], F32, tag="probs")
    sigmoid(probs[:], cls_sb[:])
    y_sb = base.tile([B, C], F32, tag="y_sb")
    nc.vector.tensor_scalar_max(y_sb[:], tgt_sb[:], 0.0)

    # weight_sum / freq / present
    ws_ps = ps.tile([C, 1], F32, tag="ps")
    nc.tensor.matmul(ws_ps[:], probs[:], ones_col[:B, :])
    wsum = cpm.tile([C, 1], F32, tag="wsum")
    nc.scalar.copy(wsum[:], ws_ps[:])
    freq_col = cpm.tile([C, 1], F32, tag="freq_col")
    nc.scalar.mul(freq_col[:], wsum[:], 1.0 / B)
    wclamp = cpm.tile([C, 1], F32, tag="wclamp")
    nc.vector.tensor_scalar_max(wclamp[:], wsum[:], EPS)
    winv = cpm.tile([C, 1], F32, tag="winv")
    nc.vector.reciprocal(winv[:], wclamp[:])
    winv_eff = cpm.tile([C, 1], F32, tag="winv_eff")
    nc.scalar.mul(winv_eff[:], winv[:], 1.0 / PIX)
    # row of wsum*PIX (to inject exact pp_b1 under the later winv_eff scaling)
    wsp_col = cpm.tile([C, 1], F32, tag="wsp_col")
    nc.scalar.mul(wsp_col[:], wclamp[:], float(PIX))
    wsp_ps = ps.tile([1, C], F32, tag="ps")
    nc.tensor.transpose(wsp_ps[:], wsp_col[:], id128[:C, :C])
    wsp_row = cpm.tile([1, C], F32, tag="wsp_row")
    nc.vector.tensor_copy(wsp_row[:], wsp_ps[:])

    ys_ps = ps.tile([C, 1], F32, tag="ps")
    nc.tensor.matmul(ys_ps[:], y_sb[:], ones_col[:B, :])
    present = cpm.tile([C, 1], F32, tag="present")
    nc.vector.tensor_single_scalar(present[:], ys_ps[:], 0.5, ALU.is_gt)

    # proto row norms via Square+accum (proto left unnormalized, scales folded)
    nrm_part = cpe.tile([C, 4], F32, tag="nrm_part")
    sq_scr = cpe.tile([C, 512], F32, tag="sq_scr")
    for j in range(4):
        pp_ = psacc.tile([C, 512], F32, tag="acc")
        nc.tensor.matmul(pp_[:], probs[:], pooled_full[:, 512 * j:512 * (j + 1)])
        nc.scalar.activation(
            sq_scr[:], pp_[:], ACTF.Square, accum_out=nrm_part[:, j:j + 1]
        )
    nrm_sq = cpm.tile([C, 1], F32, tag="nrm_sq")
    nc.vector.tensor_reduce(nrm_sq[:], nrm_part[:], axis=AX.X, op=ALU.add)
    nrm = cpm.tile([C, 1], F32, tag="nrm")
    nc.scalar.activation(nrm[:], nrm_sq[:], ACTF.Ln)
    nc.scalar.activation(nrm[:], nrm[:], ACTF.Exp, scale=0.5)  # sqrt
    nc.vector.tensor_scalar_max(nrm[:], nrm[:], EPS)
    inv_u = cpm.tile([C, 1], F32, tag="inv_u")
    nc.vector.reciprocal(inv_u[:], nrm[:])

    # protoT (f-major), G, cos
    protoT = cpe.tile([128, KF, C], F32, tag="protoT")
    for k in range(KF):
        pt_ = ps.tile([128, C], F32, tag="ps")
        nc.tensor.matmul(pt_[:], pooled_full[:, 128 * k:128 * (k + 1)], probs[:])
        if k % 2 == 0:
            nc.vector.tensor_copy(protoT[:, k, :], pt_[:])
        else:
            nc.scalar.copy(protoT[:, k, :], pt_[:])
    g_ps = psacc.tile([C, C], F32, tag="acc")
    for k in range(KF):
        nc.tensor.matmul(
            g_ps[:], protoT[:, k, :], protoT[:, k, :],
            start=(k == 0), stop=(k == KF - 1),
        )
    ir_ps = ps.tile([1, C], F32, tag="ps")
    nc.tensor.transpose(ir_ps[:], inv_u[:], id128[:C, :C])
    inv_row = cpm.tile([1, C], F32, tag="inv_row")
    nc.vector.tensor_copy(inv_row[:], ir_ps[:])
    s_ps = ps.tile([C, C], F32, tag="ps")
    nc.tensor.matmul(s_ps[:], inv_row[:], inv_row[:])
    cos_sb = cpm.tile([C, C], F32, tag="cos_sb")
    nc.scalar.copy(cos_sb[:], g_ps[:])
    nc.vector.tensor_tensor(cos_sb[:], cos_sb[:], s_ps[:], ALU.mult)
    nc.vector.tensor_scalar(cos_sb[:], cos_sb[:], 1.0, -1.0, ALU.min, ALU.max)

    # Hn1 = relu((proto_u @ pp_w1 + pp_b1*wsum*PIX) * winv_eff)
    h1_ps = psacc.tile([C, HID], F32, tag="acc")
    for k in range(KF):
        nc.tensor.matmul(
            h1_ps[:], protoT[:, k, :], pp_w1_sb[:, k, :],
            start=(k == 0), stop=False,
        )
    nc.tensor.matmul(h1_ps[:], wsp_row[:], b_pp1[:], start=False, stop=True)
    relu1 = cpe.tile([C, HID], F32, tag="relu1")
    nc.vector.tensor_scalar_mul(relu1[:], h1_ps[:], winv_eff[:])
    nc.scalar.activation(relu1[:], relu1[:], ACTF.Relu)

    r1T = cpm.tile([128, 2, C], F32, tag="r1T")
    for j in range(2):
        tp_ = ps.tile([128, C], F32, tag="ps")
        nc.tensor.transpose(tp_[:], relu1[:, 128 * j:128 * (j + 1)], id128[:C, :C])
        nc.scalar.copy(r1T[:, j, :], tp_[:])

    # release pp_w1 + early tensors (LIFO); open late pools
    cpe_cm.__exit__(None, None, None)
    w1pool_cm.__exit__(None, None, None)

    cpl_cm = tc.tile_pool(name="cpl", bufs=1)
    cpl = cpl_cm.__enter__()
    awpool_cm = tc.tile_pool(name="awpool", bufs=1)
    awpool = awpool_cm.__enter__()
    alpha_w_sb = awpool.tile([128, 2, F], F32, tag="alpha_w_sb")
    nc.sync.dma_start(alpha_w_sb[:], t_alpha_w.ap().rearrange("(k p) h -> p k h", p=128))

    def transpose_pair(src, tag="tpair"):
        # src [C, 256] -> dst [128, 2, C]
        dst = cpl.tile([128, 2, C], F32, tag=tag)
        for j in range(2):
            tp_ = ps.tile([128, C], F32, tag="ps")
            nc.tensor.transpose(tp_[:], src[:, 128 * j:128 * (j + 1)], id128[:C, :C])
            nc.scalar.copy(dst[:, j, :], tp_[:])
        return dst

    hn_ps = psacc.tile([C, HID], F32, tag="acc")
    for j in range(2):
        nc.tensor.matmul(hn_ps[:], r1T[:, j, :], pp_w2_sb[:, j, :], start=(j == 0), stop=False)
    nc.tensor.matmul(hn_ps[:], ones_row[:1, :C], b_pp2[:], start=False, stop=True)
    hn_sb = cpl.tile([C, HID], F32, tag="hn_sb")
    nc.scalar.copy(hn_sb[:], hn_ps[:])

    hnT = transpose_pair(hn_sb)
    zd_ps = psacc.tile([C, EDD], F32, tag="acc")
    for j in range(2):
        nc.tensor.matmul(zd_ps[:], hnT[:, j, :], ed_w_sb[:, j, :], start=(j == 0), stop=False)
    nc.tensor.matmul(zd_ps[:], ones_row[:1, :C], b_ed[:], start=False, stop=True)
    zd_sb = cpl.tile([C, EDD], F32, tag="zd_sb")
    nc.scalar.activation(zd_sb[:], zd_ps[:], ACTF.Relu)

    zdT_ps = ps.tile([EDD, C], F32, tag="ps")
    nc.tensor.transpose(zdT_ps[:], zd_sb[:], id128[:C, :C])
    zdT = cpl.tile([EDD, C], F32, tag="zdT")
    nc.vector.tensor_copy(zdT[:], zdT_ps[:])

    # local (sharded) pieces via iselT
    zdl_ps = ps.tile([IL, EDD], F32, tag="ps")
    nc.tensor.matmul(zdl_ps[:], isel_sb[:], zd_sb[:])
    zdl = cpl.tile([IL, EDD], F32, tag="zdl")
    nc.vector.tensor_copy(zdl[:], zdl_ps[:])
    zdTl_ps = ps.tile([EDD, IL], F32, tag="ps")
    nc.tensor.transpose(zdTl_ps[:], zdl[:], id128[:IL, :IL])
    zdTl = cpl.tile([EDD, IL], F32, tag="zdTl")
    nc.vector.tensor_copy(zdTl[:], zdTl_ps[:])

    fl_ps = ps.tile([IL, 1], F32, tag="ps")
    nc.tensor.matmul(fl_ps[:], isel_sb[:], freq_col[:])
    fl_sb = cpl.tile([IL, 1], F32, tag="fl_sb")
    nc.vector.tensor_copy(fl_sb[:], fl_ps[:])
    flr_ps = ps.tile([1, IL], F32, tag="ps")
    nc.tensor.transpose(flr_ps[:], fl_sb[:], id128[:IL, :IL])
    flr = cpl.tile([1, IL], F32, tag="flr")
    nc.vector.tensor_copy(flr[:], flr_ps[:])
    fr_ps = ps.tile([1, C], F32, tag="ps")
    nc.tensor.transpose(fr_ps[:], freq_col[:], id128[:C, :C])
    fr_sb = cpl.tile([1, C], F32, tag="fr_sb")
    nc.vector.tensor_copy(fr_sb[:], fr_ps[:])

    # PT_loc (+ freq_i term), QT (+ freq_j term)
    ptl_ps = ps.tile([128, IL], F32, tag="ps")
    nc.tensor.matmul(ptl_ps[:], ew1_a[:], zdTl[:], start=True, stop=False)
    nc.tensor.matmul(ptl_ps[:], ew1_fi[:], flr[:], start=False, stop=True)
    ptl = cpl.tile([128, IL], F32, tag="ptl")
    nc.vector.tensor_copy(ptl[:], ptl_ps[:])
    qt_ps = ps.tile([128, C], F32, tag="ps")
    nc.tensor.matmul(qt_ps[:], ew1_b[:], zdT[:], start=True, stop=False)
    nc.tensor.matmul(qt_ps[:], ew1_fj[:], fr_sb[:], start=False, stop=True)
    qt = cpl.tile([128, C], F32, tag="qt")
    nc.scalar.copy(qt[:], qt_ps[:])
    ptq = cpl.tile([128, IL, C], F16, tag="ptq")
    nc.vector.tensor_tensor(
        ptq[:],
        ptl[:].unsqueeze(2).broadcast_to([128, IL, C]),
        qt[:].unsqueeze(1).broadcast_to([128, IL, C]),
        ALU.add,
    )

    cosl_ps = ps.tile([IL, C], F32, tag="ps")
    nc.tensor.matmul(cosl_ps[:], isel_sb[:], cos_sb[:])
    cosl16 = cpl.tile([IL, C], F16, tag="cosl16")
    nc.scalar.copy(cosl16[:], cosl_ps[:])
    cos_flat = cpl.tile([1, IL * C], F16, tag="cos_flat")
    nc.sync.dma_start(cos_flat[:], cosl16[:])

    # edge feature blocks: |zi-zj| (in place) and zi*zj, fp16
    absblk = cpl.tile([EDD, IL, C], F16, tag="absblk")
    nc.vector.tensor_tensor(
        absblk[:],
        zdTl[:].unsqueeze(2).broadcast_to([EDD, IL, C]),
        zdT[:].unsqueeze(1).broadcast_to([EDD, IL, C]),
        ALU.subtract,
    )
    nc.scalar.activation(absblk[:], absblk[:], ACTF.Abs)
    prodblk = cpl.tile([EDD, IL, C], F16, tag="prodblk")
    nc.vector.tensor_tensor(
        prodblk[:],
        zdTl[:].unsqueeze(2).broadcast_to([EDD, IL, C]),
        zdT[:].unsqueeze(1).broadcast_to([EDD, IL, C]),
        ALU.mult,
    )

    nedge = IL * C
    abs_flat = absblk[:].rearrange("p r j -> p (r j)")
    prod_flat = prodblk[:].rearrange("p r j -> p (r j)")
    ptqf = ptq[:].rearrange("p r j -> p (r j)")

    e1T = cpl.tile([128, nedge], F16, tag="e1T")
    for n0, n1 in ((0, 512), (512, nedge)):
        e1_ps = psacc.tile([128, 512], F32, tag="acc")
        nc.tensor.matmul(e1_ps[:, :n1 - n0], ew1_abs16[:], abs_flat[:, n0:n1], start=True, stop=False)
        nc.tensor.matmul(e1_ps[:, :n1 - n0], ew1_prod16[:], prod_flat[:, n0:n1], start=False, stop=False)
        nc.tensor.matmul(e1_ps[:, :n1 - n0], ew1_cos16[:], cos_flat[:, n0:n1], start=False, stop=True)
        nc.vector.tensor_tensor(e1_ps[:, :n1 - n0], e1_ps[:, :n1 - n0], ptqf[:, n0:n1], ALU.add)
        nc.scalar.activation(e1T[:, n0:n1], e1_ps[:, :n1 - n0], ACTF.Relu, bias=b_em1[:])

    e2T = cpl.tile([EDD, nedge], F16, tag="e2T")
    for n0, n1 in ((0, 512), (512, nedge)):
        e2_ps = psacc.tile([EDD, 512], F32, tag="acc")
        nc.tensor.matmul(e2_ps[:, :n1 - n0], em_w2_16[:], e1T[:, n0:n1])
        nc.scalar.activation(e2T[:, n0:n1], e2_ps[:, :n1 - n0], ACTF.Relu, bias=b_em2[:])

    r_sb = cpl.tile([1, nedge], F32, tag="r_sb")
    for n0, n1 in ((0, 512), (512, nedge)):
        r_ps = ps.tile([1, 512], F32, tag="ps")
        nc.tensor.matmul(r_ps[:, :n1 - n0], em_w3_16[:], e2T[:, n0:n1])
        nc.scalar.activation(r_sb[:, n0:n1], r_ps[:, :n1 - n0], ACTF.Identity, bias=b_em3[:])

    r_loc_d = dram.tile([IL, C], F32)
    r_full_d = dram.tile([C, C], F32)
    nc.sync.dma_start(r_loc_d[:], r_sb[:])
    nc.gpsimd.collective_compute(
        "AllGather", ALU.bypass,
        replica_groups=[list(range(N_CORES))],
        ins=[r_loc_d.opt()], outs=[r_full_d.opt()],
    )
    r_full = cpl.tile([C, C], F32, tag="r_full")
    nc.sync.dma_start(r_full[:], r_full_d[:])

    # W_adj
    h_sb = cpl.tile([C, C], F32, tag="h_sb")
    nc.vector.scalar_tensor_tensor(h_sb[:], prior_sb[:], 1.0 / TEMP, r_full[:], ALU.mult, ALU.add)
    w_sg = cpl.tile([C, C], F32, tag="w_sg")
    sigmoid(w_sg[:], h_sb[:])
    wt_ps = ps.tile([C, C], F32, tag="ps")
    nc.tensor.transpose(wt_ps[:], w_sg[:], id128[:C, :C])
    w_half = cpl.tile([C, C], F32, tag="w_half")
    nc.scalar.mul(w_half[:], wt_ps[:], 0.5)
    nc.vector.scalar_tensor_tensor(w_half[:], w_sg[:], 0.5, w_half[:], ALU.mult, ALU.add)
    wadj = cpl.tile([C, C], F32, tag="wadj")
    nc.vector.tensor_mul(wadj[:], w_half[:], offd[:])
    nc.sync.dma_start(o_wadj[:, :], wadj[:])

    rowsum = cpl.tile([C, 1], F32, tag="rowsum")
    nc.vector.tensor_reduce(rowsum[:], wadj[:], axis=AX.X, op=ALU.add)
    nc.vector.tensor_scalar_max(rowsum[:], rowsum[:], EPS)
    rinv = cpl.tile([C, 1], F32, tag="rinv")
    nc.vector.reciprocal(rinv[:], rowsum[:])
    rr_ps = ps.tile([1, C], F32, tag="ps")
    nc.tensor.transpose(rr_ps[:], rinv[:], id128[:C, :C])
    rinv_row = cpl.tile([1, C], F32, tag="rinv_row")
    nc.vector.tensor_copy(rinv_row[:], rr_ps[:])
    rb_ps = ps.tile([C, C], F32, tag="ps")
    nc.tensor.matmul(rb_ps[:], ones_row[:1, :C], rinv_row[:])
    at_sb = cpl.tile([C, C], F32, tag="at_sb")
    nc.vector.tensor_tensor(at_sb[:], wadj[:], rb_ps[:], ALU.mult)

    # message passing (1 step)
    m1T = cpl.tile([128, 2, C], F32, tag="tpair")
    for j in range(2):
        mp_ = ps.tile([128, C], F32, tag="ps")
        nc.tensor.matmul(mp_[:], hn_sb[:, 128 * j:128 * (j + 1)], at_sb[:])
        nc.scalar.copy(m1T[:, j, :], mp_[:])
    m1_ps = psacc.tile([C, HID], F32, tag="acc")
    for j in range(2):
        nc.tensor.matmul(m1_ps[:], m1T[:, j, :], msg_w1_sb[:, j, :], start=(j == 0), stop=False)
    nc.tensor.matmul(m1_ps[:], ones_row[:1, :C], b_m1[:], start=False, stop=True)
    mr_sb = cpl.tile([C, HID], F32, tag="mr_sb")
    nc.scalar.activation(mr_sb[:], m1_ps[:], ACTF.Relu)
    mrT = transpose_pair(mr_sb)
    m2_ps = psacc.tile([C, HID], F32, tag="acc")
    for j in range(2):
        nc.tensor.matmul(m2_ps[:], mrT[:, j, :], msg_w2_sb[:, j, :], start=(j == 0), stop=False)
    nc.tensor.matmul(m2_ps[:], ones_row[:1, :C], b_m2[:], start=False, stop=True)
    zn_sb = cpl.tile([C, HID], F32, tag="zn_sb")
    nc.vector.tensor_tensor(zn_sb[:], m2_ps[:], hn_sb[:], ALU.add)
    nc.scalar.activation(zn_sb[:], zn_sb[:], ACTF.Relu)
    znT = transpose_pair(zn_sb, tag="znT")

    # aw head
    aw_sb = cpl.tile([C, F], F32, tag="aw_sb")
    aws_part = cpl.tile([C, 4], F32, tag="aws_part")
    for n in range(4):
        a_ps = psacc.tile([C, 512], F32, tag="acc")
        for j in range(2):
            nc.tensor.matmul(
                a_ps[:], znT[:, j, :], alpha_w_sb[:, j, 512 * n:512 * (n + 1)],
                start=(j == 0), stop=False,
            )
        nc.tensor.matmul(a_ps[:], ones_row16[:1, :C], b_alpha[:, 512 * n:512 * (n + 1)], start=False, stop=True)
        # softplus(x) = ln(exp(x) + 1)
        nc.scalar.activation(aw_sb[:, 512 * n:512 * (n + 1)], a_ps[:], ACTF.Exp)
        nc.scalar.activation(
            aw_sb[:, 512 * n:512 * (n + 1)], aw_sb[:, 512 * n:512 * (n + 1)],
            ACTF.Ln, bias=1.0,
            accum_out=aws_part[:, n:n + 1],
        )
    aws = cpl.tile([C, 1], F32, tag="aws")
    nc.vector.tensor_reduce(aws[:], aws_part[:], axis=AX.X, op=ALU.add)
    nc.vector.tensor_scalar_max(aws[:], aws[:], EPS)
    sinv = cpl.tile([C, 1], F32, tag="sinv")
    nc.vector.reciprocal(sinv[:], aws[:])
    nc.vector.tensor_scalar_mul(aw_sb[:], aw_sb[:], sinv[:])
    nc.sync.dma_start(o_aw[:, :], aw_sb[:])

    awT16 = cpl.tile([128, KF, C], F16, tag="awT16")
    for k in range(KF):
        at_ps = ps.tile([128, C], F32, tag="ps")
        nc.tensor.transpose(at_ps[:], aw_sb[:, 128 * k:128 * (k + 1)], id128[:C, :C])
        nc.scalar.copy(awT16[:, k, :], at_ps[:])

    awpool_cm.__exit__(None, None, None)

    # dlog
    dl_ps = ps.tile([C, 1], F32, tag="ps")
    for j in range(2):
        nc.tensor.matmul(dl_ps[:], znT[:, j, :], bias_w_sb[:, j, :], start=(j == 0), stop=False)
    nc.tensor.matmul(dl_ps[:], ones_row[:1, :C], b_biasb[:], start=False, stop=True)
    dlog_sb = cpl.tile([C, 1], F32, tag="dlog_sb")
    nc.scalar.copy(dlog_sb[:], dl_ps[:])
    nc.sync.dma_start(o_dlog.ap().rearrange("(p o) -> p o", o=1), dlog_sb[:])
    dlr_ps = ps.tile([1, C], F32, tag="ps")
    nc.tensor.transpose(dlr_ps[:], dlog_sb[:], id128[:C, :C])
    dlr = cpl.tile([1, C], F32, tag="dlr")
    nc.vector.tensor_copy(dlr[:], dlr_ps[:])

    # refined
    py = cpl.tile([B, C], F32, tag="py")
    nc.vector.tensor_mul(py[:], probs[:], y_sb[:])
    pyT_ps = ps.tile([C, B], F32, tag="ps")
    nc.tensor.transpose(pyT_ps[:], py[:], id128[:B, :B])
    pyT = cpl.tile([C, B], F32, tag="pyT")
    nc.vector.tensor_copy(pyT[:], pyT_ps[:])
    pa_ps = ps.tile([B, C], F32, tag="ps")
    nc.tensor.matmul(pa_ps[:], pyT[:], wadj[:])
    nc.vector.tensor_mul(py[:], y_sb[:], pa_ps[:])  # py := pos_agg
    dlb_ps = ps.tile([B, C], F32, tag="ps")
    nc.tensor.matmul(dlb_ps[:], ones_row[:1, :B], dlr[:])
    prow = cpl.tile([B, 1], F32, tag="prow")
    nc.vector.tensor_reduce(prow[:], probs[:], axis=AX.X, op=ALU.add)
    nc.scalar.mul(prow[:], prow[:], -GAMMA_NEG)
    refined = cpl.tile([B, C], F32, tag="refined")
    nc.vector.scalar_tensor_tensor(
        refined[:], py[:], BETA_POS + GAMMA_NEG, dlb_ps[:], ALU.mult, ALU.add
    )
    nc.vector.tensor_scalar_add(refined[:], refined[:], prow[:])
    nc.vector.tensor_add(refined[:], refined[:], cls_sb[:])
    nc.sync.dma_start(o_refined[:, :], refined[:])

    # ---------------- phase D: CAM ----------------
    dpool_cm = tc.tile_pool(name="dpool", bufs=2)
    dpool = dpool_cm.__enter__()
    for b in range(BL):
        cam = dpool.tile([C, PIX], F32, tag="cam")
        for hh in range(2):
            c_ps = pscam.tile([C, 512], F32, tag="cam_ps")
            for k in range(KF):
                nc.tensor.matmul(
                    c_ps[:], awT16[:, k, :], stash[:, b, k, 512 * hh:512 * (hh + 1)],
                    start=(k == 0), stop=(k == KF - 1),
                )
            nc.scalar.activation(cam[:, 512 * hh:512 * (hh + 1)], c_ps[:], ACTF.Relu)
        mn = dpool.tile([C, 1], F32, tag="mn")
        nc.vector.tensor_reduce(mn[:], cam[:], axis=AX.X, op=ALU.min)
        mx = dpool.tile([C, 1], F32, tag="mx")
        nc.vector.tensor_reduce(mx[:], cam[:], axis=AX.X, op=ALU.max)
        nc.vector.tensor_sub(mx[:], mx[:], mn[:])
        nc.vector.tensor_scalar_add(mx[:], mx[:], EPS)
        dinv = dpool.tile([C, 1], F32, tag="dinv")
        nc.vector.reciprocal(dinv[:], mx[:])
        nc.vector.tensor_scalar(cam[:], cam[:], mn[:], dinv[:], ALU.subtract, ALU.mult)
        nc.sync.dma_start(o_cam[b, :, :], cam[:])

    # ---------------- losses ----------------
    def colsum_1x1(src_col, nrows, tag):
        p_ = ps.tile([1, 1], F32, tag="ps")
        nc.tensor.matmul(p_[:], src_col[:], ones_col[:nrows, :])
        out = cpl.tile([1, 1], F32, tag=tag)
        nc.vector.tensor_copy(out[:], p_[:])
        return out

    # cls loss: bce = relu(h) - h*t + softplus(-|h|), masked mean
    m_sb = cpl.tile([B, C], F32, tag="m_sb")
    nc.vector.tensor_single_scalar(m_sb[:], tgt_sb[:], -1.0, ALU.not_equal)
    safe_t = cpl.tile([B, C], F32, tag="safe_t")
    nc.vector.tensor_mul(safe_t[:], tgt_sb[:], m_sb[:])
    nc.vector.tensor_mul(safe_t[:], refined[:], safe_t[:])  # safe_t := h*t
    rh = cpl.tile([B, C], F32, tag="rh")
    nc.scalar.activation(rh[:], refined[:], ACTF.Relu)
    ab = cpl.tile([B, C], F32, tag="ab")
    nc.scalar.activation(ab[:], refined[:], ACTF.Abs)
    softplus_neg(ab[:], ab[:])  # ab := softplus(-|h|)
    nc.vector.tensor_sub(rh[:], rh[:], safe_t[:])
    nc.vector.tensor_add(rh[:], rh[:], ab[:])
    nc.vector.tensor_mul(rh[:], rh[:], m_sb[:])  # rh := bce * m
    bm_rows = cpl.tile([B, 1], F32, tag="bm_rows")
    nc.vector.tensor_reduce(bm_rows[:], rh[:], axis=AX.X, op=ALU.add)
    bce_sum = colsum_1x1(bm_rows, B, "bce_sum")
    nc.vector.tensor_reduce(bm_rows[:], m_sb[:], axis=AX.X, op=ALU.add)
    m_sum = colsum_1x1(bm_rows, B, "m_sum")
    nc.vector.tensor_scalar_max(m_sum[:], m_sum[:], 1.0)
    m_inv = cpl.tile([1, 1], F32, tag="m_inv")
    nc.vector.reciprocal(m_inv[:], m_sum[:])
    cls_loss = cpl.tile([1, 1], F32, tag="cls_loss")
    nc.vector.tensor_mul(cls_loss[:], bce_sum[:], m_inv[:])

    # edge mask
    pr_ps = ps.tile([1, C], F32, tag="ps")
    nc.tensor.transpose(pr_ps[:], present[:], id128[:C, :C])
    pres_row = cpl.tile([1, C], F32, tag="pres_row")
    nc.vector.tensor_copy(pres_row[:], pr_ps[:])
    ppo_ps = ps.tile([C, C], F32, tag="ps")
    nc.tensor.matmul(ppo_ps[:], pres_row[:], pres_row[:])
    emf = cpl.tile([C, C], F32, tag="emf")
    nc.vector.tensor_tensor(emf[:], offd[:], ppo_ps[:], ALU.mult)

    # edge bce over h vs soft targets t = sigmoid(prior/TEMP)
    te_sb = cpl.tile([C, C], F32, tag="te_sb")
    sigmoid(te_sb[:], prior_sb[:], scale=1.0 / TEMP)
    nc.vector.tensor_mul(te_sb[:], h_sb[:], te_sb[:])  # te_sb := h*t
    rhe = cpl.tile([C, C], F32, tag="rhe")
    nc.scalar.activation(rhe[:], h_sb[:], ACTF.Relu)
    abe = cpl.tile([C, C], F32, tag="abe")
    nc.scalar.activation(abe[:], h_sb[:], ACTF.Abs)
    softplus_neg(abe[:], abe[:])
    nc.vector.tensor_sub(rhe[:], rhe[:], te_sb[:])
    nc.vector.tensor_add(rhe[:], rhe[:], abe[:])   # rhe := bce_e
    nc.vector.tensor_mul(rhe[:], rhe[:], emf[:])   # rhe := bce_e * emf

    posf = cpl.tile([C, C], F32, tag="posf")
    nc.vector.tensor_single_scalar(posf[:], prior_sb[:], 0.0, ALU.is_gt)

    rcol = cpl.tile([C, 1], F32, tag="rcol")
    nc.vector.tensor_reduce(rcol[:], emf[:], axis=AX.X, op=ALU.add)
    n_edges = colsum_1x1(rcol, C, "n_edges")
    nc.vector.tensor_mul(abe[:], posf[:], emf[:])  # abe := posf*emf
    nc.vector.tensor_reduce(rcol[:], abe[:], axis=AX.X, op=ALU.add)
    n_pos = colsum_1x1(rcol, C, "n_pos")
    nc.vector.tensor_reduce(rcol[:], rhe[:], axis=AX.X, op=ALU.add)
    s1 = colsum_1x1(rcol, C, "s1")
    nc.vector.tensor_mul(abe[:], rhe[:], posf[:])  # abe := bce_e*emf*posf
    nc.vector.tensor_reduce(rcol[:], abe[:], axis=AX.X, op=ALU.add)
    s2 = colsum_1x1(rcol, C, "s2")

    nc.scalar.activation(abe[:], r_full[:], ACTF.Abs)
    nc.vector.tensor_mul(abe[:], abe[:], emf[:])   # abe := |r|*emf
    nc.vector.tensor_reduce(rcol[:], abe[:], axis=AX.X, op=ALU.add)
    rr_sum = colsum_1x1(rcol, C, "rr_sum")
    nc.vector.tensor_reduce(rcol[:], wadj[:], axis=AX.X, op=ALU.add)
    wa_sum = colsum_1x1(rcol, C, "wa_sum")

    n_pos_c = cpl.tile([1, 1], F32, tag="n_pos_c")
    nc.vector.tensor_scalar_max(n_pos_c[:], n_pos[:], 1.0)
    n_neg = cpl.tile([1, 1], F32, tag="n_neg")
    nc.vector.tensor_sub(n_neg[:], n_edges[:], n_pos[:])
    nc.vector.tensor_scalar_max(n_neg[:], n_neg[:], 1.0)
    np_inv = cpl.tile([1, 1], F32, tag="np_inv")
    nc.vector.reciprocal(np_inv[:], n_pos_c[:])
    w_pos = cpl.tile([1, 1], F32, tag="w_pos")
    nc.vector.tensor_mul(w_pos[:], n_neg[:], np_inv[:])
    nc.vector.tensor_scalar(w_pos[:], w_pos[:], 1.0, 10.0, ALU.max, ALU.min)
    nc.vector.tensor_scalar_add(w_pos[:], w_pos[:], -1.0)  # w_pos := w_pos - 1

    nc.vector.tensor_scalar_max(n_edges[:], n_edges[:], 1.0)
    ne_inv = cpl.tile([1, 1], F32, tag="ne_inv")
    nc.vector.reciprocal(ne_inv[:], n_edges[:])

    # edge_loss = (s1 + (w_pos-1)*s2) / n_edges ; r_reg = 0.001*rr_sum/n_edges
    el_num = cpl.tile([1, 1], F32, tag="el_num")
    nc.vector.tensor_mul(el_num[:], w_pos[:], s2[:])
    nc.vector.tensor_add(el_num[:], el_num[:], s1[:])
    nc.vector.tensor_mul(el_num[:], el_num[:], ne_inv[:])  # el_num := edge_loss
    r_reg = cpl.tile([1, 1], F32, tag="r_reg")
    nc.vector.tensor_mul(r_reg[:], rr_sum[:], ne_inv[:])

    total = cpl.tile([1, 1], F32, tag="total")
    nc.vector.scalar_tensor_tensor(total[:], el_num[:], 0.1, cls_loss[:], ALU.mult, ALU.add)
    nc.vector.scalar_tensor_tensor(total[:], r_reg[:], 0.001, total[:], ALU.mult, ALU.add)
    nc.vector.scalar_tensor_tensor(total[:], wa_sum[:], 0.01 / (C * C), total[:], ALU.mult, ALU.add)
    nc.sync.dma_start(o_total.ap().rearrange("(p o) -> p o", o=1), total[:])

    dpool_cm.__exit__(None, None, None)
    cpl_cm.__exit__(None, None, None)
    cpm_cm.__exit__(None, None, None)
    wts_cm.__exit__(None, None, None)
    base_cm.__exit__(None, None, None)
    dram_cm.__exit__(None, None, None)
    pscam_cm.__exit__(None, None, None)
    psacc_cm.__exit__(None, None, None)
    ps_cm.__exit__(None, None, None)


_CACHE = {}


def _get_compiled():
    if "nc" in _CACHE:
        return _CACHE["nc"]
    nc = bacc.Bacc("TRN2", target_bir_lowering=False, debug=False, num_devices=N_CORES)
    with tile.TileContext(nc) as tc:
        _build_program(nc, tc)
    nc.compile()
    _CACHE["nc"] = nc
    return nc


def make_in_maps(inputs):
    feats = np.ascontiguousarray(np.asarray(inputs["feats"], np.float32)).reshape(B, F, PIX)
    tgt = np.asarray(inputs["img_labels"]).astype(np.float32)
    shared = {
        "cls_logits": np.asarray(inputs["cls_logits"], np.float32),
        "tgt": tgt,
        "prior_pmi": np.asarray(inputs["prior_pmi"], np.float32),
    }
    for k in ("pp_w1", "pp_w2", "msg_w1", "msg_w2", "ed_w", "em_w1", "em_w2",
              "em_w3", "alpha_w", "bias_w", "pp_b1", "pp_b2", "msg_b1", "msg_b2",
              "ed_b", "em_b1", "em_b2", "em_b3", "alpha_b", "bias_b"):
        shared[k] = np.ascontiguousarray(np.asarray(inputs[k], np.float32))
    shared["em_w3"] = shared["em_w3"].reshape(64, 1)
    shared["bias_w"] = shared["bias_w"].reshape(HID, 1)
    in_maps = []
    for c in range(N_CORES):
        isel = np.zeros((C, IL), np.float32)
        for r in range(IL):
            isel[IL * c + r, r] = 1.0
        m = dict(shared)
        m["feats_l"] = np.ascontiguousarray(feats[BL * c:BL * (c + 1)])
        m["iselT"] = isel
        in_maps.append(m)
    return in_maps


def run(inputs, trace=False):
    nc = _get_compiled()
    res = run_bass_kernel_spmd(
        nc, make_in_maps(inputs), core_ids=list(range(N_CORES)), trace=trace
    )
    r0 = res.results[0]
    cam = np.concatenate([res.results[c]["o_cam"] for c in range(N_CORES)], axis=0)
    out = (
        r0["o_wadj"],
        r0["o_aw"],
        r0["o_dlog"],
        cam.reshape(B, C, H, W),
        r0["o_refined"],
        np.float32(r0["o_total"].reshape(())),
    )
    return out, res


def kernel(**inputs):
    out, _ = run(inputs, trace=False)
    return out


def bench(inputs, iters=12):
    """Time the NEFF with device-resident inputs (no donation, no re-transfer)."""
    import time

    import jax
    import numpy as np_
    from jax.experimental.shard_map import shard_map
    from jax.sharding import Mesh, NamedSharding, PartitionSpec

    from concourse import bass2jax as b2j
    from concourse import mybir as mb

    nc = _get_compiled()
    b2j.install_neuronx_cc_hook()
    partition_name = nc.partition_id_tensor.name if nc.partition_id_tensor else None
    in_names, out_names, out_avals, zero_outs = [], [], [], []
    for alloc in nc.m.functions[0].allocations:
        if not isinstance(alloc, mb.MemoryLocationSet):
            continue
        name = alloc.memorylocations[0].name
        if alloc.kind == "ExternalInput":
            if name != partition_name:
                in_names.append(name)
        elif alloc.kind == "ExternalOutput":
            out_names.append(name)
            shape = tuple(alloc.tensor_shape)
            dtype = mb.dt.np(alloc.dtype)
            out_avals.append(jax.core.ShapedArray(shape, dtype))
            zero_outs.append(np_.zeros(shape, dtype))
    n_params = len(in_names)
    all_in_names = list(in_names) + list(out_names)
    if partition_name is not None:
        all_in_names.append(partition_name)

    def _body(*args):
        operands = list(args)
        if partition_name is not None:
            operands.append(b2j.partition_id_tensor())
        outs = b2j._bass_exec_p.bind(
            *operands,
            out_avals=tuple(out_avals),
            in_names=tuple(all_in_names),
            out_names=tuple(out_names),
            lowering_input_output_aliases=(),
            sim_require_finite=True,
            sim_require_nnan=True,
            nc=nc,
        )
        return tuple(outs)

    devices = jax.devices()[:N_CORES]
    mesh = Mesh(np_.asarray(devices), ("core",))
    n_outs = len(out_names)
    in_specs = (PartitionSpec("core"),) * (n_params + n_outs)
    out_specs = (PartitionSpec("core"),) * n_outs
    sharded = jax.jit(
        shard_map(_body, mesh=mesh, in_specs=in_specs, out_specs=out_specs, check_rep=False),
        keep_unused=True,
    )
    in_maps = make_in_maps(inputs)
    sh = NamedSharding(mesh, PartitionSpec("core"))
    concat_in = [
        jax.device_put(
            np_.concatenate([np_.asarray(in_maps[c][n]) for c in range(N_CORES)], axis=0), sh
        )
        for n in in_names
    ]
    concat_zeros = [
        jax.device_put(np_.zeros((N_CORES * z.shape[0], *z.shape[1:]), z.dtype), sh)
        for z in zero_outs
    ]
    out = sharded(*concat_in, *concat_zeros)
    jax.block_until_ready(out)
    times = []
    for _ in range(iters):
        t0 = time.perf_counter()
        out = sharded(*concat_in, *concat_zeros)
        jax.block_until_ready(out)
        times.append(time.perf_counter() - t0)
    return times, out, out_names


# revision 39
# speedup vs baseline: 1.0982x; 1.0982x over previous
"""CoocGNN Trainium2 kernel: 8-core SPMD, batch-parallel feats + replicated graph.

Contract: kernel(**inputs) takes FULL inputs (as produced by setup_inputs) and
returns the FULL output tuple (W_adj, aw, dlog, cam_vis, refined, total).
"""

import os
import sys

for _p in ("/opt/trn_rl_repo", os.path.expanduser("~/.axon_site/_ro/trn_rl_repo")):
    if os.path.isdir(_p) and _p not in sys.path:
        sys.path.insert(0, _p)

import numpy as np

import concourse.bacc as bacc
import concourse.tile as tile
from concourse import masks, mybir
from concourse.bass_utils import run_bass_kernel_spmd

F32 = mybir.dt.float32
F32R = mybir.dt.float32r
F16 = mybir.dt.float16
AX = mybir.AxisListType
ALU = mybir.AluOpType
ACTF = mybir.ActivationFunctionType

N_CORES = 8
B, C, F, H, W = 32, 80, 2048, 32, 32
PIX = H * W          # 1024
BL = B // N_CORES    # 4 images per core
HID = 256
EDD = 64
IL = C // N_CORES    # 10 edge-rows per core
TEMP = 2.5
BETA_POS = 0.5
GAMMA_NEG = 0.25
KF = F // 128        # 16 f-chunks
EPS = 1e-6


def _build_program(nc, tc):
    ps_cm = tc.tile_pool(name="ps", bufs=3, space="PSUM")
    ps = ps_cm.__enter__()
    psacc_cm = tc.tile_pool(name="psacc", bufs=3, space="PSUM")
    psacc = psacc_cm.__enter__()
    pscam_cm = tc.tile_pool(name="pscam", bufs=2, space="PSUM")
    pscam = pscam_cm.__enter__()
    dram_cm = tc.tile_pool(name="dram", bufs=1, space="DRAM")
    dram = dram_cm.__enter__()
    base_cm = tc.tile_pool(name="base", bufs=1)
    base = base_cm.__enter__()
    wts_cm = tc.tile_pool(name="wts", bufs=1)
    wts = wts_cm.__enter__()

    t_feats = nc.dram_tensor("feats_l", [BL, F, PIX], F32, kind="ExternalInput")
    t_cls = nc.dram_tensor("cls_logits", [B, C], F32, kind="ExternalInput")
    t_tgt = nc.dram_tensor("tgt", [B, C], F32, kind="ExternalInput")
    t_prior = nc.dram_tensor("prior_pmi", [C, C], F32, kind="ExternalInput")
    t_isel = nc.dram_tensor("iselT", [C, IL], F32, kind="ExternalInput")
    t_pp_w1 = nc.dram_tensor("pp_w1", [F, HID], F16, kind="ExternalInput")
    t_pp_w2 = nc.dram_tensor("pp_w2", [HID, HID], F16, kind="ExternalInput")
    t_msg_w1 = nc.dram_tensor("msg_w1", [HID, HID], F16, kind="ExternalInput")
    t_msg_w2 = nc.dram_tensor("msg_w2", [HID, HID], F16, kind="ExternalInput")
    t_ed_w = nc.dram_tensor("ed_w", [HID, EDD], F16, kind="ExternalInput")
    t_em_w1 = nc.dram_tensor("em_w1", [4 * EDD + 3, 128], F32, kind="ExternalInput")
    t_em_w2 = nc.dram_tensor("em_w2", [128, 64], F32, kind="ExternalInput")
    t_em_w3 = nc.dram_tensor("em_w3", [64, 1], F32, kind="ExternalInput")
    t_alpha_w = nc.dram_tensor("alpha_w", [HID, F], F16, kind="ExternalInput")
    t_bias_w = nc.dram_tensor("bias_w", [HID, 1], F16, kind="ExternalInput")
    t_pp_b1 = nc.dram_tensor("pp_b1", [HID], F32, kind="ExternalInput")
    t_pp_b2 = nc.dram_tensor("pp_b2", [HID], F32, kind="ExternalInput")
    t_msg_b1 = nc.dram_tensor("msg_b1", [HID], F32, kind="ExternalInput")
    t_msg_b2 = nc.dram_tensor("msg_b2", [HID], F32, kind="ExternalInput")
    t_ed_b = nc.dram_tensor("ed_b", [EDD], F32, kind="ExternalInput")
    t_em_b1 = nc.dram_tensor("em_b1", [128], F32, kind="ExternalInput")
    t_em_b2 = nc.dram_tensor("em_b2", [64], F32, kind="ExternalInput")
    t_em_b3 = nc.dram_tensor("em_b3", [1], F32, kind="ExternalInput")
    t_alpha_b = nc.dram_tensor("alpha_b", [F], F32, kind="ExternalInput")
    t_bias_b = nc.dram_tensor("bias_b", [1], F32, kind="ExternalInput")

    o_wadj = nc.dram_tensor("o_wadj", [C, C], F32, kind="ExternalOutput")
    o_aw = nc.dram_tensor("o_aw", [C, F], F32, kind="ExternalOutput")
    o_dlog = nc.dram_tensor("o_dlog", [C], F32, kind="ExternalOutput")
    o_refined = nc.dram_tensor("o_refined", [B, C], F32, kind="ExternalOutput")
    o_total = nc.dram_tensor("o_total", [1], F32, kind="ExternalOutput")
    o_cam = nc.dram_tensor("o_cam", [BL, C, PIX], F32, kind="ExternalOutput")

    # ---------------- constants / small inputs ----------------
    id128 = base.tile([128, 128], F32, tag="id128")
    masks.make_identity(nc, id128[:])
    ones_col = base.tile([128, 1], F32, tag="ones_col")
    nc.vector.memset(ones_col[:], 1.0)
    ones_row = base.tile([1, 128], F32, tag="ones_row")
    nc.vector.memset(ones_row[:], 1.0)
    ones_row16 = base.tile([1, 128], F16, tag="ones_row16")
    nc.vector.memset(ones_row16[:], 1.0)
    offd = base.tile([C, C], F32, tag="offd")
    nc.vector.memset(offd[:], 1.0)
    nc.vector.tensor_sub(offd[:], offd[:], id128[:C, :C])

    def load(pool, shape, dram_ap, tag, dt=F32):
        t = pool.tile(shape, dt, tag=tag)
        nc.sync.dma_start(t[:], dram_ap)
        return t

    cls_sb = load(base, [B, C], t_cls[:, :], "cls_sb")
    tgt_sb = load(base, [B, C], t_tgt[:, :], "tgt_sb")
    prior_sb = load(base, [C, C], t_prior[:, :], "prior_sb")
    isel_sb = load(base, [C, IL], t_isel[:, :], "isel_sb")

    pp_w2_sb = load(wts, [128, 2, HID], t_pp_w2.ap().rearrange("(k p) h -> p k h", p=128), "pp_w2_sb", dt=F16)
    msg_w1_sb = load(wts, [128, 2, HID], t_msg_w1.ap().rearrange("(k p) h -> p k h", p=128), "msg_w1_sb", dt=F16)
    msg_w2_sb = load(wts, [128, 2, HID], t_msg_w2.ap().rearrange("(k p) h -> p k h", p=128), "msg_w2_sb", dt=F16)
    ed_w_sb = load(wts, [128, 2, EDD], t_ed_w.ap().rearrange("(k p) h -> p k h", p=128), "ed_w_sb", dt=F16)
    bias_w_sb = load(wts, [128, 2, 1], t_bias_w.ap().rearrange("(k p) h -> p k h", p=128), "bias_w_sb", dt=F16)

    ew1_a = load(wts, [EDD, 128], t_em_w1[0:EDD, :], "ew1_a")
    ew1_b = load(wts, [EDD, 128], t_em_w1[EDD:2 * EDD, :], "ew1_b")
    ew1_fi = load(wts, [1, 128], t_em_w1[4 * EDD + 1:4 * EDD + 2, :], "ew1_fi")
    ew1_fj = load(wts, [1, 128], t_em_w1[4 * EDD + 2:4 * EDD + 3, :], "ew1_fj")
    # fp16 copies for the edge MLP (f32 staging in a short-lived pool)
    ew1_abs16 = wts.tile([EDD, 128], F16, tag="ew1_abs16")
    ew1_prod16 = wts.tile([EDD, 128], F16, tag="ew1_prod16")
    ew1_cos16 = wts.tile([1, 128], F16, tag="ew1_cos16")
    em_w2_16 = wts.tile([128, 64], F16, tag="em_w2_16")
    em_w3_16 = wts.tile([64, 1], F16, tag="em_w3_16")
    b_alpha = wts.tile([1, F], F16, tag="b_alpha")

    b_pp1 = load(wts, [1, HID], t_pp_b1.ap().rearrange("(o h) -> o h", o=1), "b_pp1")
    b_pp2 = load(wts, [1, HID], t_pp_b2.ap().rearrange("(o h) -> o h", o=1), "b_pp2")
    b_m1 = load(wts, [1, HID], t_msg_b1.ap().rearrange("(o h) -> o h", o=1), "b_m1")
    b_m2 = load(wts, [1, HID], t_msg_b2.ap().rearrange("(o h) -> o h", o=1), "b_m2")
    b_ed = load(wts, [1, EDD], t_ed_b.ap().rearrange("(o h) -> o h", o=1), "b_ed")
    b_em1 = load(wts, [128, 1], t_em_b1.ap().rearrange("(p o) -> p o", o=1), "b_em1")
    b_em2 = load(wts, [64, 1], t_em_b2.ap().rearrange("(p o) -> p o", o=1), "b_em2")
    b_em3 = load(wts, [1, 1], t_em_b3.ap().rearrange("(p o) -> p o", o=1), "b_em3")
    b_biasb = load(wts, [1, 1], t_bias_b.ap().rearrange("(p o) -> p o", o=1), "b_biasb")

    # cpm: mid-lived tensors that survive into late phase C
    cpm_cm = tc.tile_pool(name="cpm", bufs=1)
    cpm = cpm_cm.__enter__()

    # pp_w1 lives in its own pool; released after Hn1 so alpha_w can reuse it.
    w1pool_cm = tc.tile_pool(name="w1pool", bufs=1)
    w1pool = w1pool_cm.__enter__()
    pp_w1_sb = w1pool.tile([128, KF, HID], F16, tag="pp_w1_sb")
    nc.sync.dma_start(pp_w1_sb[:], t_pp_w1.ap().rearrange("(k p) h -> p k h", p=128))

    # f32 staging for the fp16 weight copies; freed before phase A pressure
    with tc.tile_pool(name="stg", bufs=1) as stg:
        ew1_abs = load(stg, [EDD, 128], t_em_w1[2 * EDD:3 * EDD, :], "ew1_abs")
        nc.scalar.copy(ew1_abs16[:], ew1_abs[:])
        ew1_prod = load(stg, [EDD, 128], t_em_w1[3 * EDD:4 * EDD, :], "ew1_prod")
        nc.scalar.copy(ew1_prod16[:], ew1_prod[:])
        ew1_cos = load(stg, [1, 128], t_em_w1[4 * EDD:4 * EDD + 1, :], "ew1_cos")
        nc.scalar.copy(ew1_cos16[:], ew1_cos[:])
        em_w2_sb = load(stg, [128, 64], t_em_w2[:, :], "em_w2_sb")
        nc.scalar.copy(em_w2_16[:], em_w2_sb[:])
        em_w3_sb = load(stg, [64, 1], t_em_w3[:, :], "em_w3_sb")
        nc.scalar.copy(em_w3_16[:], em_w3_sb[:])
        b_alpha32 = load(stg, [1, F], t_alpha_b.ap().rearrange("(o h) -> o h", o=1), "b_alpha32")
        nc.scalar.copy(b_alpha[:], b_alpha32[:])

    # ---------------- phase A: stream feats, stash fp16, pool ----------------
    stash = base.tile([128, BL, KF, PIX], F16, tag="stash")
    pooled_sb = base.tile([128, BL, KF], F32, tag="pooled_sb")
    pooled_loc = dram.tile([BL, F], F32)
    pooled_full_d = dram.tile([B, F], F32)

    with tc.tile_pool(name="ina", bufs=4) as ina:
        for b in range(BL):
            for i in range(KF // 2):
                tin = ina.tile([128, 2, PIX], F32, tag="tin")
                nc.sync.dma_start(
                    tin[:],
                    t_feats[b, 256 * i:256 * (i + 1), :].rearrange("(c p) n -> p c n", p=128),
                )
                # cast f32 -> fp16 stash, alternating engines; pooling reads
                # the stash so the in-tile frees right after the cast
                if (b * (KF // 2) + i) % 2 == 0:
                    nc.scalar.copy(stash[:, b, 2 * i:2 * i + 2, :], tin[:])
                else:
                    nc.gpsimd.tensor_copy(stash[:, b, 2 * i:2 * i + 2, :], tin[:])
                nc.vector.tensor_reduce(
                    pooled_sb[:, b, 2 * i:2 * i + 2], stash[:, b, 2 * i:2 * i + 2, :],
                    axis=AX.X, op=ALU.add,
                )
            nc.sync.dma_start(
                pooled_loc[:][b].rearrange("(k p) -> p k", p=128), pooled_sb[:, b, :]
            )

    nc.gpsimd.collective_compute(
        "AllGather", ALU.bypass,
        replica_groups=[list(range(N_CORES))],
        ins=[pooled_loc.opt()], outs=[pooled_full_d.opt()],
    )

    # ---------------- phase C (early): proto / cos / Hn1 ----------------
    cpe_cm = tc.tile_pool(name="cpe", bufs=1)
    cpe = cpe_cm.__enter__()

    pooled_full = cpe.tile([B, F], F32, tag="pooled_full")
    nc.sync.dma_start(pooled_full[:], pooled_full_d[:])

    def sigmoid(out_ap, in_ap, scale=1.0):
        # 1 / (1 + exp(-x*scale)) via Exp + DVE reciprocal (single ACT table)
        nc.scalar.activation(out_ap, in_ap, ACTF.Exp, scale=-scale)
        nc.vector.tensor_scalar_add(out_ap, out_ap, 1.0)
        nc.vector.reciprocal(out_ap, out_ap)

    def softplus_neg(out_ap, in_ap):
        # log1p(exp(-x)) for x >= 0
        nc.scalar.activation(out_ap, in_ap, ACTF.Exp, scale=-1.0)
        nc.scalar.activation(out_ap, out_ap, ACTF.Ln, bias=1.0)

    probs = base.tile([B, C], F32, tag="probs")
    sigmoid(probs[:], cls_sb[:])
    y_sb = base.tile([B, C], F32, tag="y_sb")
    nc.vector.tensor_scalar_max(y_sb[:], tgt_sb[:], 0.0)
    probs16 = cpe.tile([B, C], F16, tag="probs16")
    nc.scalar.copy(probs16[:], probs[:])
    pooled16 = cpe.tile([B, F], F16, tag="pooled16")
    nc.vector.tensor_copy(pooled16[:], pooled_full[:])

    # weight_sum / freq / present
    ws_ps = ps.tile([C, 1], F32, tag="ps")
    nc.tensor.matmul(ws_ps[:], probs[:], ones_col[:B, :])
    wsum = cpm.tile([C, 1], F32, tag="wsum")
    nc.scalar.copy(wsum[:], ws_ps[:])
    freq_col = cpm.tile([C, 1], F32, tag="freq_col")
    nc.scalar.mul(freq_col[:], wsum[:], 1.0 / B)
    wclamp = cpm.tile([C, 1], F32, tag="wclamp")
    nc.vector.tensor_scalar_max(wclamp[:], wsum[:], EPS)
    winv = cpm.tile([C, 1], F32, tag="winv")
    nc.vector.reciprocal(winv[:], wclamp[:])
    winv_eff = cpm.tile([C, 1], F32, tag="winv_eff")
    nc.scalar.mul(winv_eff[:], winv[:], 1.0 / PIX)
    # row of wsum*PIX (to inject exact pp_b1 under the later winv_eff scaling)
    wsp_col = cpm.tile([C, 1], F32, tag="wsp_col")
    nc.scalar.mul(wsp_col[:], wclamp[:], float(PIX))
    wsp_ps = ps.tile([1, C], F32, tag="ps")
    nc.tensor.transpose(wsp_ps[:], wsp_col[:], id128[:C, :C])
    wsp_row = cpm.tile([1, C], F32, tag="wsp_row")
    nc.vector.tensor_copy(wsp_row[:], wsp_ps[:])

    ys_ps = ps.tile([C, 1], F32, tag="ps")
    nc.tensor.matmul(ys_ps[:], y_sb[:], ones_col[:B, :])
    present = cpm.tile([C, 1], F32, tag="present")
    nc.vector.tensor_single_scalar(present[:], ys_ps[:], 0.5, ALU.is_gt)

    # proto row norms via Square+accum (proto left unnormalized, scales folded)
    nrm_part = cpe.tile([C, 4], F32, tag="nrm_part")
    sq_scr = cpe.tile([C, 512], F32, tag="sq_scr")
    for j in range(4):
        pp_ = psacc.tile([C, 512], F32, tag="acc")
        nc.tensor.matmul(
            pp_[:], probs16[:],
            pooled16[:, 512 * j:512 * (j + 1)],
        )
        nc.scalar.activation(
            sq_scr[:], pp_[:], ACTF.Square, accum_out=nrm_part[:, j:j + 1]
        )
    nrm_sq = cpm.tile([C, 1], F32, tag="nrm_sq")
    nc.vector.tensor_reduce(nrm_sq[:], nrm_part[:], axis=AX.X, op=ALU.add)
    nrm = cpm.tile([C, 1], F32, tag="nrm")
    nc.scalar.activation(nrm[:], nrm_sq[:], ACTF.Ln)
    nc.scalar.activation(nrm[:], nrm[:], ACTF.Exp, scale=0.5)  # sqrt
    nc.vector.tensor_scalar_max(nrm[:], nrm[:], EPS)
    inv_u = cpm.tile([C, 1], F32, tag="inv_u")
    nc.vector.reciprocal(inv_u[:], nrm[:])

    # protoT (f-major), G, cos
    protoT = cpe.tile([128, KF, C], F16, tag="protoT")
    for k in range(KF):
        pt_ = ps.tile([128, C], F32, tag="ps")
        nc.tensor.matmul(pt_[:], pooled_full[:, 128 * k:128 * (k + 1)], probs[:])
        if k % 2 == 0:
            nc.vector.tensor_copy(protoT[:, k, :], pt_[:])
        else:
            nc.scalar.copy(protoT[:, k, :], pt_[:])
    g_ps = psacc.tile([C, C], F32, tag="acc")
    for k in range(KF):
        nc.tensor.matmul(
            g_ps[:], protoT[:, k, :], protoT[:, k, :],
            start=(k == 0), stop=(k == KF - 1),
        )
    ir_ps = ps.tile([1, C], F32, tag="ps")
    nc.tensor.transpose(ir_ps[:], inv_u[:], id128[:C, :C])
    inv_row = cpm.tile([1, C], F32, tag="inv_row")
    nc.vector.tensor_copy(inv_row[:], ir_ps[:])
    s_ps = ps.tile([C, C], F32, tag="ps")
    nc.tensor.matmul(s_ps[:], inv_row[:], inv_row[:])
    cos_sb = cpm.tile([C, C], F32, tag="cos_sb")
    nc.scalar.copy(cos_sb[:], g_ps[:])
    nc.vector.tensor_tensor(cos_sb[:], cos_sb[:], s_ps[:], ALU.mult)
    nc.vector.tensor_scalar(cos_sb[:], cos_sb[:], 1.0, -1.0, ALU.min, ALU.max)

    # Hn1 = relu((proto_u @ pp_w1 + pp_b1*wsum*PIX) * winv_eff)
    h1_ps = psacc.tile([C, HID], F32, tag="acc")
    for k in range(KF):
        nc.tensor.matmul(
            h1_ps[:], protoT[:, k, :], pp_w1_sb[:, k, :],
            start=(k == 0), stop=False,
        )
    nc.tensor.matmul(h1_ps[:], wsp_row[:], b_pp1[:], start=False, stop=True)
    relu1 = cpe.tile([C, HID], F32, tag="relu1")
    nc.vector.tensor_scalar_mul(relu1[:], h1_ps[:], winv_eff[:])
    nc.scalar.activation(relu1[:], relu1[:], ACTF.Relu)

    r1T = cpm.tile([128, 2, C], F16, tag="r1T")
    for j in range(2):
        tp_ = ps.tile([128, C], F32, tag="ps")
        nc.tensor.transpose(tp_[:], relu1[:, 128 * j:128 * (j + 1)], id128[:C, :C])
        nc.scalar.copy(r1T[:, j, :], tp_[:])

    # release pp_w1 + early tensors (LIFO); open late pools
    cpe_cm.__exit__(None, None, None)
    w1pool_cm.__exit__(None, None, None)

    cpl_cm = tc.tile_pool(name="cpl", bufs=1)
    cpl = cpl_cm.__enter__()
    awpool_cm = tc.tile_pool(name="awpool", bufs=1)
    awpool = awpool_cm.__enter__()
    alpha_w_sb = awpool.tile([128, 2, F], F16, tag="alpha_w_sb")
    nc.sync.dma_start(alpha_w_sb[:], t_alpha_w.ap().rearrange("(k p) h -> p k h", p=128))

    def transpose_pair(src, tag="tpair"):
        # src [C, 256] -> dst [128, 2, C]
        dst = cpl.tile([128, 2, C], F16, tag=tag)
        for j in range(2):
            tp_ = ps.tile([128, C], F32, tag="ps")
            nc.tensor.transpose(tp_[:], src[:, 128 * j:128 * (j + 1)], id128[:C, :C])
            nc.scalar.copy(dst[:, j, :], tp_[:])
        return dst

    hn_ps = psacc.tile([C, HID], F32, tag="acc")
    for j in range(2):
        nc.tensor.matmul(
            hn_ps[:], r1T[:, j, :], pp_w2_sb[:, j, :],
            start=(j == 0), stop=False,
        )
    nc.tensor.matmul(hn_ps[:], ones_row[:1, :C], b_pp2[:], start=False, stop=True)
    hn_sb = cpl.tile([C, HID], F32, tag="hn_sb")
    nc.scalar.copy(hn_sb[:], hn_ps[:])

    hnT = transpose_pair(hn_sb)
    zd_ps = psacc.tile([C, EDD], F32, tag="acc")
    for j in range(2):
        nc.tensor.matmul(zd_ps[:], hnT[:, j, :], ed_w_sb[:, j, :], start=(j == 0), stop=False)
    nc.tensor.matmul(zd_ps[:], ones_row[:1, :C], b_ed[:], start=False, stop=True)
    zd_sb = cpl.tile([C, EDD], F32, tag="zd_sb")
    nc.scalar.activation(zd_sb[:], zd_ps[:], ACTF.Relu)

    zdT_ps = ps.tile([EDD, C], F32, tag="ps")
    nc.tensor.transpose(zdT_ps[:], zd_sb[:], id128[:C, :C])
    zdT = cpl.tile([EDD, C], F32, tag="zdT")
    nc.vector.tensor_copy(zdT[:], zdT_ps[:])

    # local (sharded) pieces via iselT
    zdl_ps = ps.tile([IL, EDD], F32, tag="ps")
    nc.tensor.matmul(zdl_ps[:], isel_sb[:], zd_sb[:])
    zdl = cpl.tile([IL, EDD], F32, tag="zdl")
    nc.vector.tensor_copy(zdl[:], zdl_ps[:])
    zdTl_ps = ps.tile([EDD, IL], F32, tag="ps")
    nc.tensor.transpose(zdTl_ps[:], zdl[:], id128[:IL, :IL])
    zdTl = cpl.tile([EDD, IL], F32, tag="zdTl")
    nc.vector.tensor_copy(zdTl[:], zdTl_ps[:])

    fl_ps = ps.tile([IL, 1], F32, tag="ps")
    nc.tensor.matmul(fl_ps[:], isel_sb[:], freq_col[:])
    fl_sb = cpl.tile([IL, 1], F32, tag="fl_sb")
    nc.vector.tensor_copy(fl_sb[:], fl_ps[:])
    flr_ps = ps.tile([1, IL], F32, tag="ps")
    nc.tensor.transpose(flr_ps[:], fl_sb[:], id128[:IL, :IL])
    flr = cpl.tile([1, IL], F32, tag="flr")
    nc.vector.tensor_copy(flr[:], flr_ps[:])
    fr_ps = ps.tile([1, C], F32, tag="ps")
    nc.tensor.transpose(fr_ps[:], freq_col[:], id128[:C, :C])
    fr_sb = cpl.tile([1, C], F32, tag="fr_sb")
    nc.vector.tensor_copy(fr_sb[:], fr_ps[:])

    # PT_loc (+ freq_i term), QT (+ freq_j term)
    ptl_ps = ps.tile([128, IL], F32, tag="ps")
    nc.tensor.matmul(ptl_ps[:], ew1_a[:], zdTl[:], start=True, stop=False)
    nc.tensor.matmul(ptl_ps[:], ew1_fi[:], flr[:], start=False, stop=True)
    ptl = cpl.tile([128, IL], F32, tag="ptl")
    nc.vector.tensor_copy(ptl[:], ptl_ps[:])
    qt_ps = ps.tile([128, C], F32, tag="ps")
    nc.tensor.matmul(qt_ps[:], ew1_b[:], zdT[:], start=True, stop=False)
    nc.tensor.matmul(qt_ps[:], ew1_fj[:], fr_sb[:], start=False, stop=True)
    qt = cpl.tile([128, C], F32, tag="qt")
    nc.scalar.copy(qt[:], qt_ps[:])
    ptq = cpl.tile([128, IL, C], F16, tag="ptq")
    nc.vector.tensor_tensor(
        ptq[:],
        ptl[:].unsqueeze(2).broadcast_to([128, IL, C]),
        qt[:].unsqueeze(1).broadcast_to([128, IL, C]),
        ALU.add,
    )

    cosl_ps = ps.tile([IL, C], F32, tag="ps")
    nc.tensor.matmul(cosl_ps[:], isel_sb[:], cos_sb[:])
    cosl16 = cpl.tile([IL, C], F16, tag="cosl16")
    nc.scalar.copy(cosl16[:], cosl_ps[:])
    cos_flat = cpl.tile([1, IL * C], F16, tag="cos_flat")
    nc.sync.dma_start(cos_flat[:], cosl16[:])

    # edge feature blocks: |zi-zj| (in place) and zi*zj, fp16
    absblk = cpl.tile([EDD, IL, C], F16, tag="absblk")
    nc.vector.tensor_tensor(
        absblk[:],
        zdTl[:].unsqueeze(2).broadcast_to([EDD, IL, C]),
        zdT[:].unsqueeze(1).broadcast_to([EDD, IL, C]),
        ALU.subtract,
    )
    nc.scalar.activation(absblk[:], absblk[:], ACTF.Abs)
    prodblk = cpl.tile([EDD, IL, C], F16, tag="prodblk")
    nc.vector.tensor_tensor(
        prodblk[:],
        zdTl[:].unsqueeze(2).broadcast_to([EDD, IL, C]),
        zdT[:].unsqueeze(1).broadcast_to([EDD, IL, C]),
        ALU.mult,
    )

    nedge = IL * C
    abs_flat = absblk[:].rearrange("p r j -> p (r j)")
    prod_flat = prodblk[:].rearrange("p r j -> p (r j)")
    ptqf = ptq[:].rearrange("p r j -> p (r j)")

    e1T = cpl.tile([128, nedge], F16, tag="e1T")
    for n0, n1 in ((0, 512), (512, nedge)):
        e1_ps = psacc.tile([128, 512], F32, tag="acc")
        nc.tensor.matmul(e1_ps[:, :n1 - n0], ew1_abs16[:], abs_flat[:, n0:n1], start=True, stop=False)
        nc.tensor.matmul(e1_ps[:, :n1 - n0], ew1_prod16[:], prod_flat[:, n0:n1], start=False, stop=False)
        nc.tensor.matmul(e1_ps[:, :n1 - n0], ew1_cos16[:], cos_flat[:, n0:n1], start=False, stop=True)
        nc.vector.tensor_tensor(e1_ps[:, :n1 - n0], e1_ps[:, :n1 - n0], ptqf[:, n0:n1], ALU.add)
        nc.scalar.activation(e1T[:, n0:n1], e1_ps[:, :n1 - n0], ACTF.Relu, bias=b_em1[:])

    e2T = cpl.tile([EDD, nedge], F16, tag="e2T")
    for n0, n1 in ((0, 512), (512, nedge)):
        e2_ps = psacc.tile([EDD, 512], F32, tag="acc")
        nc.tensor.matmul(e2_ps[:, :n1 - n0], em_w2_16[:], e1T[:, n0:n1])
        nc.scalar.activation(e2T[:, n0:n1], e2_ps[:, :n1 - n0], ACTF.Relu, bias=b_em2[:])

    r_sb = cpl.tile([1, nedge], F32, tag="r_sb")
    for n0, n1 in ((0, 512), (512, nedge)):
        r_ps = ps.tile([1, 512], F32, tag="ps")
        nc.tensor.matmul(r_ps[:, :n1 - n0], em_w3_16[:], e2T[:, n0:n1])
        nc.scalar.activation(r_sb[:, n0:n1], r_ps[:, :n1 - n0], ACTF.Identity, bias=b_em3[:])

    r_loc_d = dram.tile([IL, C], F32)
    r_full_d = dram.tile([C, C], F32)
    nc.sync.dma_start(r_loc_d[:], r_sb[:])
    nc.gpsimd.collective_compute(
        "AllGather", ALU.bypass,
        replica_groups=[list(range(N_CORES))],
        ins=[r_loc_d.opt()], outs=[r_full_d.opt()],
    )
    r_full = cpl.tile([C, C], F32, tag="r_full")
    nc.sync.dma_start(r_full[:], r_full_d[:])

    # W_adj
    h_sb = cpl.tile([C, C], F32, tag="h_sb")
    nc.vector.scalar_tensor_tensor(h_sb[:], prior_sb[:], 1.0 / TEMP, r_full[:], ALU.mult, ALU.add)
    w_sg = cpl.tile([C, C], F32, tag="w_sg")
    sigmoid(w_sg[:], h_sb[:])
    wt_ps = ps.tile([C, C], F32, tag="ps")
    nc.tensor.transpose(wt_ps[:], w_sg[:], id128[:C, :C])
    w_half = cpl.tile([C, C], F32, tag="w_half")
    nc.scalar.mul(w_half[:], wt_ps[:], 0.5)
    nc.vector.scalar_tensor_tensor(w_half[:], w_sg[:], 0.5, w_half[:], ALU.mult, ALU.add)
    wadj = cpl.tile([C, C], F32, tag="wadj")
    nc.vector.tensor_mul(wadj[:], w_half[:], offd[:])
    nc.sync.dma_start(o_wadj[:, :], wadj[:])

    rowsum = cpl.tile([C, 1], F32, tag="rowsum")
    nc.vector.tensor_reduce(rowsum[:], wadj[:], axis=AX.X, op=ALU.add)
    nc.vector.tensor_scalar_max(rowsum[:], rowsum[:], EPS)
    rinv = cpl.tile([C, 1], F32, tag="rinv")
    nc.vector.reciprocal(rinv[:], rowsum[:])
    rr_ps = ps.tile([1, C], F32, tag="ps")
    nc.tensor.transpose(rr_ps[:], rinv[:], id128[:C, :C])
    rinv_row = cpl.tile([1, C], F32, tag="rinv_row")
    nc.vector.tensor_copy(rinv_row[:], rr_ps[:])
    rb_ps = ps.tile([C, C], F32, tag="ps")
    nc.tensor.matmul(rb_ps[:], ones_row[:1, :C], rinv_row[:])
    at_sb = cpl.tile([C, C], F32, tag="at_sb")
    nc.vector.tensor_tensor(at_sb[:], wadj[:], rb_ps[:], ALU.mult)

    # message passing (1 step)
    m1T = cpl.tile([128, 2, C], F16, tag="tpair")
    for j in range(2):
        mp_ = ps.tile([128, C], F32, tag="ps")
        nc.tensor.matmul(mp_[:], hn_sb[:, 128 * j:128 * (j + 1)], at_sb[:])
        nc.scalar.copy(m1T[:, j, :], mp_[:])
    m1_ps = psacc.tile([C, HID], F32, tag="acc")
    for j in range(2):
        nc.tensor.matmul(
            m1_ps[:], m1T[:, j, :], msg_w1_sb[:, j, :],
            start=(j == 0), stop=False,
        )
    nc.tensor.matmul(m1_ps[:], ones_row[:1, :C], b_m1[:], start=False, stop=True)
    mr_sb = cpl.tile([C, HID], F32, tag="mr_sb")
    nc.scalar.activation(mr_sb[:], m1_ps[:], ACTF.Relu)
    mrT = transpose_pair(mr_sb)
    m2_ps = psacc.tile([C, HID], F32, tag="acc")
    for j in range(2):
        nc.tensor.matmul(
            m2_ps[:], mrT[:, j, :], msg_w2_sb[:, j, :],
            start=(j == 0), stop=False,
        )
    nc.tensor.matmul(m2_ps[:], ones_row[:1, :C], b_m2[:], start=False, stop=True)
    zn_sb = cpl.tile([C, HID], F32, tag="zn_sb")
    nc.vector.tensor_tensor(zn_sb[:], m2_ps[:], hn_sb[:], ALU.add)
    nc.scalar.activation(zn_sb[:], zn_sb[:], ACTF.Relu)
    znT = transpose_pair(zn_sb, tag="znT")

    # aw head
    aw_sb = cpl.tile([C, F], F32, tag="aw_sb")
    aws_part = cpl.tile([C, 4], F32, tag="aws_part")
    for n in range(4):
        a_ps = psacc.tile([C, 512], F32, tag="acc")
        for j in range(2):
            nc.tensor.matmul(
                a_ps[:], znT[:, j, :],
                alpha_w_sb[:, j, 512 * n:512 * (n + 1)],
                start=(j == 0), stop=False,
            )
        nc.tensor.matmul(a_ps[:], ones_row16[:1, :C], b_alpha[:, 512 * n:512 * (n + 1)], start=False, stop=True)
        # softplus(x) = ln(exp(x) + 1)
        nc.scalar.activation(aw_sb[:, 512 * n:512 * (n + 1)], a_ps[:], ACTF.Exp)
        nc.scalar.activation(
            aw_sb[:, 512 * n:512 * (n + 1)], aw_sb[:, 512 * n:512 * (n + 1)],
            ACTF.Ln, bias=1.0,
            accum_out=aws_part[:, n:n + 1],
        )
    aws = cpl.tile([C, 1], F32, tag="aws")
    nc.vector.tensor_reduce(aws[:], aws_part[:], axis=AX.X, op=ALU.add)
    nc.vector.tensor_scalar_max(aws[:], aws[:], EPS)
    sinv = cpl.tile([C, 1], F32, tag="sinv")
    nc.vector.reciprocal(sinv[:], aws[:])
    nc.vector.tensor_scalar_mul(aw_sb[:], aw_sb[:], sinv[:])
    nc.sync.dma_start(o_aw[:, :], aw_sb[:])

    awT16 = cpl.tile([128, KF, C], F16, tag="awT16")
    for k in range(KF):
        at_ps = ps.tile([128, C], F32, tag="ps")
        nc.tensor.transpose(at_ps[:], aw_sb[:, 128 * k:128 * (k + 1)], id128[:C, :C])
        nc.scalar.copy(awT16[:, k, :], at_ps[:])

    awpool_cm.__exit__(None, None, None)

    # dlog
    dl_ps = ps.tile([C, 1], F32, tag="ps")
    for j in range(2):
        nc.tensor.matmul(dl_ps[:], znT[:, j, :], bias_w_sb[:, j, :], start=(j == 0), stop=False)
    nc.tensor.matmul(dl_ps[:], ones_row[:1, :C], b_biasb[:], start=False, stop=True)
    dlog_sb = cpl.tile([C, 1], F32, tag="dlog_sb")
    nc.scalar.copy(dlog_sb[:], dl_ps[:])
    nc.sync.dma_start(o_dlog.ap().rearrange("(p o) -> p o", o=1), dlog_sb[:])
    dlr_ps = ps.tile([1, C], F32, tag="ps")
    nc.tensor.transpose(dlr_ps[:], dlog_sb[:], id128[:C, :C])
    dlr = cpl.tile([1, C], F32, tag="dlr")
    nc.vector.tensor_copy(dlr[:], dlr_ps[:])

    # refined
    py = cpl.tile([B, C], F32, tag="py")
    nc.vector.tensor_mul(py[:], probs[:], y_sb[:])
    pyT_ps = ps.tile([C, B], F32, tag="ps")
    nc.tensor.transpose(pyT_ps[:], py[:], id128[:B, :B])
    pyT = cpl.tile([C, B], F32, tag="pyT")
    nc.vector.tensor_copy(pyT[:], pyT_ps[:])
    pa_ps = ps.tile([B, C], F32, tag="ps")
    nc.tensor.matmul(pa_ps[:], pyT[:], wadj[:])
    nc.vector.tensor_mul(py[:], y_sb[:], pa_ps[:])  # py := pos_agg
    dlb_ps = ps.tile([B, C], F32, tag="ps")
    nc.tensor.matmul(dlb_ps[:], ones_row[:1, :B], dlr[:])
    prow = cpl.tile([B, 1], F32, tag="prow")
    nc.vector.tensor_reduce(prow[:], probs[:], axis=AX.X, op=ALU.add)
    nc.scalar.mul(prow[:], prow[:], -GAMMA_NEG)
    refined = cpl.tile([B, C], F32, tag="refined")
    nc.vector.scalar_tensor_tensor(
        refined[:], py[:], BETA_POS + GAMMA_NEG, dlb_ps[:], ALU.mult, ALU.add
    )
    nc.vector.tensor_scalar_add(refined[:], refined[:], prow[:])
    nc.vector.tensor_add(refined[:], refined[:], cls_sb[:])
    nc.sync.dma_start(o_refined[:, :], refined[:])

    # ---------------- phase D: CAM ----------------
    dpool_cm = tc.tile_pool(name="dpool", bufs=2)
    dpool = dpool_cm.__enter__()
    for b in range(BL):
        cam = dpool.tile([C, PIX], F32, tag="cam")
        for hh in range(2):
            c_ps = pscam.tile([C, 512], F32, tag="cam_ps")
            for k in range(KF):
                nc.tensor.matmul(
                    c_ps[:], awT16[:, k, :], stash[:, b, k, 512 * hh:512 * (hh + 1)],
                    start=(k == 0), stop=(k == KF - 1),
                )
            nc.scalar.activation(cam[:, 512 * hh:512 * (hh + 1)], c_ps[:], ACTF.Relu)
        mn = dpool.tile([C, 1], F32, tag="mn")
        nc.vector.tensor_reduce(mn[:], cam[:], axis=AX.X, op=ALU.min)
        mx = dpool.tile([C, 1], F32, tag="mx")
        nc.vector.tensor_reduce(mx[:], cam[:], axis=AX.X, op=ALU.max)
        nc.vector.tensor_sub(mx[:], mx[:], mn[:])
        nc.vector.tensor_scalar_add(mx[:], mx[:], EPS)
        dinv = dpool.tile([C, 1], F32, tag="dinv")
        nc.vector.reciprocal(dinv[:], mx[:])
        nc.vector.tensor_scalar(cam[:], cam[:], mn[:], dinv[:], ALU.subtract, ALU.mult)
        nc.sync.dma_start(o_cam[b, :, :], cam[:])

    # ---------------- losses ----------------
    def colsum_1x1(src_col, nrows, tag):
        p_ = ps.tile([1, 1], F32, tag="ps")
        nc.tensor.matmul(p_[:], src_col[:], ones_col[:nrows, :])
        out = cpl.tile([1, 1], F32, tag=tag)
        nc.vector.tensor_copy(out[:], p_[:])
        return out

    # cls loss: bce = relu(h) - h*t + softplus(-|h|), masked mean
    m_sb = cpl.tile([B, C], F32, tag="m_sb")
    nc.vector.tensor_single_scalar(m_sb[:], tgt_sb[:], -1.0, ALU.not_equal)
    safe_t = cpl.tile([B, C], F32, tag="safe_t")
    nc.vector.tensor_mul(safe_t[:], tgt_sb[:], m_sb[:])
    nc.vector.tensor_mul(safe_t[:], refined[:], safe_t[:])  # safe_t := h*t
    rh = cpl.tile([B, C], F32, tag="rh")
    nc.scalar.activation(rh[:], refined[:], ACTF.Relu)
    ab = cpl.tile([B, C], F32, tag="ab")
    nc.scalar.activation(ab[:], refined[:], ACTF.Abs)
    softplus_neg(ab[:], ab[:])  # ab := softplus(-|h|)
    nc.vector.tensor_sub(rh[:], rh[:], safe_t[:])
    nc.vector.tensor_add(rh[:], rh[:], ab[:])
    nc.vector.tensor_mul(rh[:], rh[:], m_sb[:])  # rh := bce * m
    bm_rows = cpl.tile([B, 1], F32, tag="bm_rows")
    nc.vector.tensor_reduce(bm_rows[:], rh[:], axis=AX.X, op=ALU.add)
    bce_sum = colsum_1x1(bm_rows, B, "bce_sum")
    nc.vector.tensor_reduce(bm_rows[:], m_sb[:], axis=AX.X, op=ALU.add)
    m_sum = colsum_1x1(bm_rows, B, "m_sum")
    nc.vector.tensor_scalar_max(m_sum[:], m_sum[:], 1.0)
    m_inv = cpl.tile([1, 1], F32, tag="m_inv")
    nc.vector.reciprocal(m_inv[:], m_sum[:])
    cls_loss = cpl.tile([1, 1], F32, tag="cls_loss")
    nc.vector.tensor_mul(cls_loss[:], bce_sum[:], m_inv[:])

    # edge mask
    pr_ps = ps.tile([1, C], F32, tag="ps")
    nc.tensor.transpose(pr_ps[:], present[:], id128[:C, :C])
    pres_row = cpl.tile([1, C], F32, tag="pres_row")
    nc.vector.tensor_copy(pres_row[:], pr_ps[:])
    ppo_ps = ps.tile([C, C], F32, tag="ps")
    nc.tensor.matmul(ppo_ps[:], pres_row[:], pres_row[:])
    emf = cpl.tile([C, C], F32, tag="emf")
    nc.vector.tensor_tensor(emf[:], offd[:], ppo_ps[:], ALU.mult)

    # edge bce over h vs soft targets t = sigmoid(prior/TEMP)
    te_sb = cpl.tile([C, C], F32, tag="te_sb")
    sigmoid(te_sb[:], prior_sb[:], scale=1.0 / TEMP)
    nc.vector.tensor_mul(te_sb[:], h_sb[:], te_sb[:])  # te_sb := h*t
    rhe = cpl.tile([C, C], F32, tag="rhe")
    nc.scalar.activation(rhe[:], h_sb[:], ACTF.Relu)
    abe = cpl.tile([C, C], F32, tag="abe")
    nc.scalar.activation(abe[:], h_sb[:], ACTF.Abs)
    softplus_neg(abe[:], abe[:])
    nc.vector.tensor_sub(rhe[:], rhe[:], te_sb[:])
    nc.vector.tensor_add(rhe[:], rhe[:], abe[:])   # rhe := bce_e
    nc.vector.tensor_mul(rhe[:], rhe[:], emf[:])   # rhe := bce_e * emf

    posf = cpl.tile([C, C], F32, tag="posf")
    nc.vector.tensor_single_scalar(posf[:], prior_sb[:], 0.0, ALU.is_gt)

    rcol = cpl.tile([C, 1], F32, tag="rcol")
    nc.vector.tensor_reduce(rcol[:], emf[:], axis=AX.X, op=ALU.add)
    n_edges = colsum_1x1(rcol, C, "n_edges")
    nc.vector.tensor_mul(abe[:], posf[:], emf[:])  # abe := posf*emf
    nc.vector.tensor_reduce(rcol[:], abe[:], axis=AX.X, op=ALU.add)
    n_pos = colsum_1x1(rcol, C, "n_pos")
    nc.vector.tensor_reduce(rcol[:], rhe[:], axis=AX.X, op=ALU.add)
    s1 = colsum_1x1(rcol, C, "s1")
    nc.vector.tensor_mul(abe[:], rhe[:], posf[:])  # abe := bce_e*emf*posf
    nc.vector.tensor_reduce(rcol[:], abe[:], axis=AX.X, op=ALU.add)
    s2 = colsum_1x1(rcol, C, "s2")

    nc.scalar.activation(abe[:], r_full[:], ACTF.Abs)
    nc.vector.tensor_mul(abe[:], abe[:], emf[:])   # abe := |r|*emf
    nc.vector.tensor_reduce(rcol[:], abe[:], axis=AX.X, op=ALU.add)
    rr_sum = colsum_1x1(rcol, C, "rr_sum")
    nc.vector.tensor_reduce(rcol[:], wadj[:], axis=AX.X, op=ALU.add)
    wa_sum = colsum_1x1(rcol, C, "wa_sum")

    n_pos_c = cpl.tile([1, 1], F32, tag="n_pos_c")
    nc.vector.tensor_scalar_max(n_pos_c[:], n_pos[:], 1.0)
    n_neg = cpl.tile([1, 1], F32, tag="n_neg")
    nc.vector.tensor_sub(n_neg[:], n_edges[:], n_pos[:])
    nc.vector.tensor_scalar_max(n_neg[:], n_neg[:], 1.0)
    np_inv = cpl.tile([1, 1], F32, tag="np_inv")
    nc.vector.reciprocal(np_inv[:], n_pos_c[:])
    w_pos = cpl.tile([1, 1], F32, tag="w_pos")
    nc.vector.tensor_mul(w_pos[:], n_neg[:], np_inv[:])
    nc.vector.tensor_scalar(w_pos[:], w_pos[:], 1.0, 10.0, ALU.max, ALU.min)
    nc.vector.tensor_scalar_add(w_pos[:], w_pos[:], -1.0)  # w_pos := w_pos - 1

    nc.vector.tensor_scalar_max(n_edges[:], n_edges[:], 1.0)
    ne_inv = cpl.tile([1, 1], F32, tag="ne_inv")
    nc.vector.reciprocal(ne_inv[:], n_edges[:])

    # edge_loss = (s1 + (w_pos-1)*s2) / n_edges ; r_reg = 0.001*rr_sum/n_edges
    el_num = cpl.tile([1, 1], F32, tag="el_num")
    nc.vector.tensor_mul(el_num[:], w_pos[:], s2[:])
    nc.vector.tensor_add(el_num[:], el_num[:], s1[:])
    nc.vector.tensor_mul(el_num[:], el_num[:], ne_inv[:])  # el_num := edge_loss
    r_reg = cpl.tile([1, 1], F32, tag="r_reg")
    nc.vector.tensor_mul(r_reg[:], rr_sum[:], ne_inv[:])

    total = cpl.tile([1, 1], F32, tag="total")
    nc.vector.scalar_tensor_tensor(total[:], el_num[:], 0.1, cls_loss[:], ALU.mult, ALU.add)
    nc.vector.scalar_tensor_tensor(total[:], r_reg[:], 0.001, total[:], ALU.mult, ALU.add)
    nc.vector.scalar_tensor_tensor(total[:], wa_sum[:], 0.01 / (C * C), total[:], ALU.mult, ALU.add)
    nc.sync.dma_start(o_total.ap().rearrange("(p o) -> p o", o=1), total[:])

    dpool_cm.__exit__(None, None, None)
    cpl_cm.__exit__(None, None, None)
    cpm_cm.__exit__(None, None, None)
    wts_cm.__exit__(None, None, None)
    base_cm.__exit__(None, None, None)
    dram_cm.__exit__(None, None, None)
    pscam_cm.__exit__(None, None, None)
    psacc_cm.__exit__(None, None, None)
    ps_cm.__exit__(None, None, None)


_CACHE = {}


def _get_compiled():
    if "nc" in _CACHE:
        return _CACHE["nc"]
    nc = bacc.Bacc("TRN2", target_bir_lowering=False, debug=False, num_devices=N_CORES)
    with tile.TileContext(nc) as tc:
        _build_program(nc, tc)
    nc.compile()
    _CACHE["nc"] = nc
    return nc


def make_in_maps(inputs):
    feats = np.ascontiguousarray(np.asarray(inputs["feats"], np.float32)).reshape(B, F, PIX)
    tgt = np.asarray(inputs["img_labels"]).astype(np.float32)
    shared = {
        "cls_logits": np.asarray(inputs["cls_logits"], np.float32),
        "tgt": tgt,
        "prior_pmi": np.asarray(inputs["prior_pmi"], np.float32),
    }
    for k in ("em_w1", "em_w2", "em_w3", "pp_b1", "pp_b2", "msg_b1", "msg_b2",
              "ed_b", "em_b1", "em_b2", "em_b3", "alpha_b", "bias_b"):
        shared[k] = np.ascontiguousarray(np.asarray(inputs[k], np.float32))
    for k in ("pp_w1", "pp_w2", "msg_w1", "msg_w2", "ed_w", "alpha_w", "bias_w"):
        shared[k] = np.ascontiguousarray(np.asarray(inputs[k], np.float32).astype(np.float16))
    shared["em_w3"] = shared["em_w3"].reshape(64, 1)
    shared["bias_w"] = shared["bias_w"].reshape(HID, 1)
    in_maps = []
    for c in range(N_CORES):
        isel = np.zeros((C, IL), np.float32)
        for r in range(IL):
            isel[IL * c + r, r] = 1.0
        m = dict(shared)
        m["feats_l"] = np.ascontiguousarray(feats[BL * c:BL * (c + 1)])
        m["iselT"] = isel
        in_maps.append(m)
    return in_maps


def run(inputs, trace=False):
    nc = _get_compiled()
    res = run_bass_kernel_spmd(
        nc, make_in_maps(inputs), core_ids=list(range(N_CORES)), trace=trace
    )
    r0 = res.results[0]
    cam = np.concatenate([res.results[c]["o_cam"] for c in range(N_CORES)], axis=0)
    out = (
        r0["o_wadj"],
        r0["o_aw"],
        r0["o_dlog"],
        cam.reshape(B, C, H, W),
        r0["o_refined"],
        np.float32(r0["o_total"].reshape(())),
    )
    return out, res


def kernel(**inputs):
    out, _ = run(inputs, trace=False)
    return out


def bench(inputs, iters=12):
    """Time the NEFF with device-resident inputs (no donation, no re-transfer)."""
    import time

    import jax
    import numpy as np_
    from jax.experimental.shard_map import shard_map
    from jax.sharding import Mesh, NamedSharding, PartitionSpec

    from concourse import bass2jax as b2j
    from concourse import mybir as mb

    nc = _get_compiled()
    b2j.install_neuronx_cc_hook()
    partition_name = nc.partition_id_tensor.name if nc.partition_id_tensor else None
    in_names, out_names, out_avals, zero_outs = [], [], [], []
    for alloc in nc.m.functions[0].allocations:
        if not isinstance(alloc, mb.MemoryLocationSet):
            continue
        name = alloc.memorylocations[0].name
        if alloc.kind == "ExternalInput":
            if name != partition_name:
                in_names.append(name)
        elif alloc.kind == "ExternalOutput":
            out_names.append(name)
            shape = tuple(alloc.tensor_shape)
            dtype = mb.dt.np(alloc.dtype)
            out_avals.append(jax.core.ShapedArray(shape, dtype))
            zero_outs.append(np_.zeros(shape, dtype))
    n_params = len(in_names)
    all_in_names = list(in_names) + list(out_names)
    if partition_name is not None:
        all_in_names.append(partition_name)

    def _body(*args):
        operands = list(args)
        if partition_name is not None:
            operands.append(b2j.partition_id_tensor())
        outs = b2j._bass_exec_p.bind(
            *operands,
            out_avals=tuple(out_avals),
            in_names=tuple(all_in_names),
            out_names=tuple(out_names),
            lowering_input_output_aliases=(),
            sim_require_finite=True,
            sim_require_nnan=True,
            nc=nc,
        )
        return tuple(outs)

    devices = jax.devices()[:N_CORES]
    mesh = Mesh(np_.asarray(devices), ("core",))
    n_outs = len(out_names)
    in_specs = (PartitionSpec("core"),) * (n_params + n_outs)
    out_specs = (PartitionSpec("core"),) * n_outs
    sharded = jax.jit(
        shard_map(_body, mesh=mesh, in_specs=in_specs, out_specs=out_specs, check_rep=False),
        keep_unused=True,
    )
    in_maps = make_in_maps(inputs)
    sh = NamedSharding(mesh, PartitionSpec("core"))
    concat_in = [
        jax.device_put(
            np_.concatenate([np_.asarray(in_maps[c][n]) for c in range(N_CORES)], axis=0), sh
        )
        for n in in_names
    ]
    concat_zeros = [
        jax.device_put(np_.zeros((N_CORES * z.shape[0], *z.shape[1:]), z.dtype), sh)
        for z in zero_outs
    ]
    out = sharded(*concat_in, *concat_zeros)
    jax.block_until_ready(out)
    times = []
    for _ in range(iters):
        t0 = time.perf_counter()
        out = sharded(*concat_in, *concat_zeros)
        jax.block_until_ready(out)
        times.append(time.perf_counter() - t0)
    return times, out, out_names


# revision 41
# speedup vs baseline: 1.1283x; 1.0274x over previous
"""CoocGNN Trainium2 kernel: 8-core SPMD, batch-parallel feats + replicated graph.

Contract: kernel(**inputs) takes FULL inputs (as produced by setup_inputs) and
returns the FULL output tuple (W_adj, aw, dlog, cam_vis, refined, total).
"""

import os
import sys

for _p in ("/opt/trn_rl_repo", os.path.expanduser("~/.axon_site/_ro/trn_rl_repo")):
    if os.path.isdir(_p) and _p not in sys.path:
        sys.path.insert(0, _p)

import numpy as np

import concourse.bacc as bacc
import concourse.tile as tile
from concourse import masks, mybir
from concourse.bass_utils import run_bass_kernel_spmd

F32 = mybir.dt.float32
F32R = mybir.dt.float32r
F16 = mybir.dt.float16
AX = mybir.AxisListType
ALU = mybir.AluOpType
ACTF = mybir.ActivationFunctionType

N_CORES = 8
B, C, F, H, W = 32, 80, 2048, 32, 32
PIX = H * W          # 1024
BL = B // N_CORES    # 4 images per core
HID = 256
EDD = 64
IL = C // N_CORES    # 10 edge-rows per core
TEMP = 2.5
BETA_POS = 0.5
GAMMA_NEG = 0.25
KF = F // 128        # 16 f-chunks
EPS = 1e-6


def _build_program(nc, tc):
    ps_cm = tc.tile_pool(name="ps", bufs=3, space="PSUM")
    ps = ps_cm.__enter__()
    psacc_cm = tc.tile_pool(name="psacc", bufs=3, space="PSUM")
    psacc = psacc_cm.__enter__()
    pscam_cm = tc.tile_pool(name="pscam", bufs=2, space="PSUM")
    pscam = pscam_cm.__enter__()
    dram_cm = tc.tile_pool(name="dram", bufs=1, space="DRAM")
    dram = dram_cm.__enter__()
    base_cm = tc.tile_pool(name="base", bufs=1)
    base = base_cm.__enter__()
    wts_cm = tc.tile_pool(name="wts", bufs=1)
    wts = wts_cm.__enter__()

    t_feats = nc.dram_tensor("feats_l", [BL, F, PIX], F32, kind="ExternalInput")
    t_cls = nc.dram_tensor("cls_logits", [B, C], F32, kind="ExternalInput")
    t_tgt = nc.dram_tensor("tgt", [B, C], F32, kind="ExternalInput")
    t_prior = nc.dram_tensor("prior_pmi", [C, C], F32, kind="ExternalInput")
    t_isel = nc.dram_tensor("iselT", [C, IL], F32, kind="ExternalInput")
    t_pp_w1 = nc.dram_tensor("pp_w1", [F, HID], F16, kind="ExternalInput")
    t_pp_w2 = nc.dram_tensor("pp_w2", [HID, HID], F16, kind="ExternalInput")
    t_msg_w1 = nc.dram_tensor("msg_w1", [HID, HID], F16, kind="ExternalInput")
    t_msg_w2 = nc.dram_tensor("msg_w2", [HID, HID], F16, kind="ExternalInput")
    t_ed_w = nc.dram_tensor("ed_w", [HID, EDD], F16, kind="ExternalInput")
    t_em_w1 = nc.dram_tensor("em_w1", [4 * EDD + 3, 128], F32, kind="ExternalInput")
    t_em_w2 = nc.dram_tensor("em_w2", [128, 64], F32, kind="ExternalInput")
    t_em_w3 = nc.dram_tensor("em_w3", [64, 1], F32, kind="ExternalInput")
    t_alpha_w = nc.dram_tensor("alpha_w", [HID, F], F16, kind="ExternalInput")
    t_bias_w = nc.dram_tensor("bias_w", [HID, 1], F16, kind="ExternalInput")
    t_pp_b1 = nc.dram_tensor("pp_b1", [HID], F32, kind="ExternalInput")
    t_pp_b2 = nc.dram_tensor("pp_b2", [HID], F32, kind="ExternalInput")
    t_msg_b1 = nc.dram_tensor("msg_b1", [HID], F32, kind="ExternalInput")
    t_msg_b2 = nc.dram_tensor("msg_b2", [HID], F32, kind="ExternalInput")
    t_ed_b = nc.dram_tensor("ed_b", [EDD], F32, kind="ExternalInput")
    t_em_b1 = nc.dram_tensor("em_b1", [128], F32, kind="ExternalInput")
    t_em_b2 = nc.dram_tensor("em_b2", [64], F32, kind="ExternalInput")
    t_em_b3 = nc.dram_tensor("em_b3", [1], F32, kind="ExternalInput")
    t_alpha_b = nc.dram_tensor("alpha_b", [F], F32, kind="ExternalInput")
    t_bias_b = nc.dram_tensor("bias_b", [1], F32, kind="ExternalInput")

    o_wadj = nc.dram_tensor("o_wadj", [C, C], F32, kind="ExternalOutput")
    o_aw = nc.dram_tensor("o_aw", [C, F], F32, kind="ExternalOutput")
    o_dlog = nc.dram_tensor("o_dlog", [C], F32, kind="ExternalOutput")
    o_refined = nc.dram_tensor("o_refined", [B, C], F32, kind="ExternalOutput")
    o_total = nc.dram_tensor("o_total", [1], F32, kind="ExternalOutput")
    o_cam = nc.dram_tensor("o_cam", [BL, C, PIX], F32, kind="ExternalOutput")

    # ---------------- constants / small inputs ----------------
    id128 = base.tile([128, 128], F32, tag="id128")
    masks.make_identity(nc, id128[:])
    ones_col = base.tile([128, 1], F32, tag="ones_col")
    nc.vector.memset(ones_col[:], 1.0)
    ones_row = base.tile([1, 128], F32, tag="ones_row")
    nc.vector.memset(ones_row[:], 1.0)
    ones_row16 = base.tile([1, 128], F16, tag="ones_row16")
    nc.vector.memset(ones_row16[:], 1.0)
    offd = base.tile([C, C], F32, tag="offd")
    nc.vector.memset(offd[:], 1.0)
    nc.vector.tensor_sub(offd[:], offd[:], id128[:C, :C])

    def load(pool, shape, dram_ap, tag, dt=F32):
        t = pool.tile(shape, dt, tag=tag)
        nc.sync.dma_start(t[:], dram_ap)
        return t

    cls_sb = load(base, [B, C], t_cls[:, :], "cls_sb")
    tgt_sb = load(base, [B, C], t_tgt[:, :], "tgt_sb")
    prior_sb = load(base, [C, C], t_prior[:, :], "prior_sb")
    isel_sb = load(base, [C, IL], t_isel[:, :], "isel_sb")

    pp_w2_sb = load(wts, [128, 2, HID], t_pp_w2.ap().rearrange("(k p) h -> p k h", p=128), "pp_w2_sb", dt=F16)
    msg_w1_sb = load(wts, [128, 2, HID], t_msg_w1.ap().rearrange("(k p) h -> p k h", p=128), "msg_w1_sb", dt=F16)
    msg_w2_sb = load(wts, [128, 2, HID], t_msg_w2.ap().rearrange("(k p) h -> p k h", p=128), "msg_w2_sb", dt=F16)
    ed_w_sb = load(wts, [128, 2, EDD], t_ed_w.ap().rearrange("(k p) h -> p k h", p=128), "ed_w_sb", dt=F16)
    bias_w_sb = load(wts, [128, 2, 1], t_bias_w.ap().rearrange("(k p) h -> p k h", p=128), "bias_w_sb", dt=F16)

    ew1_a = load(wts, [EDD, 128], t_em_w1[0:EDD, :], "ew1_a")
    ew1_b = load(wts, [EDD, 128], t_em_w1[EDD:2 * EDD, :], "ew1_b")
    ew1_fi = load(wts, [1, 128], t_em_w1[4 * EDD + 1:4 * EDD + 2, :], "ew1_fi")
    ew1_fj = load(wts, [1, 128], t_em_w1[4 * EDD + 2:4 * EDD + 3, :], "ew1_fj")
    # fp16 copies for the edge MLP (f32 staging in a short-lived pool)
    ew1_abs16 = wts.tile([EDD, 128], F16, tag="ew1_abs16")
    ew1_prod16 = wts.tile([EDD, 128], F16, tag="ew1_prod16")
    ew1_cos16 = wts.tile([1, 128], F16, tag="ew1_cos16")
    em_w2_16 = wts.tile([128, 64], F16, tag="em_w2_16")
    em_w3_16 = wts.tile([64, 1], F16, tag="em_w3_16")
    b_alpha = wts.tile([1, F], F16, tag="b_alpha")

    b_pp1 = load(wts, [1, HID], t_pp_b1.ap().rearrange("(o h) -> o h", o=1), "b_pp1")
    b_pp2 = load(wts, [1, HID], t_pp_b2.ap().rearrange("(o h) -> o h", o=1), "b_pp2")
    b_m1 = load(wts, [1, HID], t_msg_b1.ap().rearrange("(o h) -> o h", o=1), "b_m1")
    b_m2 = load(wts, [1, HID], t_msg_b2.ap().rearrange("(o h) -> o h", o=1), "b_m2")
    b_ed = load(wts, [1, EDD], t_ed_b.ap().rearrange("(o h) -> o h", o=1), "b_ed")
    b_em1 = load(wts, [128, 1], t_em_b1.ap().rearrange("(p o) -> p o", o=1), "b_em1")
    b_em2 = load(wts, [64, 1], t_em_b2.ap().rearrange("(p o) -> p o", o=1), "b_em2")
    b_em3 = load(wts, [1, 1], t_em_b3.ap().rearrange("(p o) -> p o", o=1), "b_em3")
    b_biasb = load(wts, [1, 1], t_bias_b.ap().rearrange("(p o) -> p o", o=1), "b_biasb")

    # cpm: mid-lived tensors that survive into late phase C
    cpm_cm = tc.tile_pool(name="cpm", bufs=1)
    cpm = cpm_cm.__enter__()

    # pp_w1 lives in its own pool; released after Hn1 so alpha_w can reuse it.
    w1pool_cm = tc.tile_pool(name="w1pool", bufs=1)
    w1pool = w1pool_cm.__enter__()
    pp_w1_sb = w1pool.tile([128, KF, HID], F16, tag="pp_w1_sb")
    nc.sync.dma_start(pp_w1_sb[:], t_pp_w1.ap().rearrange("(k p) h -> p k h", p=128))

    # f32 staging for the fp16 weight copies; freed before phase A pressure
    with tc.tile_pool(name="stg", bufs=1) as stg:
        ew1_abs = load(stg, [EDD, 128], t_em_w1[2 * EDD:3 * EDD, :], "ew1_abs")
        nc.scalar.copy(ew1_abs16[:], ew1_abs[:])
        ew1_prod = load(stg, [EDD, 128], t_em_w1[3 * EDD:4 * EDD, :], "ew1_prod")
        nc.scalar.copy(ew1_prod16[:], ew1_prod[:])
        ew1_cos = load(stg, [1, 128], t_em_w1[4 * EDD:4 * EDD + 1, :], "ew1_cos")
        nc.scalar.copy(ew1_cos16[:], ew1_cos[:])
        em_w2_sb = load(stg, [128, 64], t_em_w2[:, :], "em_w2_sb")
        nc.scalar.copy(em_w2_16[:], em_w2_sb[:])
        em_w3_sb = load(stg, [64, 1], t_em_w3[:, :], "em_w3_sb")
        nc.scalar.copy(em_w3_16[:], em_w3_sb[:])
        b_alpha32 = load(stg, [1, F], t_alpha_b.ap().rearrange("(o h) -> o h", o=1), "b_alpha32")
        nc.scalar.copy(b_alpha[:], b_alpha32[:])

    # ---------------- phase A: stream feats, stash fp16, pool ----------------
    # stash is split into two tiles, one per casting engine, so the Scalar and
    # GpSimd casts never alias the same tile (Tile tracks deps per-tile).
    stash_a = base.tile([128, KF, 2, PIX], F16, tag="stash_a")
    stash_b = base.tile([128, KF, 2, PIX], F16, tag="stash_b")
    pooled_sb = base.tile([128, BL, KF], F32, tag="pooled_sb")
    pooled_loc = dram.tile([BL, F], F32)
    pooled_full_d = dram.tile([B, F], F32)

    def stash_slice(b, k, n0, n1):
        g = b * (KF // 2) + k // 2
        t = stash_a if g % 2 == 0 else stash_b
        return t[:, g // 2, k % 2, n0:n1]

    with tc.tile_pool(name="ina", bufs=4) as ina:
        for b in range(BL):
            for i in range(KF // 2):
                g = b * (KF // 2) + i
                tin = ina.tile([128, 2, PIX], F32, tag="tin")
                nc.sync.dma_start(
                    tin[:],
                    t_feats[b, 256 * i:256 * (i + 1), :].rearrange("(c p) n -> p c n", p=128),
                )
                dst = (stash_a if g % 2 == 0 else stash_b)[:, g // 2, :, :]
                if g % 2 == 0:
                    nc.scalar.copy(dst, tin[:])
                else:
                    nc.gpsimd.tensor_copy(dst, tin[:])
                nc.vector.tensor_reduce(
                    pooled_sb[:, b, 2 * i:2 * i + 2], tin[:], axis=AX.X, op=ALU.add
                )
            nc.sync.dma_start(
                pooled_loc[:][b].rearrange("(k p) -> p k", p=128), pooled_sb[:, b, :]
            )

    nc.gpsimd.collective_compute(
        "AllGather", ALU.bypass,
        replica_groups=[list(range(N_CORES))],
        ins=[pooled_loc.opt()], outs=[pooled_full_d.opt()],
    )

    # ---------------- phase C (early): proto / cos / Hn1 ----------------
    cpe_cm = tc.tile_pool(name="cpe", bufs=1)
    cpe = cpe_cm.__enter__()

    pooled_full = cpe.tile([B, F], F32, tag="pooled_full")
    nc.sync.dma_start(pooled_full[:], pooled_full_d[:])

    def sigmoid(out_ap, in_ap, scale=1.0):
        # 1 / (1 + exp(-x*scale)) via Exp + DVE reciprocal (single ACT table)
        nc.scalar.activation(out_ap, in_ap, ACTF.Exp, scale=-scale)
        nc.vector.tensor_scalar_add(out_ap, out_ap, 1.0)
        nc.vector.reciprocal(out_ap, out_ap)

    def softplus_neg(out_ap, in_ap):
        # log1p(exp(-x)) for x >= 0
        nc.scalar.activation(out_ap, in_ap, ACTF.Exp, scale=-1.0)
        nc.scalar.activation(out_ap, out_ap, ACTF.Ln, bias=1.0)

    probs = base.tile([B, C], F32, tag="probs")
    sigmoid(probs[:], cls_sb[:])
    y_sb = base.tile([B, C], F32, tag="y_sb")
    nc.vector.tensor_scalar_max(y_sb[:], tgt_sb[:], 0.0)
    probs16 = cpe.tile([B, C], F16, tag="probs16")
    nc.scalar.copy(probs16[:], probs[:])
    pooled16 = cpe.tile([B, F], F16, tag="pooled16")
    nc.vector.tensor_copy(pooled16[:], pooled_full[:])

    # weight_sum / freq / present
    ws_ps = ps.tile([C, 1], F32, tag="ps")
    nc.tensor.matmul(ws_ps[:], probs[:], ones_col[:B, :])
    wsum = cpm.tile([C, 1], F32, tag="wsum")
    nc.scalar.copy(wsum[:], ws_ps[:])
    freq_col = cpm.tile([C, 1], F32, tag="freq_col")
    nc.scalar.mul(freq_col[:], wsum[:], 1.0 / B)
    wclamp = cpm.tile([C, 1], F32, tag="wclamp")
    nc.vector.tensor_scalar_max(wclamp[:], wsum[:], EPS)
    winv = cpm.tile([C, 1], F32, tag="winv")
    nc.vector.reciprocal(winv[:], wclamp[:])
    winv_eff = cpm.tile([C, 1], F32, tag="winv_eff")
    nc.scalar.mul(winv_eff[:], winv[:], 1.0 / PIX)
    # row of wsum*PIX (to inject exact pp_b1 under the later winv_eff scaling)
    wsp_col = cpm.tile([C, 1], F32, tag="wsp_col")
    nc.scalar.mul(wsp_col[:], wclamp[:], float(PIX))
    wsp_ps = ps.tile([1, C], F32, tag="ps")
    nc.tensor.transpose(wsp_ps[:], wsp_col[:], id128[:C, :C])
    wsp_row = cpm.tile([1, C], F32, tag="wsp_row")
    nc.vector.tensor_copy(wsp_row[:], wsp_ps[:])

    ys_ps = ps.tile([C, 1], F32, tag="ps")
    nc.tensor.matmul(ys_ps[:], y_sb[:], ones_col[:B, :])
    present = cpm.tile([C, 1], F32, tag="present")
    nc.vector.tensor_single_scalar(present[:], ys_ps[:], 0.5, ALU.is_gt)

    # proto row norms via Square+accum (proto left unnormalized, scales folded)
    nrm_part = cpe.tile([C, 4], F32, tag="nrm_part")
    sq_scr = cpe.tile([C, 512], F32, tag="sq_scr")
    for j in range(4):
        pp_ = psacc.tile([C, 512], F32, tag="acc")
        nc.tensor.matmul(
            pp_[:], probs16[:],
            pooled16[:, 512 * j:512 * (j + 1)],
        )
        nc.scalar.activation(
            sq_scr[:], pp_[:], ACTF.Square, accum_out=nrm_part[:, j:j + 1]
        )
    nrm_sq = cpm.tile([C, 1], F32, tag="nrm_sq")
    nc.vector.tensor_reduce(nrm_sq[:], nrm_part[:], axis=AX.X, op=ALU.add)
    nrm = cpm.tile([C, 1], F32, tag="nrm")
    nc.scalar.activation(nrm[:], nrm_sq[:], ACTF.Ln)
    nc.scalar.activation(nrm[:], nrm[:], ACTF.Exp, scale=0.5)  # sqrt
    nc.vector.tensor_scalar_max(nrm[:], nrm[:], EPS)
    inv_u = cpm.tile([C, 1], F32, tag="inv_u")
    nc.vector.reciprocal(inv_u[:], nrm[:])

    # protoT (f-major), G, cos
    protoT = cpe.tile([128, KF, C], F16, tag="protoT")
    for k in range(KF):
        pt_ = ps.tile([128, C], F32, tag="ps")
        nc.tensor.matmul(pt_[:], pooled_full[:, 128 * k:128 * (k + 1)], probs[:])
        if k % 2 == 0:
            nc.vector.tensor_copy(protoT[:, k, :], pt_[:])
        else:
            nc.scalar.copy(protoT[:, k, :], pt_[:])
    g_ps = psacc.tile([C, C], F32, tag="acc")
    for k in range(KF):
        nc.tensor.matmul(
            g_ps[:], protoT[:, k, :], protoT[:, k, :],
            start=(k == 0), stop=(k == KF - 1),
        )
    ir_ps = ps.tile([1, C], F32, tag="ps")
    nc.tensor.transpose(ir_ps[:], inv_u[:], id128[:C, :C])
    inv_row = cpm.tile([1, C], F32, tag="inv_row")
    nc.vector.tensor_copy(inv_row[:], ir_ps[:])
    s_ps = ps.tile([C, C], F32, tag="ps")
    nc.tensor.matmul(s_ps[:], inv_row[:], inv_row[:])
    cos_sb = cpm.tile([C, C], F32, tag="cos_sb")
    nc.scalar.copy(cos_sb[:], g_ps[:])
    nc.vector.tensor_tensor(cos_sb[:], cos_sb[:], s_ps[:], ALU.mult)
    nc.vector.tensor_scalar(cos_sb[:], cos_sb[:], 1.0, -1.0, ALU.min, ALU.max)

    # Hn1 = relu((proto_u @ pp_w1 + pp_b1*wsum*PIX) * winv_eff)
    h1_ps = psacc.tile([C, HID], F32, tag="acc")
    for k in range(KF):
        nc.tensor.matmul(
            h1_ps[:], protoT[:, k, :], pp_w1_sb[:, k, :],
            start=(k == 0), stop=False,
        )
    nc.tensor.matmul(h1_ps[:], wsp_row[:], b_pp1[:], start=False, stop=True)
    relu1 = cpe.tile([C, HID], F32, tag="relu1")
    nc.vector.tensor_scalar_mul(relu1[:], h1_ps[:], winv_eff[:])
    nc.scalar.activation(relu1[:], relu1[:], ACTF.Relu)

    r1T = cpm.tile([128, 2, C], F16, tag="r1T")
    for j in range(2):
        tp_ = ps.tile([128, C], F32, tag="ps")
        nc.tensor.transpose(tp_[:], relu1[:, 128 * j:128 * (j + 1)], id128[:C, :C])
        nc.scalar.copy(r1T[:, j, :], tp_[:])

    # release pp_w1 + early tensors (LIFO); open late pools
    cpe_cm.__exit__(None, None, None)
    w1pool_cm.__exit__(None, None, None)

    cpl_cm = tc.tile_pool(name="cpl", bufs=1)
    cpl = cpl_cm.__enter__()
    awpool_cm = tc.tile_pool(name="awpool", bufs=1)
    awpool = awpool_cm.__enter__()
    alpha_w_sb = awpool.tile([128, 2, F], F16, tag="alpha_w_sb")
    nc.sync.dma_start(alpha_w_sb[:], t_alpha_w.ap().rearrange("(k p) h -> p k h", p=128))

    def transpose_pair(src, tag="tpair"):
        # src [C, 256] -> dst [128, 2, C]
        dst = cpl.tile([128, 2, C], F16, tag=tag)
        for j in range(2):
            tp_ = ps.tile([128, C], F32, tag="ps")
            nc.tensor.transpose(tp_[:], src[:, 128 * j:128 * (j + 1)], id128[:C, :C])
            nc.scalar.copy(dst[:, j, :], tp_[:])
        return dst

    hn_ps = psacc.tile([C, HID], F32, tag="acc")
    for j in range(2):
        nc.tensor.matmul(
            hn_ps[:], r1T[:, j, :], pp_w2_sb[:, j, :],
            start=(j == 0), stop=False,
        )
    nc.tensor.matmul(hn_ps[:], ones_row[:1, :C], b_pp2[:], start=False, stop=True)
    hn_sb = cpl.tile([C, HID], F32, tag="hn_sb")
    nc.scalar.copy(hn_sb[:], hn_ps[:])

    hnT = transpose_pair(hn_sb)
    zd_ps = psacc.tile([C, EDD], F32, tag="acc")
    for j in range(2):
        nc.tensor.matmul(zd_ps[:], hnT[:, j, :], ed_w_sb[:, j, :], start=(j == 0), stop=False)
    nc.tensor.matmul(zd_ps[:], ones_row[:1, :C], b_ed[:], start=False, stop=True)
    zd_sb = cpl.tile([C, EDD], F32, tag="zd_sb")
    nc.scalar.activation(zd_sb[:], zd_ps[:], ACTF.Relu)

    zdT_ps = ps.tile([EDD, C], F32, tag="ps")
    nc.tensor.transpose(zdT_ps[:], zd_sb[:], id128[:C, :C])
    zdT = cpl.tile([EDD, C], F32, tag="zdT")
    nc.vector.tensor_copy(zdT[:], zdT_ps[:])

    # local (sharded) pieces via iselT
    zdl_ps = ps.tile([IL, EDD], F32, tag="ps")
    nc.tensor.matmul(zdl_ps[:], isel_sb[:], zd_sb[:])
    zdl = cpl.tile([IL, EDD], F32, tag="zdl")
    nc.vector.tensor_copy(zdl[:], zdl_ps[:])
    zdTl_ps = ps.tile([EDD, IL], F32, tag="ps")
    nc.tensor.transpose(zdTl_ps[:], zdl[:], id128[:IL, :IL])
    zdTl = cpl.tile([EDD, IL], F32, tag="zdTl")
    nc.vector.tensor_copy(zdTl[:], zdTl_ps[:])

    fl_ps = ps.tile([IL, 1], F32, tag="ps")
    nc.tensor.matmul(fl_ps[:], isel_sb[:], freq_col[:])
    fl_sb = cpl.tile([IL, 1], F32, tag="fl_sb")
    nc.vector.tensor_copy(fl_sb[:], fl_ps[:])
    flr_ps = ps.tile([1, IL], F32, tag="ps")
    nc.tensor.transpose(flr_ps[:], fl_sb[:], id128[:IL, :IL])
    flr = cpl.tile([1, IL], F32, tag="flr")
    nc.vector.tensor_copy(flr[:], flr_ps[:])
    fr_ps = ps.tile([1, C], F32, tag="ps")
    nc.tensor.transpose(fr_ps[:], freq_col[:], id128[:C, :C])
    fr_sb = cpl.tile([1, C], F32, tag="fr_sb")
    nc.vector.tensor_copy(fr_sb[:], fr_ps[:])

    # PT_loc (+ freq_i term), QT (+ freq_j term)
    ptl_ps = ps.tile([128, IL], F32, tag="ps")
    nc.tensor.matmul(ptl_ps[:], ew1_a[:], zdTl[:], start=True, stop=False)
    nc.tensor.matmul(ptl_ps[:], ew1_fi[:], flr[:], start=False, stop=True)
    ptl = cpl.tile([128, IL], F32, tag="ptl")
    nc.vector.tensor_copy(ptl[:], ptl_ps[:])
    qt_ps = ps.tile([128, C], F32, tag="ps")
    nc.tensor.matmul(qt_ps[:], ew1_b[:], zdT[:], start=True, stop=False)
    nc.tensor.matmul(qt_ps[:], ew1_fj[:], fr_sb[:], start=False, stop=True)
    qt = cpl.tile([128, C], F32, tag="qt")
    nc.scalar.copy(qt[:], qt_ps[:])
    ptq = cpl.tile([128, IL, C], F16, tag="ptq")
    nc.vector.tensor_tensor(
        ptq[:],
        ptl[:].unsqueeze(2).broadcast_to([128, IL, C]),
        qt[:].unsqueeze(1).broadcast_to([128, IL, C]),
        ALU.add,
    )

    cosl_ps = ps.tile([IL, C], F32, tag="ps")
    nc.tensor.matmul(cosl_ps[:], isel_sb[:], cos_sb[:])
    cosl16 = cpl.tile([IL, C], F16, tag="cosl16")
    nc.scalar.copy(cosl16[:], cosl_ps[:])
    cos_flat = cpl.tile([1, IL * C], F16, tag="cos_flat")
    nc.sync.dma_start(cos_flat[:], cosl16[:])

    # edge feature blocks: |zi-zj| (in place) and zi*zj, fp16
    absblk = cpl.tile([EDD, IL, C], F16, tag="absblk")
    nc.vector.tensor_tensor(
        absblk[:],
        zdTl[:].unsqueeze(2).broadcast_to([EDD, IL, C]),
        zdT[:].unsqueeze(1).broadcast_to([EDD, IL, C]),
        ALU.subtract,
    )
    nc.scalar.activation(absblk[:], absblk[:], ACTF.Abs)
    prodblk = cpl.tile([EDD, IL, C], F16, tag="prodblk")
    nc.vector.tensor_tensor(
        prodblk[:],
        zdTl[:].unsqueeze(2).broadcast_to([EDD, IL, C]),
        zdT[:].unsqueeze(1).broadcast_to([EDD, IL, C]),
        ALU.mult,
    )

    nedge = IL * C
    abs_flat = absblk[:].rearrange("p r j -> p (r j)")
    prod_flat = prodblk[:].rearrange("p r j -> p (r j)")
    ptqf = ptq[:].rearrange("p r j -> p (r j)")

    e1T = cpl.tile([128, nedge], F16, tag="e1T")
    for n0, n1 in ((0, 512), (512, nedge)):
        e1_ps = psacc.tile([128, 512], F32, tag="acc")
        nc.tensor.matmul(e1_ps[:, :n1 - n0], ew1_abs16[:], abs_flat[:, n0:n1], start=True, stop=False)
        nc.tensor.matmul(e1_ps[:, :n1 - n0], ew1_prod16[:], prod_flat[:, n0:n1], start=False, stop=False)
        nc.tensor.matmul(e1_ps[:, :n1 - n0], ew1_cos16[:], cos_flat[:, n0:n1], start=False, stop=True)
        nc.vector.tensor_tensor(e1_ps[:, :n1 - n0], e1_ps[:, :n1 - n0], ptqf[:, n0:n1], ALU.add)
        nc.scalar.activation(e1T[:, n0:n1], e1_ps[:, :n1 - n0], ACTF.Relu, bias=b_em1[:])

    e2T = cpl.tile([EDD, nedge], F16, tag="e2T")
    for n0, n1 in ((0, 512), (512, nedge)):
        e2_ps = psacc.tile([EDD, 512], F32, tag="acc")
        nc.tensor.matmul(e2_ps[:, :n1 - n0], em_w2_16[:], e1T[:, n0:n1])
        nc.scalar.activation(e2T[:, n0:n1], e2_ps[:, :n1 - n0], ACTF.Relu, bias=b_em2[:])

    r_sb = cpl.tile([1, nedge], F32, tag="r_sb")
    for n0, n1 in ((0, 512), (512, nedge)):
        r_ps = ps.tile([1, 512], F32, tag="ps")
        nc.tensor.matmul(r_ps[:, :n1 - n0], em_w3_16[:], e2T[:, n0:n1])
        nc.scalar.activation(r_sb[:, n0:n1], r_ps[:, :n1 - n0], ACTF.Identity, bias=b_em3[:])

    r_loc_d = dram.tile([IL, C], F32)
    r_full_d = dram.tile([C, C], F32)
    nc.sync.dma_start(r_loc_d[:], r_sb[:])
    nc.gpsimd.collective_compute(
        "AllGather", ALU.bypass,
        replica_groups=[list(range(N_CORES))],
        ins=[r_loc_d.opt()], outs=[r_full_d.opt()],
    )
    r_full = cpl.tile([C, C], F32, tag="r_full")
    nc.sync.dma_start(r_full[:], r_full_d[:])

    # W_adj
    h_sb = cpl.tile([C, C], F32, tag="h_sb")
    nc.vector.scalar_tensor_tensor(h_sb[:], prior_sb[:], 1.0 / TEMP, r_full[:], ALU.mult, ALU.add)
    w_sg = cpl.tile([C, C], F32, tag="w_sg")
    sigmoid(w_sg[:], h_sb[:])
    wt_ps = ps.tile([C, C], F32, tag="ps")
    nc.tensor.transpose(wt_ps[:], w_sg[:], id128[:C, :C])
    w_half = cpl.tile([C, C], F32, tag="w_half")
    nc.scalar.mul(w_half[:], wt_ps[:], 0.5)
    nc.vector.scalar_tensor_tensor(w_half[:], w_sg[:], 0.5, w_half[:], ALU.mult, ALU.add)
    wadj = cpl.tile([C, C], F32, tag="wadj")
    nc.vector.tensor_mul(wadj[:], w_half[:], offd[:])
    nc.sync.dma_start(o_wadj[:, :], wadj[:])

    rowsum = cpl.tile([C, 1], F32, tag="rowsum")
    nc.vector.tensor_reduce(rowsum[:], wadj[:], axis=AX.X, op=ALU.add)
    nc.vector.tensor_scalar_max(rowsum[:], rowsum[:], EPS)
    rinv = cpl.tile([C, 1], F32, tag="rinv")
    nc.vector.reciprocal(rinv[:], rowsum[:])
    rr_ps = ps.tile([1, C], F32, tag="ps")
    nc.tensor.transpose(rr_ps[:], rinv[:], id128[:C, :C])
    rinv_row = cpl.tile([1, C], F32, tag="rinv_row")
    nc.vector.tensor_copy(rinv_row[:], rr_ps[:])
    rb_ps = ps.tile([C, C], F32, tag="ps")
    nc.tensor.matmul(rb_ps[:], ones_row[:1, :C], rinv_row[:])
    at_sb = cpl.tile([C, C], F32, tag="at_sb")
    nc.vector.tensor_tensor(at_sb[:], wadj[:], rb_ps[:], ALU.mult)

    # message passing (1 step)
    m1T = cpl.tile([128, 2, C], F16, tag="tpair")
    for j in range(2):
        mp_ = ps.tile([128, C], F32, tag="ps")
        nc.tensor.matmul(mp_[:], hn_sb[:, 128 * j:128 * (j + 1)], at_sb[:])
        nc.scalar.copy(m1T[:, j, :], mp_[:])
    m1_ps = psacc.tile([C, HID], F32, tag="acc")
    for j in range(2):
        nc.tensor.matmul(
            m1_ps[:], m1T[:, j, :], msg_w1_sb[:, j, :],
            start=(j == 0), stop=False,
        )
    nc.tensor.matmul(m1_ps[:], ones_row[:1, :C], b_m1[:], start=False, stop=True)
    mr_sb = cpl.tile([C, HID], F32, tag="mr_sb")
    nc.scalar.activation(mr_sb[:], m1_ps[:], ACTF.Relu)
    mrT = transpose_pair(mr_sb)
    m2_ps = psacc.tile([C, HID], F32, tag="acc")
    for j in range(2):
        nc.tensor.matmul(
            m2_ps[:], mrT[:, j, :], msg_w2_sb[:, j, :],
            start=(j == 0), stop=False,
        )
    nc.tensor.matmul(m2_ps[:], ones_row[:1, :C], b_m2[:], start=False, stop=True)
    zn_sb = cpl.tile([C, HID], F32, tag="zn_sb")
    nc.vector.tensor_tensor(zn_sb[:], m2_ps[:], hn_sb[:], ALU.add)
    nc.scalar.activation(zn_sb[:], zn_sb[:], ACTF.Relu)
    znT = transpose_pair(zn_sb, tag="znT")

    # aw head
    aw_sb = cpl.tile([C, F], F32, tag="aw_sb")
    aws_part = cpl.tile([C, 4], F32, tag="aws_part")
    for n in range(4):
        a_ps = psacc.tile([C, 512], F32, tag="acc")
        for j in range(2):
            nc.tensor.matmul(
                a_ps[:], znT[:, j, :],
                alpha_w_sb[:, j, 512 * n:512 * (n + 1)],
                start=(j == 0), stop=False,
            )
        nc.tensor.matmul(a_ps[:], ones_row16[:1, :C], b_alpha[:, 512 * n:512 * (n + 1)], start=False, stop=True)
        # softplus(x) = ln(exp(x) + 1)
        nc.scalar.activation(aw_sb[:, 512 * n:512 * (n + 1)], a_ps[:], ACTF.Exp)
        nc.scalar.activation(
            aw_sb[:, 512 * n:512 * (n + 1)], aw_sb[:, 512 * n:512 * (n + 1)],
            ACTF.Ln, bias=1.0,
            accum_out=aws_part[:, n:n + 1],
        )
    aws = cpl.tile([C, 1], F32, tag="aws")
    nc.vector.tensor_reduce(aws[:], aws_part[:], axis=AX.X, op=ALU.add)
    nc.vector.tensor_scalar_max(aws[:], aws[:], EPS)
    sinv = cpl.tile([C, 1], F32, tag="sinv")
    nc.vector.reciprocal(sinv[:], aws[:])
    nc.vector.tensor_scalar_mul(aw_sb[:], aw_sb[:], sinv[:])
    nc.sync.dma_start(o_aw[:, :], aw_sb[:])

    awT16 = cpl.tile([128, KF, C], F16, tag="awT16")
    for k in range(KF):
        at_ps = ps.tile([128, C], F32, tag="ps")
        nc.tensor.transpose(at_ps[:], aw_sb[:, 128 * k:128 * (k + 1)], id128[:C, :C])
        nc.scalar.copy(awT16[:, k, :], at_ps[:])

    awpool_cm.__exit__(None, None, None)

    # dlog
    dl_ps = ps.tile([C, 1], F32, tag="ps")
    for j in range(2):
        nc.tensor.matmul(dl_ps[:], znT[:, j, :], bias_w_sb[:, j, :], start=(j == 0), stop=False)
    nc.tensor.matmul(dl_ps[:], ones_row[:1, :C], b_biasb[:], start=False, stop=True)
    dlog_sb = cpl.tile([C, 1], F32, tag="dlog_sb")
    nc.scalar.copy(dlog_sb[:], dl_ps[:])
    nc.sync.dma_start(o_dlog.ap().rearrange("(p o) -> p o", o=1), dlog_sb[:])
    dlr_ps = ps.tile([1, C], F32, tag="ps")
    nc.tensor.transpose(dlr_ps[:], dlog_sb[:], id128[:C, :C])
    dlr = cpl.tile([1, C], F32, tag="dlr")
    nc.vector.tensor_copy(dlr[:], dlr_ps[:])

    # refined
    py = cpl.tile([B, C], F32, tag="py")
    nc.vector.tensor_mul(py[:], probs[:], y_sb[:])
    pyT_ps = ps.tile([C, B], F32, tag="ps")
    nc.tensor.transpose(pyT_ps[:], py[:], id128[:B, :B])
    pyT = cpl.tile([C, B], F32, tag="pyT")
    nc.vector.tensor_copy(pyT[:], pyT_ps[:])
    pa_ps = ps.tile([B, C], F32, tag="ps")
    nc.tensor.matmul(pa_ps[:], pyT[:], wadj[:])
    nc.vector.tensor_mul(py[:], y_sb[:], pa_ps[:])  # py := pos_agg
    dlb_ps = ps.tile([B, C], F32, tag="ps")
    nc.tensor.matmul(dlb_ps[:], ones_row[:1, :B], dlr[:])
    prow = cpl.tile([B, 1], F32, tag="prow")
    nc.vector.tensor_reduce(prow[:], probs[:], axis=AX.X, op=ALU.add)
    nc.scalar.mul(prow[:], prow[:], -GAMMA_NEG)
    refined = cpl.tile([B, C], F32, tag="refined")
    nc.vector.scalar_tensor_tensor(
        refined[:], py[:], BETA_POS + GAMMA_NEG, dlb_ps[:], ALU.mult, ALU.add
    )
    nc.vector.tensor_scalar_add(refined[:], refined[:], prow[:])
    nc.vector.tensor_add(refined[:], refined[:], cls_sb[:])
    nc.sync.dma_start(o_refined[:, :], refined[:])

    # ---------------- phase D: CAM ----------------
    dpool_cm = tc.tile_pool(name="dpool", bufs=2)
    dpool = dpool_cm.__enter__()
    for b in range(BL):
        cam = dpool.tile([C, PIX], F32, tag="cam")
        for hh in range(2):
            c_ps = pscam.tile([C, 512], F32, tag="cam_ps")
            for k in range(KF):
                nc.tensor.matmul(
                    c_ps[:], awT16[:, k, :], stash_slice(b, k, 512 * hh, 512 * (hh + 1)),
                    start=(k == 0), stop=(k == KF - 1),
                )
            nc.scalar.activation(cam[:, 512 * hh:512 * (hh + 1)], c_ps[:], ACTF.Relu)
        mn = dpool.tile([C, 1], F32, tag="mn")
        nc.vector.tensor_reduce(mn[:], cam[:], axis=AX.X, op=ALU.min)
        mx = dpool.tile([C, 1], F32, tag="mx")
        nc.vector.tensor_reduce(mx[:], cam[:], axis=AX.X, op=ALU.max)
        nc.vector.tensor_sub(mx[:], mx[:], mn[:])
        nc.vector.tensor_scalar_add(mx[:], mx[:], EPS)
        dinv = dpool.tile([C, 1], F32, tag="dinv")
        nc.vector.reciprocal(dinv[:], mx[:])
        nc.vector.tensor_scalar(cam[:], cam[:], mn[:], dinv[:], ALU.subtract, ALU.mult)
        nc.sync.dma_start(o_cam[b, :, :], cam[:])

    # ---------------- losses ----------------
    def colsum_1x1(src_col, nrows, tag):
        p_ = ps.tile([1, 1], F32, tag="ps")
        nc.tensor.matmul(p_[:], src_col[:], ones_col[:nrows, :])
        out = cpl.tile([1, 1], F32, tag=tag)
        nc.vector.tensor_copy(out[:], p_[:])
        return out

    # cls loss: bce = relu(h) - h*t + softplus(-|h|), masked mean
    m_sb = cpl.tile([B, C], F32, tag="m_sb")
    nc.vector.tensor_single_scalar(m_sb[:], tgt_sb[:], -1.0, ALU.not_equal)
    safe_t = cpl.tile([B, C], F32, tag="safe_t")
    nc.vector.tensor_mul(safe_t[:], tgt_sb[:], m_sb[:])
    nc.vector.tensor_mul(safe_t[:], refined[:], safe_t[:])  # safe_t := h*t
    rh = cpl.tile([B, C], F32, tag="rh")
    nc.scalar.activation(rh[:], refined[:], ACTF.Relu)
    ab = cpl.tile([B, C], F32, tag="ab")
    nc.scalar.activation(ab[:], refined[:], ACTF.Abs)
    softplus_neg(ab[:], ab[:])  # ab := softplus(-|h|)
    nc.vector.tensor_sub(rh[:], rh[:], safe_t[:])
    nc.vector.tensor_add(rh[:], rh[:], ab[:])
    nc.vector.tensor_mul(rh[:], rh[:], m_sb[:])  # rh := bce * m
    bm_rows = cpl.tile([B, 1], F32, tag="bm_rows")
    nc.vector.tensor_reduce(bm_rows[:], rh[:], axis=AX.X, op=ALU.add)
    bce_sum = colsum_1x1(bm_rows, B, "bce_sum")
    nc.vector.tensor_reduce(bm_rows[:], m_sb[:], axis=AX.X, op=ALU.add)
    m_sum = colsum_1x1(bm_rows, B, "m_sum")
    nc.vector.tensor_scalar_max(m_sum[:], m_sum[:], 1.0)
    m_inv = cpl.tile([1, 1], F32, tag="m_inv")
    nc.vector.reciprocal(m_inv[:], m_sum[:])
    cls_loss = cpl.tile([1, 1], F32, tag="cls_loss")
    nc.vector.tensor_mul(cls_loss[:], bce_sum[:], m_inv[:])

    # edge mask
    pr_ps = ps.tile([1, C], F32, tag="ps")
    nc.tensor.transpose(pr_ps[:], present[:], id128[:C, :C])
    pres_row = cpl.tile([1, C], F32, tag="pres_row")
    nc.vector.tensor_copy(pres_row[:], pr_ps[:])
    ppo_ps = ps.tile([C, C], F32, tag="ps")
    nc.tensor.matmul(ppo_ps[:], pres_row[:], pres_row[:])
    emf = cpl.tile([C, C], F32, tag="emf")
    nc.vector.tensor_tensor(emf[:], offd[:], ppo_ps[:], ALU.mult)

    # edge bce over h vs soft targets t = sigmoid(prior/TEMP)
    te_sb = cpl.tile([C, C], F32, tag="te_sb")
    sigmoid(te_sb[:], prior_sb[:], scale=1.0 / TEMP)
    nc.vector.tensor_mul(te_sb[:], h_sb[:], te_sb[:])  # te_sb := h*t
    rhe = cpl.tile([C, C], F32, tag="rhe")
    nc.scalar.activation(rhe[:], h_sb[:], ACTF.Relu)
    abe = cpl.tile([C, C], F32, tag="abe")
    nc.scalar.activation(abe[:], h_sb[:], ACTF.Abs)
    softplus_neg(abe[:], abe[:])
    nc.vector.tensor_sub(rhe[:], rhe[:], te_sb[:])
    nc.vector.tensor_add(rhe[:], rhe[:], abe[:])   # rhe := bce_e
    nc.vector.tensor_mul(rhe[:], rhe[:], emf[:])   # rhe := bce_e * emf

    posf = cpl.tile([C, C], F32, tag="posf")
    nc.vector.tensor_single_scalar(posf[:], prior_sb[:], 0.0, ALU.is_gt)

    rcol = cpl.tile([C, 1], F32, tag="rcol")
    nc.vector.tensor_reduce(rcol[:], emf[:], axis=AX.X, op=ALU.add)
    n_edges = colsum_1x1(rcol, C, "n_edges")
    nc.vector.tensor_mul(abe[:], posf[:], emf[:])  # abe := posf*emf
    nc.vector.tensor_reduce(rcol[:], abe[:], axis=AX.X, op=ALU.add)
    n_pos = colsum_1x1(rcol, C, "n_pos")
    nc.vector.tensor_reduce(rcol[:], rhe[:], axis=AX.X, op=ALU.add)
    s1 = colsum_1x1(rcol, C, "s1")
    nc.vector.tensor_mul(abe[:], rhe[:], posf[:])  # abe := bce_e*emf*posf
    nc.vector.tensor_reduce(rcol[:], abe[:], axis=AX.X, op=ALU.add)
    s2 = colsum_1x1(rcol, C, "s2")

    nc.scalar.activation(abe[:], r_full[:], ACTF.Abs)
    nc.vector.tensor_mul(abe[:], abe[:], emf[:])   # abe := |r|*emf
    nc.vector.tensor_reduce(rcol[:], abe[:], axis=AX.X, op=ALU.add)
    rr_sum = colsum_1x1(rcol, C, "rr_sum")
    nc.vector.tensor_reduce(rcol[:], wadj[:], axis=AX.X, op=ALU.add)
    wa_sum = colsum_1x1(rcol, C, "wa_sum")

    n_pos_c = cpl.tile([1, 1], F32, tag="n_pos_c")
    nc.vector.tensor_scalar_max(n_pos_c[:], n_pos[:], 1.0)
    n_neg = cpl.tile([1, 1], F32, tag="n_neg")
    nc.vector.tensor_sub(n_neg[:], n_edges[:], n_pos[:])
    nc.vector.tensor_scalar_max(n_neg[:], n_neg[:], 1.0)
    np_inv = cpl.tile([1, 1], F32, tag="np_inv")
    nc.vector.reciprocal(np_inv[:], n_pos_c[:])
    w_pos = cpl.tile([1, 1], F32, tag="w_pos")
    nc.vector.tensor_mul(w_pos[:], n_neg[:], np_inv[:])
    nc.vector.tensor_scalar(w_pos[:], w_pos[:], 1.0, 10.0, ALU.max, ALU.min)
    nc.vector.tensor_scalar_add(w_pos[:], w_pos[:], -1.0)  # w_pos := w_pos - 1

    nc.vector.tensor_scalar_max(n_edges[:], n_edges[:], 1.0)
    ne_inv = cpl.tile([1, 1], F32, tag="ne_inv")
    nc.vector.reciprocal(ne_inv[:], n_edges[:])

    # edge_loss = (s1 + (w_pos-1)*s2) / n_edges ; r_reg = 0.001*rr_sum/n_edges
    el_num = cpl.tile([1, 1], F32, tag="el_num")
    nc.vector.tensor_mul(el_num[:], w_pos[:], s2[:])
    nc.vector.tensor_add(el_num[:], el_num[:], s1[:])
    nc.vector.tensor_mul(el_num[:], el_num[:], ne_inv[:])  # el_num := edge_loss
    r_reg = cpl.tile([1, 1], F32, tag="r_reg")
    nc.vector.tensor_mul(r_reg[:], rr_sum[:], ne_inv[:])

    total = cpl.tile([1, 1], F32, tag="total")
    nc.vector.scalar_tensor_tensor(total[:], el_num[:], 0.1, cls_loss[:], ALU.mult, ALU.add)
    nc.vector.scalar_tensor_tensor(total[:], r_reg[:], 0.001, total[:], ALU.mult, ALU.add)
    nc.vector.scalar_tensor_tensor(total[:], wa_sum[:], 0.01 / (C * C), total[:], ALU.mult, ALU.add)
    nc.sync.dma_start(o_total.ap().rearrange("(p o) -> p o", o=1), total[:])

    dpool_cm.__exit__(None, None, None)
    cpl_cm.__exit__(None, None, None)
    cpm_cm.__exit__(None, None, None)
    wts_cm.__exit__(None, None, None)
    base_cm.__exit__(None, None, None)
    dram_cm.__exit__(None, None, None)
    pscam_cm.__exit__(None, None, None)
    psacc_cm.__exit__(None, None, None)
    ps_cm.__exit__(None, None, None)


_CACHE = {}


def _get_compiled():
    if "nc" in _CACHE:
        return _CACHE["nc"]
    nc = bacc.Bacc("TRN2", target_bir_lowering=False, debug=False, num_devices=N_CORES)
    with tile.TileContext(nc) as tc:
        _build_program(nc, tc)
    nc.compile()
    _CACHE["nc"] = nc
    return nc


def make_in_maps(inputs):
    feats = np.ascontiguousarray(np.asarray(inputs["feats"], np.float32)).reshape(B, F, PIX)
    tgt = np.asarray(inputs["img_labels"]).astype(np.float32)
    shared = {
        "cls_logits": np.asarray(inputs["cls_logits"], np.float32),
        "tgt": tgt,
        "prior_pmi": np.asarray(inputs["prior_pmi"], np.float32),
    }
    for k in ("em_w1", "em_w2", "em_w3", "pp_b1", "pp_b2", "msg_b1", "msg_b2",
              "ed_b", "em_b1", "em_b2", "em_b3", "alpha_b", "bias_b"):
        shared[k] = np.ascontiguousarray(np.asarray(inputs[k], np.float32))
    for k in ("pp_w1", "pp_w2", "msg_w1", "msg_w2", "ed_w", "alpha_w", "bias_w"):
        shared[k] = np.ascontiguousarray(np.asarray(inputs[k], np.float32).astype(np.float16))
    shared["em_w3"] = shared["em_w3"].reshape(64, 1)
    shared["bias_w"] = shared["bias_w"].reshape(HID, 1)
    in_maps = []
    for c in range(N_CORES):
        isel = np.zeros((C, IL), np.float32)
        for r in range(IL):
            isel[IL * c + r, r] = 1.0
        m = dict(shared)
        m["feats_l"] = np.ascontiguousarray(feats[BL * c:BL * (c + 1)])
        m["iselT"] = isel
        in_maps.append(m)
    return in_maps


def run(inputs, trace=False):
    nc = _get_compiled()
    res = run_bass_kernel_spmd(
        nc, make_in_maps(inputs), core_ids=list(range(N_CORES)), trace=trace
    )
    r0 = res.results[0]
    cam = np.concatenate([res.results[c]["o_cam"] for c in range(N_CORES)], axis=0)
    out = (
        r0["o_wadj"],
        r0["o_aw"],
        r0["o_dlog"],
        cam.reshape(B, C, H, W),
        r0["o_refined"],
        np.float32(r0["o_total"].reshape(())),
    )
    return out, res


def kernel(**inputs):
    out, _ = run(inputs, trace=False)
    return out


def bench(inputs, iters=12):
    """Time the NEFF with device-resident inputs (no donation, no re-transfer)."""
    import time

    import jax
    import numpy as np_
    from jax.experimental.shard_map import shard_map
    from jax.sharding import Mesh, NamedSharding, PartitionSpec

    from concourse import bass2jax as b2j
    from concourse import mybir as mb

    nc = _get_compiled()
    b2j.install_neuronx_cc_hook()
    partition_name = nc.partition_id_tensor.name if nc.partition_id_tensor else None
    in_names, out_names, out_avals, zero_outs = [], [], [], []
    for alloc in nc.m.functions[0].allocations:
        if not isinstance(alloc, mb.MemoryLocationSet):
            continue
        name = alloc.memorylocations[0].name
        if alloc.kind == "ExternalInput":
            if name != partition_name:
                in_names.append(name)
        elif alloc.kind == "ExternalOutput":
            out_names.append(name)
            shape = tuple(alloc.tensor_shape)
            dtype = mb.dt.np(alloc.dtype)
            out_avals.append(jax.core.ShapedArray(shape, dtype))
            zero_outs.append(np_.zeros(shape, dtype))
    n_params = len(in_names)
    all_in_names = list(in_names) + list(out_names)
    if partition_name is not None:
        all_in_names.append(partition_name)

    def _body(*args):
        operands = list(args)
        if partition_name is not None:
            operands.append(b2j.partition_id_tensor())
        outs = b2j._bass_exec_p.bind(
            *operands,
            out_avals=tuple(out_avals),
            in_names=tuple(all_in_names),
            out_names=tuple(out_names),
            lowering_input_output_aliases=(),
            sim_require_finite=True,
            sim_require_nnan=True,
            nc=nc,
        )
        return tuple(outs)

    devices = jax.devices()[:N_CORES]
    mesh = Mesh(np_.asarray(devices), ("core",))
    n_outs = len(out_names)
    in_specs = (PartitionSpec("core"),) * (n_params + n_outs)
    out_specs = (PartitionSpec("core"),) * n_outs
    sharded = jax.jit(
        shard_map(_body, mesh=mesh, in_specs=in_specs, out_specs=out_specs, check_rep=False),
        keep_unused=True,
    )
    in_maps = make_in_maps(inputs)
    sh = NamedSharding(mesh, PartitionSpec("core"))
    concat_in = [
        jax.device_put(
            np_.concatenate([np_.asarray(in_maps[c][n]) for c in range(N_CORES)], axis=0), sh
        )
        for n in in_names
    ]
    concat_zeros = [
        jax.device_put(np_.zeros((N_CORES * z.shape[0], *z.shape[1:]), z.dtype), sh)
        for z in zero_outs
    ]
    out = sharded(*concat_in, *concat_zeros)
    jax.block_until_ready(out)
    times = []
    for _ in range(iters):
        t0 = time.perf_counter()
        out = sharded(*concat_in, *concat_zeros)
        jax.block_until_ready(out)
        times.append(time.perf_counter() - t0)
    return times, out, out_names


# revision 46
# speedup vs baseline: 1.4281x; 1.2657x over previous
"""CoocGNN Trainium2 kernel: 8-core SPMD, batch-parallel feats + replicated graph.

Contract: kernel(**inputs) takes FULL inputs (as produced by setup_inputs) and
returns the FULL output tuple (W_adj, aw, dlog, cam_vis, refined, total).
"""

import os
import sys

for _p in ("/opt/trn_rl_repo", os.path.expanduser("~/.axon_site/_ro/trn_rl_repo")):
    if os.path.isdir(_p) and _p not in sys.path:
        sys.path.insert(0, _p)

import numpy as np

import concourse.bacc as bacc
import concourse.tile as tile
from concourse import masks, mybir
from concourse.bass_utils import run_bass_kernel_spmd

F32 = mybir.dt.float32
F32R = mybir.dt.float32r
F16 = mybir.dt.float16
AX = mybir.AxisListType
ALU = mybir.AluOpType
ACTF = mybir.ActivationFunctionType

N_CORES = 8
B, C, F, H, W = 32, 80, 2048, 32, 32
PIX = H * W          # 1024
BL = B // N_CORES    # 4 images per core
HID = 256
EDD = 64
IL = C // N_CORES    # 10 edge-rows per core
TEMP = 2.5
BETA_POS = 0.5
GAMMA_NEG = 0.25
KF = F // 128        # 16 f-chunks
EPS = 1e-6


def _build_program(nc, tc):
    ps_cm = tc.tile_pool(name="ps", bufs=3, space="PSUM")
    ps = ps_cm.__enter__()
    psacc_cm = tc.tile_pool(name="psacc", bufs=3, space="PSUM")
    psacc = psacc_cm.__enter__()
    pscam_cm = tc.tile_pool(name="pscam", bufs=2, space="PSUM")
    pscam = pscam_cm.__enter__()
    dram_cm = tc.tile_pool(name="dram", bufs=1, space="DRAM")
    dram = dram_cm.__enter__()
    base_cm = tc.tile_pool(name="base", bufs=1)
    base = base_cm.__enter__()
    wts_cm = tc.tile_pool(name="wts", bufs=1)
    wts = wts_cm.__enter__()

    t_feats = nc.dram_tensor("feats_l", [BL, F, PIX], F32, kind="ExternalInput")
    t_cls = nc.dram_tensor("cls_logits", [B, C], F32, kind="ExternalInput")
    t_tgt = nc.dram_tensor("tgt", [B, C], F32, kind="ExternalInput")
    t_prior = nc.dram_tensor("prior_pmi", [C, C], F32, kind="ExternalInput")
    t_isel = nc.dram_tensor("iselT", [C, IL], F32, kind="ExternalInput")
    t_pp_w1 = nc.dram_tensor("pp_w1", [F, HID], F16, kind="ExternalInput")
    t_pp_w2 = nc.dram_tensor("pp_w2", [HID, HID], F16, kind="ExternalInput")
    t_msg_w1 = nc.dram_tensor("msg_w1", [HID, HID], F16, kind="ExternalInput")
    t_msg_w2 = nc.dram_tensor("msg_w2", [HID, HID], F16, kind="ExternalInput")
    t_ed_w = nc.dram_tensor("ed_w", [HID, EDD], F16, kind="ExternalInput")
    t_em_w1 = nc.dram_tensor("em_w1", [4 * EDD + 3, 128], F32, kind="ExternalInput")
    t_em_w2 = nc.dram_tensor("em_w2", [128, 64], F32, kind="ExternalInput")
    t_em_w3 = nc.dram_tensor("em_w3", [64, 1], F32, kind="ExternalInput")
    t_alpha_w = nc.dram_tensor("alpha_w", [HID, F], F16, kind="ExternalInput")
    t_bias_w = nc.dram_tensor("bias_w", [HID, 1], F16, kind="ExternalInput")
    t_pp_b1 = nc.dram_tensor("pp_b1", [HID], F32, kind="ExternalInput")
    t_pp_b2 = nc.dram_tensor("pp_b2", [HID], F32, kind="ExternalInput")
    t_msg_b1 = nc.dram_tensor("msg_b1", [HID], F32, kind="ExternalInput")
    t_msg_b2 = nc.dram_tensor("msg_b2", [HID], F32, kind="ExternalInput")
    t_ed_b = nc.dram_tensor("ed_b", [EDD], F32, kind="ExternalInput")
    t_em_b1 = nc.dram_tensor("em_b1", [128], F32, kind="ExternalInput")
    t_em_b2 = nc.dram_tensor("em_b2", [64], F32, kind="ExternalInput")
    t_em_b3 = nc.dram_tensor("em_b3", [1], F32, kind="ExternalInput")
    t_alpha_b = nc.dram_tensor("alpha_b", [F], F32, kind="ExternalInput")
    t_bias_b = nc.dram_tensor("bias_b", [1], F32, kind="ExternalInput")

    o_wadj = nc.dram_tensor("o_wadj", [C, C], F32, kind="ExternalOutput")
    o_aw = nc.dram_tensor("o_aw", [C, F], F32, kind="ExternalOutput")
    o_dlog = nc.dram_tensor("o_dlog", [C], F32, kind="ExternalOutput")
    o_refined = nc.dram_tensor("o_refined", [B, C], F32, kind="ExternalOutput")
    o_total = nc.dram_tensor("o_total", [1], F32, kind="ExternalOutput")
    o_cam = nc.dram_tensor("o_cam", [BL, C, PIX], F32, kind="ExternalOutput")

    # ---------------- constants / small inputs ----------------
    id128 = base.tile([128, 128], F32, tag="id128")
    masks.make_identity(nc, id128[:])
    ones_col = base.tile([128, 1], F32, tag="ones_col")
    nc.vector.memset(ones_col[:], 1.0)
    ones_row = base.tile([1, 128], F32, tag="ones_row")
    nc.vector.memset(ones_row[:], 1.0)
    ones_row16 = base.tile([1, 128], F16, tag="ones_row16")
    nc.vector.memset(ones_row16[:], 1.0)
    offd = base.tile([C, C], F32, tag="offd")
    nc.vector.memset(offd[:], 1.0)
    nc.vector.tensor_sub(offd[:], offd[:], id128[:C, :C])

    def load(pool, shape, dram_ap, tag, dt=F32):
        t = pool.tile(shape, dt, tag=tag)
        nc.sync.dma_start(t[:], dram_ap)
        return t

    cls_sb = load(base, [B, C], t_cls[:, :], "cls_sb")
    tgt_sb = load(base, [B, C], t_tgt[:, :], "tgt_sb")
    prior_sb = load(base, [C, C], t_prior[:, :], "prior_sb")
    isel_sb = load(base, [C, IL], t_isel[:, :], "isel_sb")

    pp_w2_sb = load(wts, [128, 2, HID], t_pp_w2.ap().rearrange("(k p) h -> p k h", p=128), "pp_w2_sb", dt=F16)
    msg_w1_sb = load(wts, [128, 2, HID], t_msg_w1.ap().rearrange("(k p) h -> p k h", p=128), "msg_w1_sb", dt=F16)
    msg_w2_sb = load(wts, [128, 2, HID], t_msg_w2.ap().rearrange("(k p) h -> p k h", p=128), "msg_w2_sb", dt=F16)
    ed_w_sb = load(wts, [128, 2, EDD], t_ed_w.ap().rearrange("(k p) h -> p k h", p=128), "ed_w_sb", dt=F16)
    bias_w_sb = load(wts, [128, 2, 1], t_bias_w.ap().rearrange("(k p) h -> p k h", p=128), "bias_w_sb", dt=F16)

    ew1_a = load(wts, [EDD, 128], t_em_w1[0:EDD, :], "ew1_a")
    ew1_b = load(wts, [EDD, 128], t_em_w1[EDD:2 * EDD, :], "ew1_b")
    ew1_fi = load(wts, [1, 128], t_em_w1[4 * EDD + 1:4 * EDD + 2, :], "ew1_fi")
    ew1_fj = load(wts, [1, 128], t_em_w1[4 * EDD + 2:4 * EDD + 3, :], "ew1_fj")
    # fp16 copies for the edge MLP (f32 staging in a short-lived pool)
    ew1_abs16 = wts.tile([EDD, 128], F16, tag="ew1_abs16")
    ew1_prod16 = wts.tile([EDD, 128], F16, tag="ew1_prod16")
    ew1_cos16 = wts.tile([1, 128], F16, tag="ew1_cos16")
    em_w2_16 = wts.tile([128, 64], F16, tag="em_w2_16")
    em_w3_16 = wts.tile([64, 1], F16, tag="em_w3_16")
    b_alpha = wts.tile([1, F], F16, tag="b_alpha")

    b_pp1 = load(wts, [1, HID], t_pp_b1.ap().rearrange("(o h) -> o h", o=1), "b_pp1")
    b_pp2 = load(wts, [1, HID], t_pp_b2.ap().rearrange("(o h) -> o h", o=1), "b_pp2")
    b_m1 = load(wts, [1, HID], t_msg_b1.ap().rearrange("(o h) -> o h", o=1), "b_m1")
    b_m2 = load(wts, [1, HID], t_msg_b2.ap().rearrange("(o h) -> o h", o=1), "b_m2")
    b_ed = load(wts, [1, EDD], t_ed_b.ap().rearrange("(o h) -> o h", o=1), "b_ed")
    b_em1 = load(wts, [128, 1], t_em_b1.ap().rearrange("(p o) -> p o", o=1), "b_em1")
    b_em2 = load(wts, [64, 1], t_em_b2.ap().rearrange("(p o) -> p o", o=1), "b_em2")
    b_em3 = load(wts, [1, 1], t_em_b3.ap().rearrange("(p o) -> p o", o=1), "b_em3")
    b_biasb = load(wts, [1, 1], t_bias_b.ap().rearrange("(p o) -> p o", o=1), "b_biasb")

    # cpm: mid-lived tensors that survive into late phase C
    cpm_cm = tc.tile_pool(name="cpm", bufs=1)
    cpm = cpm_cm.__enter__()

    # pp_w1 lives in its own pool; released after Hn1 so alpha_w can reuse it.
    w1pool_cm = tc.tile_pool(name="w1pool", bufs=1)
    w1pool = w1pool_cm.__enter__()
    pp_w1_sb = w1pool.tile([128, KF, HID], F16, tag="pp_w1_sb")
    nc.sync.dma_start(pp_w1_sb[:], t_pp_w1.ap().rearrange("(p q) h -> p q h", q=KF))

    # f32 staging for the fp16 weight copies; freed before phase A pressure
    with tc.tile_pool(name="stg", bufs=1) as stg:
        ew1_abs = load(stg, [EDD, 128], t_em_w1[2 * EDD:3 * EDD, :], "ew1_abs")
        nc.scalar.copy(ew1_abs16[:], ew1_abs[:])
        ew1_prod = load(stg, [EDD, 128], t_em_w1[3 * EDD:4 * EDD, :], "ew1_prod")
        nc.scalar.copy(ew1_prod16[:], ew1_prod[:])
        ew1_cos = load(stg, [1, 128], t_em_w1[4 * EDD:4 * EDD + 1, :], "ew1_cos")
        nc.scalar.copy(ew1_cos16[:], ew1_cos[:])
        em_w2_sb = load(stg, [128, 64], t_em_w2[:, :], "em_w2_sb")
        nc.scalar.copy(em_w2_16[:], em_w2_sb[:])
        em_w3_sb = load(stg, [64, 1], t_em_w3[:, :], "em_w3_sb")
        nc.scalar.copy(em_w3_16[:], em_w3_sb[:])
        b_alpha32 = load(stg, [1, F], t_alpha_b.ap().rearrange("(o h) -> o h", o=1), "b_alpha32")
        nc.scalar.copy(b_alpha[:], b_alpha32[:])

    # ---------------- phase A: stream feats, stash fp16, pool ----------------
    # f-to-partition mapping is f = 16*p + q: each partition owns 16
    # consecutive feature rows, so every DMA descriptor is 16 KiB contiguous.
    stash = base.tile([128, BL, KF, PIX], F16, tag="stash")
    pooled_sb = base.tile([128, BL, KF], F32, tag="pooled_sb")
    pooled_loc = dram.tile([BL, F], F32)
    pooled_full_d = dram.tile([B, F], F32)

    def stash_slice(b, q, n0, n1):
        return stash[:, b, q, n0:n1]

    with tc.tile_pool(name="ina", bufs=3) as ina:
        for b in range(BL):
            for j in range(4):
                tin = ina.tile([128, 4, PIX], F32, tag="tin")
                nc.sync.dma_start(
                    tin[:],
                    t_feats[b, :, :].rearrange("(p q) n -> p q n", q=KF)[:, 4 * j:4 * (j + 1), :],
                )
                nc.scalar.copy(stash[:, b, 4 * j:4 * (j + 1), :], tin[:])
                nc.vector.tensor_reduce(
                    pooled_sb[:, b, 4 * j:4 * (j + 1)], tin[:], axis=AX.X, op=ALU.add
                )
            nc.sync.dma_start(
                pooled_loc[:][b].rearrange("(p q) -> p q", q=KF), pooled_sb[:, b, :]
            )

    nc.gpsimd.collective_compute(
        "AllGather", ALU.bypass,
        replica_groups=[list(range(N_CORES))],
        ins=[pooled_loc.opt()], outs=[pooled_full_d.opt()],
    )

    # ---------------- phase C (early): proto / cos / Hn1 ----------------
    cpe_cm = tc.tile_pool(name="cpe", bufs=1)
    cpe = cpe_cm.__enter__()

    pooled_full = cpe.tile([B, F], F32, tag="pooled_full")
    nc.sync.dma_start(pooled_full[:], pooled_full_d[:])

    def sigmoid(out_ap, in_ap, scale=1.0):
        # 1 / (1 + exp(-x*scale)) via Exp + DVE reciprocal (single ACT table)
        nc.scalar.activation(out_ap, in_ap, ACTF.Exp, scale=-scale)
        nc.vector.tensor_scalar_add(out_ap, out_ap, 1.0)
        nc.vector.reciprocal(out_ap, out_ap)

    def softplus_neg(out_ap, in_ap):
        # log1p(exp(-x)) for x >= 0
        nc.scalar.activation(out_ap, in_ap, ACTF.Exp, scale=-1.0)
        nc.scalar.activation(out_ap, out_ap, ACTF.Ln, bias=1.0)

    probs = base.tile([B, C], F32, tag="probs")
    sigmoid(probs[:], cls_sb[:])
    y_sb = base.tile([B, C], F32, tag="y_sb")
    nc.vector.tensor_scalar_max(y_sb[:], tgt_sb[:], 0.0)
    probs16 = cpe.tile([B, C], F16, tag="probs16")
    nc.scalar.copy(probs16[:], probs[:])
    pooled16 = cpe.tile([B, F], F16, tag="pooled16")
    nc.vector.tensor_copy(pooled16[:], pooled_full[:])

    # weight_sum / freq / present
    ws_ps = ps.tile([C, 1], F32, tag="ps")
    nc.tensor.matmul(ws_ps[:], probs[:], ones_col[:B, :])
    wsum = cpm.tile([C, 1], F32, tag="wsum")
    nc.scalar.copy(wsum[:], ws_ps[:])
    freq_col = cpm.tile([C, 1], F32, tag="freq_col")
    nc.scalar.mul(freq_col[:], wsum[:], 1.0 / B)
    wclamp = cpm.tile([C, 1], F32, tag="wclamp")
    nc.vector.tensor_scalar_max(wclamp[:], wsum[:], EPS)
    winv = cpm.tile([C, 1], F32, tag="winv")
    nc.vector.reciprocal(winv[:], wclamp[:])
    winv_eff = cpm.tile([C, 1], F32, tag="winv_eff")
    nc.scalar.mul(winv_eff[:], winv[:], 1.0 / PIX)
    # row of wsum*PIX (to inject exact pp_b1 under the later winv_eff scaling)
    wsp_col = cpm.tile([C, 1], F32, tag="wsp_col")
    nc.scalar.mul(wsp_col[:], wclamp[:], float(PIX))
    wsp_ps = ps.tile([1, C], F32, tag="ps")
    nc.tensor.transpose(wsp_ps[:], wsp_col[:], id128[:C, :C])
    wsp_row = cpm.tile([1, C], F32, tag="wsp_row")
    nc.vector.tensor_copy(wsp_row[:], wsp_ps[:])

    ys_ps = ps.tile([C, 1], F32, tag="ps")
    nc.tensor.matmul(ys_ps[:], y_sb[:], ones_col[:B, :])
    present = cpm.tile([C, 1], F32, tag="present")
    nc.vector.tensor_single_scalar(present[:], ys_ps[:], 0.5, ALU.is_gt)

    # proto row norms via Square+accum (proto left unnormalized, scales folded)
    nrm_part = cpe.tile([C, 4], F32, tag="nrm_part")
    sq_scr = cpe.tile([C, 512], F32, tag="sq_scr")
    for j in range(4):
        pp_ = psacc.tile([C, 512], F32, tag="acc")
        nc.tensor.matmul(
            pp_[:], probs16[:],
            pooled16[:, 512 * j:512 * (j + 1)],
        )
        nc.scalar.activation(
            sq_scr[:], pp_[:], ACTF.Square, accum_out=nrm_part[:, j:j + 1]
        )
    nrm_sq = cpm.tile([C, 1], F32, tag="nrm_sq")
    nc.vector.tensor_reduce(nrm_sq[:], nrm_part[:], axis=AX.X, op=ALU.add)
    nrm = cpm.tile([C, 1], F32, tag="nrm")
    nc.scalar.activation(nrm[:], nrm_sq[:], ACTF.Ln)
    nc.scalar.activation(nrm[:], nrm[:], ACTF.Exp, scale=0.5)  # sqrt
    nc.vector.tensor_scalar_max(nrm[:], nrm[:], EPS)
    inv_u = cpm.tile([C, 1], F32, tag="inv_u")
    nc.vector.reciprocal(inv_u[:], nrm[:])

    # protoT (f-major), G, cos
    protoT = cpe.tile([128, KF, C], F16, tag="protoT")
    pooled16_q = pooled16[:].rearrange("b (p q) -> b q p", q=KF)
    for k in range(KF):
        pt_ = ps.tile([128, C], F32, tag="ps")
        nc.tensor.matmul(pt_[:], pooled16_q[:, k, :], probs16[:])
        if k % 2 == 0:
            nc.vector.tensor_copy(protoT[:, k, :], pt_[:])
        else:
            nc.scalar.copy(protoT[:, k, :], pt_[:])
    g_ps = psacc.tile([C, C], F32, tag="acc")
    for k in range(KF):
        nc.tensor.matmul(
            g_ps[:], protoT[:, k, :], protoT[:, k, :],
            start=(k == 0), stop=(k == KF - 1),
        )
    ir_ps = ps.tile([1, C], F32, tag="ps")
    nc.tensor.transpose(ir_ps[:], inv_u[:], id128[:C, :C])
    inv_row = cpm.tile([1, C], F32, tag="inv_row")
    nc.vector.tensor_copy(inv_row[:], ir_ps[:])
    s_ps = ps.tile([C, C], F32, tag="ps")
    nc.tensor.matmul(s_ps[:], inv_row[:], inv_row[:])
    cos_sb = cpm.tile([C, C], F32, tag="cos_sb")
    nc.scalar.copy(cos_sb[:], g_ps[:])
    nc.vector.tensor_tensor(cos_sb[:], cos_sb[:], s_ps[:], ALU.mult)
    nc.vector.tensor_scalar(cos_sb[:], cos_sb[:], 1.0, -1.0, ALU.min, ALU.max)

    # Hn1 = relu((proto_u @ pp_w1 + pp_b1*wsum*PIX) * winv_eff)
    h1_ps = psacc.tile([C, HID], F32, tag="acc")
    for k in range(KF):
        nc.tensor.matmul(
            h1_ps[:], protoT[:, k, :], pp_w1_sb[:, k, :],
            start=(k == 0), stop=False,
        )
    nc.tensor.matmul(h1_ps[:], wsp_row[:], b_pp1[:], start=False, stop=True)
    relu1 = cpe.tile([C, HID], F32, tag="relu1")
    nc.vector.tensor_scalar_mul(relu1[:], h1_ps[:], winv_eff[:])
    nc.scalar.activation(relu1[:], relu1[:], ACTF.Relu)

    r1T = cpm.tile([128, 2, C], F16, tag="r1T")
    for j in range(2):
        tp_ = ps.tile([128, C], F32, tag="ps")
        nc.tensor.transpose(tp_[:], relu1[:, 128 * j:128 * (j + 1)], id128[:C, :C])
        nc.scalar.copy(r1T[:, j, :], tp_[:])

    # release pp_w1 + early tensors (LIFO); open late pools
    cpe_cm.__exit__(None, None, None)
    w1pool_cm.__exit__(None, None, None)

    cpl_cm = tc.tile_pool(name="cpl", bufs=1)
    cpl = cpl_cm.__enter__()
    awpool_cm = tc.tile_pool(name="awpool", bufs=1)
    awpool = awpool_cm.__enter__()
    alpha_w_sb = awpool.tile([128, 2, F], F16, tag="alpha_w_sb")
    nc.sync.dma_start(alpha_w_sb[:], t_alpha_w.ap().rearrange("(k p) h -> p k h", p=128))

    def transpose_pair(src, tag="tpair"):
        # src [C, 256] -> dst [128, 2, C]
        dst = cpl.tile([128, 2, C], F16, tag=tag)
        for j in range(2):
            tp_ = ps.tile([128, C], F32, tag="ps")
            nc.tensor.transpose(tp_[:], src[:, 128 * j:128 * (j + 1)], id128[:C, :C])
            nc.scalar.copy(dst[:, j, :], tp_[:])
        return dst

    hn_ps = psacc.tile([C, HID], F32, tag="acc")
    for j in range(2):
        nc.tensor.matmul(
            hn_ps[:], r1T[:, j, :], pp_w2_sb[:, j, :],
            start=(j == 0), stop=False,
        )
    nc.tensor.matmul(hn_ps[:], ones_row[:1, :C], b_pp2[:], start=False, stop=True)
    hn_sb = cpl.tile([C, HID], F32, tag="hn_sb")
    nc.scalar.copy(hn_sb[:], hn_ps[:])

    hnT = transpose_pair(hn_sb)
    zd_ps = psacc.tile([C, EDD], F32, tag="acc")
    for j in range(2):
        nc.tensor.matmul(zd_ps[:], hnT[:, j, :], ed_w_sb[:, j, :], start=(j == 0), stop=False)
    nc.tensor.matmul(zd_ps[:], ones_row[:1, :C], b_ed[:], start=False, stop=True)
    zd_sb = cpl.tile([C, EDD], F32, tag="zd_sb")
    nc.scalar.activation(zd_sb[:], zd_ps[:], ACTF.Relu)

    zdT_ps = ps.tile([EDD, C], F32, tag="ps")
    nc.tensor.transpose(zdT_ps[:], zd_sb[:], id128[:C, :C])
    zdT = cpl.tile([EDD, C], F32, tag="zdT")
    nc.vector.tensor_copy(zdT[:], zdT_ps[:])

    # local (sharded) pieces via iselT
    zdl_ps = ps.tile([IL, EDD], F32, tag="ps")
    nc.tensor.matmul(zdl_ps[:], isel_sb[:], zd_sb[:])
    zdl = cpl.tile([IL, EDD], F32, tag="zdl")
    nc.vector.tensor_copy(zdl[:], zdl_ps[:])
    zdTl_ps = ps.tile([EDD, IL], F32, tag="ps")
    nc.tensor.transpose(zdTl_ps[:], zdl[:], id128[:IL, :IL])
    zdTl = cpl.tile([EDD, IL], F32, tag="zdTl")
    nc.vector.tensor_copy(zdTl[:], zdTl_ps[:])

    fl_ps = ps.tile([IL, 1], F32, tag="ps")
    nc.tensor.matmul(fl_ps[:], isel_sb[:], freq_col[:])
    fl_sb = cpl.tile([IL, 1], F32, tag="fl_sb")
    nc.vector.tensor_copy(fl_sb[:], fl_ps[:])
    flr_ps = ps.tile([1, IL], F32, tag="ps")
    nc.tensor.transpose(flr_ps[:], fl_sb[:], id128[:IL, :IL])
    flr = cpl.tile([1, IL], F32, tag="flr")
    nc.vector.tensor_copy(flr[:], flr_ps[:])
    fr_ps = ps.tile([1, C], F32, tag="ps")
    nc.tensor.transpose(fr_ps[:], freq_col[:], id128[:C, :C])
    fr_sb = cpl.tile([1, C], F32, tag="fr_sb")
    nc.vector.tensor_copy(fr_sb[:], fr_ps[:])

    # PT_loc (+ freq_i term), QT (+ freq_j term)
    ptl_ps = ps.tile([128, IL], F32, tag="ps")
    nc.tensor.matmul(ptl_ps[:], ew1_a[:], zdTl[:], start=True, stop=False)
    nc.tensor.matmul(ptl_ps[:], ew1_fi[:], flr[:], start=False, stop=True)
    ptl = cpl.tile([128, IL], F32, tag="ptl")
    nc.vector.tensor_copy(ptl[:], ptl_ps[:])
    qt_ps = ps.tile([128, C], F32, tag="ps")
    nc.tensor.matmul(qt_ps[:], ew1_b[:], zdT[:], start=True, stop=False)
    nc.tensor.matmul(qt_ps[:], ew1_fj[:], fr_sb[:], start=False, stop=True)
    qt = cpl.tile([128, C], F32, tag="qt")
    nc.scalar.copy(qt[:], qt_ps[:])
    ptq = cpl.tile([128, IL, C], F16, tag="ptq")
    nc.vector.tensor_tensor(
        ptq[:],
        ptl[:].unsqueeze(2).broadcast_to([128, IL, C]),
        qt[:].unsqueeze(1).broadcast_to([128, IL, C]),
        ALU.add,
    )

    cosl_ps = ps.tile([IL, C], F32, tag="ps")
    nc.tensor.matmul(cosl_ps[:], isel_sb[:], cos_sb[:])
    cosl16 = cpl.tile([IL, C], F16, tag="cosl16")
    nc.scalar.copy(cosl16[:], cosl_ps[:])
    cos_flat = cpl.tile([1, IL * C], F16, tag="cos_flat")
    nc.sync.dma_start(cos_flat[:], cosl16[:])

    # edge feature blocks: |zi-zj| (in place) and zi*zj, fp16
    absblk = cpl.tile([EDD, IL, C], F16, tag="absblk")
    nc.vector.tensor_tensor(
        absblk[:],
        zdTl[:].unsqueeze(2).broadcast_to([EDD, IL, C]),
        zdT[:].unsqueeze(1).broadcast_to([EDD, IL, C]),
        ALU.subtract,
    )
    nc.scalar.activation(absblk[:], absblk[:], ACTF.Abs)
    prodblk = cpl.tile([EDD, IL, C], F16, tag="prodblk")
    nc.vector.tensor_tensor(
        prodblk[:],
        zdTl[:].unsqueeze(2).broadcast_to([EDD, IL, C]),
        zdT[:].unsqueeze(1).broadcast_to([EDD, IL, C]),
        ALU.mult,
    )

    nedge = IL * C
    abs_flat = absblk[:].rearrange("p r j -> p (r j)")
    prod_flat = prodblk[:].rearrange("p r j -> p (r j)")
    ptqf = ptq[:].rearrange("p r j -> p (r j)")

    e1T = cpl.tile([128, nedge], F16, tag="e1T")
    for n0, n1 in ((0, 512), (512, nedge)):
        e1_ps = psacc.tile([128, 512], F32, tag="acc")
        nc.tensor.matmul(e1_ps[:, :n1 - n0], ew1_abs16[:], abs_flat[:, n0:n1], start=True, stop=False)
        nc.tensor.matmul(e1_ps[:, :n1 - n0], ew1_prod16[:], prod_flat[:, n0:n1], start=False, stop=False)
        nc.tensor.matmul(e1_ps[:, :n1 - n0], ew1_cos16[:], cos_flat[:, n0:n1], start=False, stop=True)
        nc.vector.tensor_tensor(e1_ps[:, :n1 - n0], e1_ps[:, :n1 - n0], ptqf[:, n0:n1], ALU.add)
        nc.scalar.activation(e1T[:, n0:n1], e1_ps[:, :n1 - n0], ACTF.Relu, bias=b_em1[:])

    e2T = cpl.tile([EDD, nedge], F16, tag="e2T")
    for n0, n1 in ((0, 512), (512, nedge)):
        e2_ps = psacc.tile([EDD, 512], F32, tag="acc")
        nc.tensor.matmul(e2_ps[:, :n1 - n0], em_w2_16[:], e1T[:, n0:n1])
        nc.scalar.activation(e2T[:, n0:n1], e2_ps[:, :n1 - n0], ACTF.Relu, bias=b_em2[:])

    r_sb = cpl.tile([1, nedge], F32, tag="r_sb")
    for n0, n1 in ((0, 512), (512, nedge)):
        r_ps = ps.tile([1, 512], F32, tag="ps")
        nc.tensor.matmul(r_ps[:, :n1 - n0], em_w3_16[:], e2T[:, n0:n1])
        nc.scalar.activation(r_sb[:, n0:n1], r_ps[:, :n1 - n0], ACTF.Identity, bias=b_em3[:])

    r_loc_d = dram.tile([IL, C], F32)
    r_full_d = dram.tile([C, C], F32)
    nc.sync.dma_start(r_loc_d[:], r_sb[:])
    nc.gpsimd.collective_compute(
        "AllGather", ALU.bypass,
        replica_groups=[list(range(N_CORES))],
        ins=[r_loc_d.opt()], outs=[r_full_d.opt()],
    )
    r_full = cpl.tile([C, C], F32, tag="r_full")
    nc.sync.dma_start(r_full[:], r_full_d[:])

    # W_adj
    h_sb = cpl.tile([C, C], F32, tag="h_sb")
    nc.vector.scalar_tensor_tensor(h_sb[:], prior_sb[:], 1.0 / TEMP, r_full[:], ALU.mult, ALU.add)
    w_sg = cpl.tile([C, C], F32, tag="w_sg")
    sigmoid(w_sg[:], h_sb[:])
    wt_ps = ps.tile([C, C], F32, tag="ps")
    nc.tensor.transpose(wt_ps[:], w_sg[:], id128[:C, :C])
    w_half = cpl.tile([C, C], F32, tag="w_half")
    nc.scalar.mul(w_half[:], wt_ps[:], 0.5)
    nc.vector.scalar_tensor_tensor(w_half[:], w_sg[:], 0.5, w_half[:], ALU.mult, ALU.add)
    wadj = cpl.tile([C, C], F32, tag="wadj")
    nc.vector.tensor_mul(wadj[:], w_half[:], offd[:])
    nc.sync.dma_start(o_wadj[:, :], wadj[:])

    rowsum = cpl.tile([C, 1], F32, tag="rowsum")
    nc.vector.tensor_reduce(rowsum[:], wadj[:], axis=AX.X, op=ALU.add)
    nc.vector.tensor_scalar_max(rowsum[:], rowsum[:], EPS)
    rinv = cpl.tile([C, 1], F32, tag="rinv")
    nc.vector.reciprocal(rinv[:], rowsum[:])
    rr_ps = ps.tile([1, C], F32, tag="ps")
    nc.tensor.transpose(rr_ps[:], rinv[:], id128[:C, :C])
    rinv_row = cpl.tile([1, C], F32, tag="rinv_row")
    nc.vector.tensor_copy(rinv_row[:], rr_ps[:])
    rb_ps = ps.tile([C, C], F32, tag="ps")
    nc.tensor.matmul(rb_ps[:], ones_row[:1, :C], rinv_row[:])
    at_sb = cpl.tile([C, C], F32, tag="at_sb")
    nc.vector.tensor_tensor(at_sb[:], wadj[:], rb_ps[:], ALU.mult)

    # message passing (1 step)
    m1T = cpl.tile([128, 2, C], F16, tag="tpair")
    for j in range(2):
        mp_ = ps.tile([128, C], F32, tag="ps")
        nc.tensor.matmul(mp_[:], hn_sb[:, 128 * j:128 * (j + 1)], at_sb[:])
        nc.scalar.copy(m1T[:, j, :], mp_[:])
    m1_ps = psacc.tile([C, HID], F32, tag="acc")
    for j in range(2):
        nc.tensor.matmul(
            m1_ps[:], m1T[:, j, :], msg_w1_sb[:, j, :],
            start=(j == 0), stop=False,
        )
    nc.tensor.matmul(m1_ps[:], ones_row[:1, :C], b_m1[:], start=False, stop=True)
    mr_sb = cpl.tile([C, HID], F32, tag="mr_sb")
    nc.scalar.activation(mr_sb[:], m1_ps[:], ACTF.Relu)
    mrT = transpose_pair(mr_sb)
    m2_ps = psacc.tile([C, HID], F32, tag="acc")
    for j in range(2):
        nc.tensor.matmul(
            m2_ps[:], mrT[:, j, :], msg_w2_sb[:, j, :],
            start=(j == 0), stop=False,
        )
    nc.tensor.matmul(m2_ps[:], ones_row[:1, :C], b_m2[:], start=False, stop=True)
    zn_sb = cpl.tile([C, HID], F32, tag="zn_sb")
    nc.vector.tensor_tensor(zn_sb[:], m2_ps[:], hn_sb[:], ALU.add)
    nc.scalar.activation(zn_sb[:], zn_sb[:], ACTF.Relu)
    znT = transpose_pair(zn_sb, tag="znT")

    # aw head
    aw_sb = cpl.tile([C, F], F32, tag="aw_sb")
    aws_part = cpl.tile([C, 4], F32, tag="aws_part")
    for n in range(4):
        a_ps = psacc.tile([C, 512], F32, tag="acc")
        for j in range(2):
            nc.tensor.matmul(
                a_ps[:], znT[:, j, :],
                alpha_w_sb[:, j, 512 * n:512 * (n + 1)],
                start=(j == 0), stop=False,
            )
        nc.tensor.matmul(a_ps[:], ones_row16[:1, :C], b_alpha[:, 512 * n:512 * (n + 1)], start=False, stop=True)
        # softplus(x) = ln(exp(x) + 1)
        nc.scalar.activation(aw_sb[:, 512 * n:512 * (n + 1)], a_ps[:], ACTF.Exp)
        nc.scalar.activation(
            aw_sb[:, 512 * n:512 * (n + 1)], aw_sb[:, 512 * n:512 * (n + 1)],
            ACTF.Ln, bias=1.0,
            accum_out=aws_part[:, n:n + 1],
        )
    aws = cpl.tile([C, 1], F32, tag="aws")
    nc.vector.tensor_reduce(aws[:], aws_part[:], axis=AX.X, op=ALU.add)
    nc.vector.tensor_scalar_max(aws[:], aws[:], EPS)
    sinv = cpl.tile([C, 1], F32, tag="sinv")
    nc.vector.reciprocal(sinv[:], aws[:])
    nc.vector.tensor_scalar_mul(aw_sb[:], aw_sb[:], sinv[:])
    nc.sync.dma_start(o_aw[:, :], aw_sb[:])

    awT16 = cpl.tile([128, KF, C], F16, tag="awT16")
    aw_q = aw_sb[:].rearrange("c (p q) -> c q p", q=KF)
    for k in range(KF):
        at_ps = ps.tile([128, C], F32, tag="ps")
        nc.tensor.transpose(at_ps[:], aw_q[:, k, :], id128[:C, :C])
        nc.scalar.copy(awT16[:, k, :], at_ps[:])

    awpool_cm.__exit__(None, None, None)

    # dlog
    dl_ps = ps.tile([C, 1], F32, tag="ps")
    for j in range(2):
        nc.tensor.matmul(dl_ps[:], znT[:, j, :], bias_w_sb[:, j, :], start=(j == 0), stop=False)
    nc.tensor.matmul(dl_ps[:], ones_row[:1, :C], b_biasb[:], start=False, stop=True)
    dlog_sb = cpl.tile([C, 1], F32, tag="dlog_sb")
    nc.scalar.copy(dlog_sb[:], dl_ps[:])
    nc.sync.dma_start(o_dlog.ap().rearrange("(p o) -> p o", o=1), dlog_sb[:])
    dlr_ps = ps.tile([1, C], F32, tag="ps")
    nc.tensor.transpose(dlr_ps[:], dlog_sb[:], id128[:C, :C])
    dlr = cpl.tile([1, C], F32, tag="dlr")
    nc.vector.tensor_copy(dlr[:], dlr_ps[:])

    # refined
    py = cpl.tile([B, C], F32, tag="py")
    nc.vector.tensor_mul(py[:], probs[:], y_sb[:])
    pyT_ps = ps.tile([C, B], F32, tag="ps")
    nc.tensor.transpose(pyT_ps[:], py[:], id128[:B, :B])
    pyT = cpl.tile([C, B], F32, tag="pyT")
    nc.vector.tensor_copy(pyT[:], pyT_ps[:])
    pa_ps = ps.tile([B, C], F32, tag="ps")
    nc.tensor.matmul(pa_ps[:], pyT[:], wadj[:])
    nc.vector.tensor_mul(py[:], y_sb[:], pa_ps[:])  # py := pos_agg
    dlb_ps = ps.tile([B, C], F32, tag="ps")
    nc.tensor.matmul(dlb_ps[:], ones_row[:1, :B], dlr[:])
    prow = cpl.tile([B, 1], F32, tag="prow")
    nc.vector.tensor_reduce(prow[:], probs[:], axis=AX.X, op=ALU.add)
    nc.scalar.mul(prow[:], prow[:], -GAMMA_NEG)
    refined = cpl.tile([B, C], F32, tag="refined")
    nc.vector.scalar_tensor_tensor(
        refined[:], py[:], BETA_POS + GAMMA_NEG, dlb_ps[:], ALU.mult, ALU.add
    )
    nc.vector.tensor_scalar_add(refined[:], refined[:], prow[:])
    nc.vector.tensor_add(refined[:], refined[:], cls_sb[:])
    nc.sync.dma_start(o_refined[:, :], refined[:])

    # ---------------- phase D: CAM ----------------
    dpool_cm = tc.tile_pool(name="dpool", bufs=2)
    dpool = dpool_cm.__enter__()
    for b in range(BL):
        cam = dpool.tile([C, PIX], F32, tag="cam")
        for hh in range(2):
            c_ps = pscam.tile([C, 512], F32, tag="cam_ps")
            for k in range(KF):
                nc.tensor.matmul(
                    c_ps[:], awT16[:, k, :], stash_slice(b, k, 512 * hh, 512 * (hh + 1)),
                    start=(k == 0), stop=(k == KF - 1),
                )
            nc.scalar.activation(cam[:, 512 * hh:512 * (hh + 1)], c_ps[:], ACTF.Relu)
        mn = dpool.tile([C, 1], F32, tag="mn")
        nc.vector.tensor_reduce(mn[:], cam[:], axis=AX.X, op=ALU.min)
        mx = dpool.tile([C, 1], F32, tag="mx")
        nc.vector.tensor_reduce(mx[:], cam[:], axis=AX.X, op=ALU.max)
        nc.vector.tensor_sub(mx[:], mx[:], mn[:])
        nc.vector.tensor_scalar_add(mx[:], mx[:], EPS)
        dinv = dpool.tile([C, 1], F32, tag="dinv")
        nc.vector.reciprocal(dinv[:], mx[:])
        nc.vector.tensor_scalar(cam[:], cam[:], mn[:], dinv[:], ALU.subtract, ALU.mult)
        nc.sync.dma_start(o_cam[b, :, :], cam[:])

    # ---------------- losses ----------------
    def colsum_1x1(src_col, nrows, tag):
        p_ = ps.tile([1, 1], F32, tag="ps")
        nc.tensor.matmul(p_[:], src_col[:], ones_col[:nrows, :])
        out = cpl.tile([1, 1], F32, tag=tag)
        nc.vector.tensor_copy(out[:], p_[:])
        return out

    # cls loss: bce = relu(h) - h*t + softplus(-|h|), masked mean
    m_sb = cpl.tile([B, C], F32, tag="m_sb")
    nc.vector.tensor_single_scalar(m_sb[:], tgt_sb[:], -1.0, ALU.not_equal)
    safe_t = cpl.tile([B, C], F32, tag="safe_t")
    nc.vector.tensor_mul(safe_t[:], tgt_sb[:], m_sb[:])
    nc.vector.tensor_mul(safe_t[:], refined[:], safe_t[:])  # safe_t := h*t
    rh = cpl.tile([B, C], F32, tag="rh")
    nc.scalar.activation(rh[:], refined[:], ACTF.Relu)
    ab = cpl.tile([B, C], F32, tag="ab")
    nc.scalar.activation(ab[:], refined[:], ACTF.Abs)
    softplus_neg(ab[:], ab[:])  # ab := softplus(-|h|)
    nc.vector.tensor_sub(rh[:], rh[:], safe_t[:])
    nc.vector.tensor_add(rh[:], rh[:], ab[:])
    nc.vector.tensor_mul(rh[:], rh[:], m_sb[:])  # rh := bce * m
    bm_rows = cpl.tile([B, 1], F32, tag="bm_rows")
    nc.vector.tensor_reduce(bm_rows[:], rh[:], axis=AX.X, op=ALU.add)
    bce_sum = colsum_1x1(bm_rows, B, "bce_sum")
    nc.vector.tensor_reduce(bm_rows[:], m_sb[:], axis=AX.X, op=ALU.add)
    m_sum = colsum_1x1(bm_rows, B, "m_sum")
    nc.vector.tensor_scalar_max(m_sum[:], m_sum[:], 1.0)
    m_inv = cpl.tile([1, 1], F32, tag="m_inv")
    nc.vector.reciprocal(m_inv[:], m_sum[:])
    cls_loss = cpl.tile([1, 1], F32, tag="cls_loss")
    nc.vector.tensor_mul(cls_loss[:], bce_sum[:], m_inv[:])

    # edge mask
    pr_ps = ps.tile([1, C], F32, tag="ps")
    nc.tensor.transpose(pr_ps[:], present[:], id128[:C, :C])
    pres_row = cpl.tile([1, C], F32, tag="pres_row")
    nc.vector.tensor_copy(pres_row[:], pr_ps[:])
    ppo_ps = ps.tile([C, C], F32, tag="ps")
    nc.tensor.matmul(ppo_ps[:], pres_row[:], pres_row[:])
    emf = cpl.tile([C, C], F32, tag="emf")
    nc.vector.tensor_tensor(emf[:], offd[:], ppo_ps[:], ALU.mult)

    # edge bce over h vs soft targets t = sigmoid(prior/TEMP)
    te_sb = cpl.tile([C, C], F32, tag="te_sb")
    sigmoid(te_sb[:], prior_sb[:], scale=1.0 / TEMP)
    nc.vector.tensor_mul(te_sb[:], h_sb[:], te_sb[:])  # te_sb := h*t
    rhe = cpl.tile([C, C], F32, tag="rhe")
    nc.scalar.activation(rhe[:], h_sb[:], ACTF.Relu)
    abe = cpl.tile([C, C], F32, tag="abe")
    nc.scalar.activation(abe[:], h_sb[:], ACTF.Abs)
    softplus_neg(abe[:], abe[:])
    nc.vector.tensor_sub(rhe[:], rhe[:], te_sb[:])
    nc.vector.tensor_add(rhe[:], rhe[:], abe[:])   # rhe := bce_e
    nc.vector.tensor_mul(rhe[:], rhe[:], emf[:])   # rhe := bce_e * emf

    posf = cpl.tile([C, C], F32, tag="posf")
    nc.vector.tensor_single_scalar(posf[:], prior_sb[:], 0.0, ALU.is_gt)

    rcol = cpl.tile([C, 1], F32, tag="rcol")
    nc.vector.tensor_reduce(rcol[:], emf[:], axis=AX.X, op=ALU.add)
    n_edges = colsum_1x1(rcol, C, "n_edges")
    nc.vector.tensor_mul(abe[:], posf[:], emf[:])  # abe := posf*emf
    nc.vector.tensor_reduce(rcol[:], abe[:], axis=AX.X, op=ALU.add)
    n_pos = colsum_1x1(rcol, C, "n_pos")
    nc.vector.tensor_reduce(rcol[:], rhe[:], axis=AX.X, op=ALU.add)
    s1 = colsum_1x1(rcol, C, "s1")
    nc.vector.tensor_mul(abe[:], rhe[:], posf[:])  # abe := bce_e*emf*posf
    nc.vector.tensor_reduce(rcol[:], abe[:], axis=AX.X, op=ALU.add)
    s2 = colsum_1x1(rcol, C, "s2")

    nc.scalar.activation(abe[:], r_full[:], ACTF.Abs)
    nc.vector.tensor_mul(abe[:], abe[:], emf[:])   # abe := |r|*emf
    nc.vector.tensor_reduce(rcol[:], abe[:], axis=AX.X, op=ALU.add)
    rr_sum = colsum_1x1(rcol, C, "rr_sum")
    nc.vector.tensor_reduce(rcol[:], wadj[:], axis=AX.X, op=ALU.add)
    wa_sum = colsum_1x1(rcol, C, "wa_sum")

    n_pos_c = cpl.tile([1, 1], F32, tag="n_pos_c")
    nc.vector.tensor_scalar_max(n_pos_c[:], n_pos[:], 1.0)
    n_neg = cpl.tile([1, 1], F32, tag="n_neg")
    nc.vector.tensor_sub(n_neg[:], n_edges[:], n_pos[:])
    nc.vector.tensor_scalar_max(n_neg[:], n_neg[:], 1.0)
    np_inv = cpl.tile([1, 1], F32, tag="np_inv")
    nc.vector.reciprocal(np_inv[:], n_pos_c[:])
    w_pos = cpl.tile([1, 1], F32, tag="w_pos")
    nc.vector.tensor_mul(w_pos[:], n_neg[:], np_inv[:])
    nc.vector.tensor_scalar(w_pos[:], w_pos[:], 1.0, 10.0, ALU.max, ALU.min)
    nc.vector.tensor_scalar_add(w_pos[:], w_pos[:], -1.0)  # w_pos := w_pos - 1

    nc.vector.tensor_scalar_max(n_edges[:], n_edges[:], 1.0)
    ne_inv = cpl.tile([1, 1], F32, tag="ne_inv")
    nc.vector.reciprocal(ne_inv[:], n_edges[:])

    # edge_loss = (s1 + (w_pos-1)*s2) / n_edges ; r_reg = 0.001*rr_sum/n_edges
    el_num = cpl.tile([1, 1], F32, tag="el_num")
    nc.vector.tensor_mul(el_num[:], w_pos[:], s2[:])
    nc.vector.tensor_add(el_num[:], el_num[:], s1[:])
    nc.vector.tensor_mul(el_num[:], el_num[:], ne_inv[:])  # el_num := edge_loss
    r_reg = cpl.tile([1, 1], F32, tag="r_reg")
    nc.vector.tensor_mul(r_reg[:], rr_sum[:], ne_inv[:])

    total = cpl.tile([1, 1], F32, tag="total")
    nc.vector.scalar_tensor_tensor(total[:], el_num[:], 0.1, cls_loss[:], ALU.mult, ALU.add)
    nc.vector.scalar_tensor_tensor(total[:], r_reg[:], 0.001, total[:], ALU.mult, ALU.add)
    nc.vector.scalar_tensor_tensor(total[:], wa_sum[:], 0.01 / (C * C), total[:], ALU.mult, ALU.add)
    nc.sync.dma_start(o_total.ap().rearrange("(p o) -> p o", o=1), total[:])

    dpool_cm.__exit__(None, None, None)
    cpl_cm.__exit__(None, None, None)
    cpm_cm.__exit__(None, None, None)
    wts_cm.__exit__(None, None, None)
    base_cm.__exit__(None, None, None)
    dram_cm.__exit__(None, None, None)
    pscam_cm.__exit__(None, None, None)
    psacc_cm.__exit__(None, None, None)
    ps_cm.__exit__(None, None, None)


_CACHE = {}


def _get_compiled():
    if "nc" in _CACHE:
        return _CACHE["nc"]
    nc = bacc.Bacc("TRN2", target_bir_lowering=False, debug=False, num_devices=N_CORES)
    with tile.TileContext(nc) as tc:
        _build_program(nc, tc)
    nc.compile()
    _CACHE["nc"] = nc
    return nc


def make_in_maps(inputs):
    feats = np.ascontiguousarray(np.asarray(inputs["feats"], np.float32)).reshape(B, F, PIX)
    tgt = np.asarray(inputs["img_labels"]).astype(np.float32)
    shared = {
        "cls_logits": np.asarray(inputs["cls_logits"], np.float32),
        "tgt": tgt,
        "prior_pmi": np.asarray(inputs["prior_pmi"], np.float32),
    }
    for k in ("em_w1", "em_w2", "em_w3", "pp_b1", "pp_b2", "msg_b1", "msg_b2",
              "ed_b", "em_b1", "em_b2", "em_b3", "alpha_b", "bias_b"):
        shared[k] = np.ascontiguousarray(np.asarray(inputs[k], np.float32))
    for k in ("pp_w1", "pp_w2", "msg_w1", "msg_w2", "ed_w", "alpha_w", "bias_w"):
        shared[k] = np.ascontiguousarray(np.asarray(inputs[k], np.float32).astype(np.float16))
    shared["em_w3"] = shared["em_w3"].reshape(64, 1)
    shared["bias_w"] = shared["bias_w"].reshape(HID, 1)
    in_maps = []
    for c in range(N_CORES):
        isel = np.zeros((C, IL), np.float32)
        for r in range(IL):
            isel[IL * c + r, r] = 1.0
        m = dict(shared)
        m["feats_l"] = np.ascontiguousarray(feats[BL * c:BL * (c + 1)])
        m["iselT"] = isel
        in_maps.append(m)
    return in_maps


def run(inputs, trace=False):
    nc = _get_compiled()
    res = run_bass_kernel_spmd(
        nc, make_in_maps(inputs), core_ids=list(range(N_CORES)), trace=trace
    )
    r0 = res.results[0]
    cam = np.concatenate([res.results[c]["o_cam"] for c in range(N_CORES)], axis=0)
    out = (
        r0["o_wadj"],
        r0["o_aw"],
        r0["o_dlog"],
        cam.reshape(B, C, H, W),
        r0["o_refined"],
        np.float32(r0["o_total"].reshape(())),
    )
    return out, res


def kernel(**inputs):
    out, _ = run(inputs, trace=False)
    return out


def bench(inputs, iters=12):
    """Time the NEFF with device-resident inputs (no donation, no re-transfer)."""
    import time

    import jax
    import numpy as np_
    from jax.experimental.shard_map import shard_map
    from jax.sharding import Mesh, NamedSharding, PartitionSpec

    from concourse import bass2jax as b2j
    from concourse import mybir as mb

    nc = _get_compiled()
    b2j.install_neuronx_cc_hook()
    partition_name = nc.partition_id_tensor.name if nc.partition_id_tensor else None
    in_names, out_names, out_avals, zero_outs = [], [], [], []
    for alloc in nc.m.functions[0].allocations:
        if not isinstance(alloc, mb.MemoryLocationSet):
            continue
        name = alloc.memorylocations[0].name
        if alloc.kind == "ExternalInput":
            if name != partition_name:
                in_names.append(name)
        elif alloc.kind == "ExternalOutput":
            out_names.append(name)
            shape = tuple(alloc.tensor_shape)
            dtype = mb.dt.np(alloc.dtype)
            out_avals.append(jax.core.ShapedArray(shape, dtype))
            zero_outs.append(np_.zeros(shape, dtype))
    n_params = len(in_names)
    all_in_names = list(in_names) + list(out_names)
    if partition_name is not None:
        all_in_names.append(partition_name)

    def _body(*args):
        operands = list(args)
        if partition_name is not None:
            operands.append(b2j.partition_id_tensor())
        outs = b2j._bass_exec_p.bind(
            *operands,
            out_avals=tuple(out_avals),
            in_names=tuple(all_in_names),
            out_names=tuple(out_names),
            lowering_input_output_aliases=(),
            sim_require_finite=True,
            sim_require_nnan=True,
            nc=nc,
        )
        return tuple(outs)

    devices = jax.devices()[:N_CORES]
    mesh = Mesh(np_.asarray(devices), ("core",))
    n_outs = len(out_names)
    in_specs = (PartitionSpec("core"),) * (n_params + n_outs)
    out_specs = (PartitionSpec("core"),) * n_outs
    sharded = jax.jit(
        shard_map(_body, mesh=mesh, in_specs=in_specs, out_specs=out_specs, check_rep=False),
        keep_unused=True,
    )
    in_maps = make_in_maps(inputs)
    sh = NamedSharding(mesh, PartitionSpec("core"))
    concat_in = [
        jax.device_put(
            np_.concatenate([np_.asarray(in_maps[c][n]) for c in range(N_CORES)], axis=0), sh
        )
        for n in in_names
    ]
    concat_zeros = [
        jax.device_put(np_.zeros((N_CORES * z.shape[0], *z.shape[1:]), z.dtype), sh)
        for z in zero_outs
    ]
    out = sharded(*concat_in, *concat_zeros)
    jax.block_until_ready(out)
    times = []
    for _ in range(iters):
        t0 = time.perf_counter()
        out = sharded(*concat_in, *concat_zeros)
        jax.block_until_ready(out)
        times.append(time.perf_counter() - t0)
    return times, out, out_names


# revision 54
# speedup vs baseline: 1.5069x; 1.0552x over previous
"""CoocGNN Trainium2 kernel: 8-core SPMD, batch-parallel feats + replicated graph.

Contract: kernel(**inputs) takes FULL inputs (as produced by setup_inputs) and
returns the FULL output tuple (W_adj, aw, dlog, cam_vis, refined, total).
"""

import os
import sys

for _p in ("/opt/trn_rl_repo", os.path.expanduser("~/.axon_site/_ro/trn_rl_repo")):
    if os.path.isdir(_p) and _p not in sys.path:
        sys.path.insert(0, _p)

import numpy as np

import concourse.bacc as bacc
import concourse.tile as tile
from concourse import masks, mybir
from concourse.bass_utils import run_bass_kernel_spmd

F32 = mybir.dt.float32
F32R = mybir.dt.float32r
F16 = mybir.dt.float16
AX = mybir.AxisListType
ALU = mybir.AluOpType
ACTF = mybir.ActivationFunctionType

N_CORES = 8
B, C, F, H, W = 32, 80, 2048, 32, 32
PIX = H * W          # 1024
BL = B // N_CORES    # 4 images per core
HID = 256
EDD = 64
IL = C // N_CORES    # 10 edge-rows per core
TEMP = 2.5
BETA_POS = 0.5
GAMMA_NEG = 0.25
KF = F // 128        # 16 f-chunks
EPS = 1e-6


def _build_program(nc, tc):
    ps_cm = tc.tile_pool(name="ps", bufs=3, space="PSUM")
    ps = ps_cm.__enter__()
    psacc_cm = tc.tile_pool(name="psacc", bufs=3, space="PSUM")
    psacc = psacc_cm.__enter__()
    pscam_cm = tc.tile_pool(name="pscam", bufs=2, space="PSUM")
    pscam = pscam_cm.__enter__()
    dram_cm = tc.tile_pool(name="dram", bufs=1, space="DRAM")
    dram = dram_cm.__enter__()
    base_cm = tc.tile_pool(name="base", bufs=1)
    base = base_cm.__enter__()
    wts_cm = tc.tile_pool(name="wts", bufs=1)
    wts = wts_cm.__enter__()

    t_feats = nc.dram_tensor("feats_l", [BL, F, PIX], F32, kind="ExternalInput")
    t_cls = nc.dram_tensor("cls_logits", [B, C], F32, kind="ExternalInput")
    t_tgt = nc.dram_tensor("tgt", [B, C], F32, kind="ExternalInput")
    t_prior = nc.dram_tensor("prior_pmi", [C, C], F32, kind="ExternalInput")
    t_isel = nc.dram_tensor("iselT", [C, IL], F32, kind="ExternalInput")
    t_pp_w1 = nc.dram_tensor("pp_w1", [F, HID], F16, kind="ExternalInput")
    t_pp_w2 = nc.dram_tensor("pp_w2", [HID, HID], F16, kind="ExternalInput")
    t_msg_w1 = nc.dram_tensor("msg_w1", [HID, HID], F16, kind="ExternalInput")
    t_msg_w2 = nc.dram_tensor("msg_w2", [HID, HID], F16, kind="ExternalInput")
    t_ed_w = nc.dram_tensor("ed_w", [HID, EDD], F16, kind="ExternalInput")
    t_em_w1 = nc.dram_tensor("em_w1", [4 * EDD + 3, 128], F32, kind="ExternalInput")
    t_em_w2 = nc.dram_tensor("em_w2", [128, 64], F32, kind="ExternalInput")
    t_em_w3 = nc.dram_tensor("em_w3", [64, 1], F32, kind="ExternalInput")
    t_alpha_w = nc.dram_tensor("alpha_w", [HID, F], F16, kind="ExternalInput")
    t_bias_w = nc.dram_tensor("bias_w", [HID, 1], F16, kind="ExternalInput")
    t_pp_b1 = nc.dram_tensor("pp_b1", [HID], F32, kind="ExternalInput")
    t_pp_b2 = nc.dram_tensor("pp_b2", [HID], F32, kind="ExternalInput")
    t_msg_b1 = nc.dram_tensor("msg_b1", [HID], F32, kind="ExternalInput")
    t_msg_b2 = nc.dram_tensor("msg_b2", [HID], F32, kind="ExternalInput")
    t_ed_b = nc.dram_tensor("ed_b", [EDD], F32, kind="ExternalInput")
    t_em_b1 = nc.dram_tensor("em_b1", [128], F32, kind="ExternalInput")
    t_em_b2 = nc.dram_tensor("em_b2", [64], F32, kind="ExternalInput")
    t_em_b3 = nc.dram_tensor("em_b3", [1], F32, kind="ExternalInput")
    t_alpha_b = nc.dram_tensor("alpha_b", [F], F32, kind="ExternalInput")
    t_bias_b = nc.dram_tensor("bias_b", [1], F32, kind="ExternalInput")

    o_wadj = nc.dram_tensor("o_wadj", [C, C], F32, kind="ExternalOutput")
    o_aw = nc.dram_tensor("o_aw", [C, F], F32, kind="ExternalOutput")
    o_dlog = nc.dram_tensor("o_dlog", [C], F32, kind="ExternalOutput")
    o_refined = nc.dram_tensor("o_refined", [B, C], F32, kind="ExternalOutput")
    o_total = nc.dram_tensor("o_total", [1], F32, kind="ExternalOutput")
    o_cam = nc.dram_tensor("o_cam", [BL, C, PIX], F32, kind="ExternalOutput")

    # ---------------- constants / small inputs ----------------
    id128 = base.tile([128, 128], F32, tag="id128")
    masks.make_identity(nc, id128[:])
    ones_col = base.tile([128, 1], F32, tag="ones_col")
    nc.vector.memset(ones_col[:], 1.0)
    ones_row = base.tile([1, 128], F32, tag="ones_row")
    nc.vector.memset(ones_row[:], 1.0)
    ones_row16 = base.tile([1, 128], F16, tag="ones_row16")
    nc.vector.memset(ones_row16[:], 1.0)
    offd = base.tile([C, C], F32, tag="offd")
    nc.vector.memset(offd[:], 1.0)
    nc.vector.tensor_sub(offd[:], offd[:], id128[:C, :C])

    def load(pool, shape, dram_ap, tag, dt=F32):
        t = pool.tile(shape, dt, tag=tag)
        nc.sync.dma_start(t[:], dram_ap)
        return t

    cls_sb = load(base, [B, C], t_cls[:, :], "cls_sb")
    tgt_sb = load(base, [B, C], t_tgt[:, :], "tgt_sb")
    prior_sb = load(base, [C, C], t_prior[:, :], "prior_sb")
    isel_sb = load(base, [C, IL], t_isel[:, :], "isel_sb")

    pp_w2_sb = load(wts, [128, 2, HID], t_pp_w2.ap().rearrange("(k p) h -> p k h", p=128), "pp_w2_sb", dt=F16)
    msg_w1_sb = load(wts, [128, 2, HID], t_msg_w1.ap().rearrange("(k p) h -> p k h", p=128), "msg_w1_sb", dt=F16)
    msg_w2_sb = load(wts, [128, 2, HID], t_msg_w2.ap().rearrange("(k p) h -> p k h", p=128), "msg_w2_sb", dt=F16)
    ed_w_sb = load(wts, [128, 2, EDD], t_ed_w.ap().rearrange("(k p) h -> p k h", p=128), "ed_w_sb", dt=F16)
    bias_w_sb = load(wts, [128, 2, 1], t_bias_w.ap().rearrange("(k p) h -> p k h", p=128), "bias_w_sb", dt=F16)

    ew1_a = load(wts, [EDD, 128], t_em_w1[0:EDD, :], "ew1_a")
    ew1_b = load(wts, [EDD, 128], t_em_w1[EDD:2 * EDD, :], "ew1_b")
    ew1_fi = load(wts, [1, 128], t_em_w1[4 * EDD + 1:4 * EDD + 2, :], "ew1_fi")
    ew1_fj = load(wts, [1, 128], t_em_w1[4 * EDD + 2:4 * EDD + 3, :], "ew1_fj")
    # fp16 copies for the edge MLP (f32 staging in a short-lived pool)
    ew1_abs16 = wts.tile([EDD, 128], F16, tag="ew1_abs16")
    ew1_prod16 = wts.tile([EDD, 128], F16, tag="ew1_prod16")
    ew1_cos16 = wts.tile([1, 128], F16, tag="ew1_cos16")
    em_w2_16 = wts.tile([128, 64], F16, tag="em_w2_16")
    em_w3_16 = wts.tile([64, 1], F16, tag="em_w3_16")
    b_alpha = wts.tile([1, F], F16, tag="b_alpha")

    b_pp1 = load(wts, [1, HID], t_pp_b1.ap().rearrange("(o h) -> o h", o=1), "b_pp1")
    b_pp2 = load(wts, [1, HID], t_pp_b2.ap().rearrange("(o h) -> o h", o=1), "b_pp2")
    b_m1 = load(wts, [1, HID], t_msg_b1.ap().rearrange("(o h) -> o h", o=1), "b_m1")
    b_m2 = load(wts, [1, HID], t_msg_b2.ap().rearrange("(o h) -> o h", o=1), "b_m2")
    b_ed = load(wts, [1, EDD], t_ed_b.ap().rearrange("(o h) -> o h", o=1), "b_ed")
    b_em1 = load(wts, [128, 1], t_em_b1.ap().rearrange("(p o) -> p o", o=1), "b_em1")
    b_em2 = load(wts, [64, 1], t_em_b2.ap().rearrange("(p o) -> p o", o=1), "b_em2")
    b_em3 = load(wts, [1, 1], t_em_b3.ap().rearrange("(p o) -> p o", o=1), "b_em3")
    b_biasb = load(wts, [1, 1], t_bias_b.ap().rearrange("(p o) -> p o", o=1), "b_biasb")

    # cpm: mid-lived tensors that survive into late phase C
    cpm_cm = tc.tile_pool(name="cpm", bufs=1)
    cpm = cpm_cm.__enter__()

    # pp_w1 lives in its own pool; released after Hn1 so alpha_w can reuse it.
    w1pool_cm = tc.tile_pool(name="w1pool", bufs=1)
    w1pool = w1pool_cm.__enter__()
    pp_w1_sb = w1pool.tile([128, KF, HID], F16, tag="pp_w1_sb")
    nc.sync.dma_start(pp_w1_sb[:], t_pp_w1.ap().rearrange("(p q) h -> p q h", q=KF))

    # f32 staging for the fp16 weight copies; freed before phase A pressure
    with tc.tile_pool(name="stg", bufs=1) as stg:
        ew1_abs = load(stg, [EDD, 128], t_em_w1[2 * EDD:3 * EDD, :], "ew1_abs")
        nc.scalar.copy(ew1_abs16[:], ew1_abs[:])
        ew1_prod = load(stg, [EDD, 128], t_em_w1[3 * EDD:4 * EDD, :], "ew1_prod")
        nc.scalar.copy(ew1_prod16[:], ew1_prod[:])
        ew1_cos = load(stg, [1, 128], t_em_w1[4 * EDD:4 * EDD + 1, :], "ew1_cos")
        nc.scalar.copy(ew1_cos16[:], ew1_cos[:])
        em_w2_sb = load(stg, [128, 64], t_em_w2[:, :], "em_w2_sb")
        nc.scalar.copy(em_w2_16[:], em_w2_sb[:])
        em_w3_sb = load(stg, [64, 1], t_em_w3[:, :], "em_w3_sb")
        nc.scalar.copy(em_w3_16[:], em_w3_sb[:])
        b_alpha32 = load(stg, [1, F], t_alpha_b.ap().rearrange("(o h) -> o h", o=1), "b_alpha32")
        nc.scalar.copy(b_alpha[:], b_alpha32[:])

    # ---------------- phase A: stream feats, stash fp16, pool ----------------
    # f-to-partition mapping is f = 16*p + q: each partition owns 16
    # consecutive feature rows, so every DMA descriptor is 8 KiB contiguous.
    # Batch is strided-sharded (core r owns global images r, 8+r, 16+r, 24+r),
    # so the per-image AllGather of pooled rows lands contiguous [8b:8b+8).
    stash = base.tile([128, BL, KF, PIX], F16, tag="stash")
    pooled_sb = base.tile([128, BL, KF], F32, tag="pooled_sb")
    pooled_loc = [dram.tile([1, F], F32, name=f"pooled_loc{b}") for b in range(BL)]
    pooled_rows = [dram.tile([N_CORES, F], F32, name=f"pooled_rows{b}") for b in range(BL)]

    def stash_slice(b, q, n0, n1):
        return stash[:, b, q, n0:n1]

    # pooled_full/pooled16 live through phase A (filled per-image as AGs land)
    pfull_cm = tc.tile_pool(name="pfull", bufs=1)
    pfull = pfull_cm.__enter__()
    pooled_full = pfull.tile([B, F], F32, tag="pooled_full")
    pooled16 = pfull.tile([B, F], F16, tag="pooled16")

    with tc.tile_pool(name="ina", bufs=4) as ina:
        for b in range(BL):
            for j in range(KF // 2):
                tin = ina.tile([128, 2, PIX], F32, tag="tin")
                nc.sync.dma_start(
                    tin[:],
                    t_feats[b, :, :].rearrange("(p q) n -> p q n", q=KF)[:, 2 * j:2 * (j + 1), :],
                )
                nc.scalar.copy(stash[:, b, 2 * j:2 * (j + 1), :], tin[:])
                nc.vector.tensor_reduce(
                    pooled_sb[:, b, 2 * j:2 * (j + 1)], tin[:], axis=AX.X, op=ALU.add
                )
                # keep the PE HAM-warm through phase A
                wm = ps.tile([1, 64], F32, tag="ps")
                nc.tensor.matmul(wm[:], tin[:, 0, 0:1], tin[:, 0, 0:64])
            nc.sync.dma_start(
                pooled_loc[b][:].rearrange("o (p q) -> (o p) q", q=KF), pooled_sb[:, b, :]
            )
            nc.gpsimd.collective_compute(
                "AllGather", ALU.bypass,
                replica_groups=[list(range(N_CORES))],
                ins=[pooled_loc[b].opt()], outs=[pooled_rows[b].opt()],
            )
            nc.sync.dma_start(
                pooled_full[N_CORES * b:N_CORES * (b + 1), :], pooled_rows[b][:]
            )

    nc.scalar.copy(pooled16[:], pooled_full[:])

    # ---------------- phase C (early): proto / cos / Hn1 ----------------
    cpe_cm = tc.tile_pool(name="cpe", bufs=1)
    cpe = cpe_cm.__enter__()

    def sigmoid(out_ap, in_ap, scale=1.0):
        # 1 / (1 + exp(-x*scale)) via Exp + DVE reciprocal (single ACT table)
        nc.scalar.activation(out_ap, in_ap, ACTF.Exp, scale=-scale)
        nc.vector.tensor_scalar_add(out_ap, out_ap, 1.0)
        nc.vector.reciprocal(out_ap, out_ap)

    def softplus_neg(out_ap, in_ap):
        # log1p(exp(-x)) for x >= 0
        nc.scalar.activation(out_ap, in_ap, ACTF.Exp, scale=-1.0)
        nc.scalar.activation(out_ap, out_ap, ACTF.Ln, bias=1.0)

    probs = base.tile([B, C], F32, tag="probs")
    sigmoid(probs[:], cls_sb[:])
    y_sb = base.tile([B, C], F32, tag="y_sb")
    nc.vector.tensor_scalar_max(y_sb[:], tgt_sb[:], 0.0)
    probs16 = cpm.tile([B, C], F16, tag="probs16")
    nc.scalar.copy(probs16[:], probs[:])

    # weight_sum / freq / present
    ws_ps = ps.tile([C, 1], F32, tag="ps")
    nc.tensor.matmul(ws_ps[:], probs[:], ones_col[:B, :])
    wsum = cpm.tile([C, 1], F32, tag="wsum")
    nc.scalar.copy(wsum[:], ws_ps[:])
    freq_col = cpm.tile([C, 1], F32, tag="freq_col")
    nc.scalar.mul(freq_col[:], wsum[:], 1.0 / B)
    wclamp = cpm.tile([C, 1], F32, tag="wclamp")
    nc.vector.tensor_scalar_max(wclamp[:], wsum[:], EPS)
    winv = cpm.tile([C, 1], F32, tag="winv")
    nc.vector.reciprocal(winv[:], wclamp[:])
    winv_eff = cpm.tile([C, 1], F32, tag="winv_eff")
    nc.scalar.mul(winv_eff[:], winv[:], 1.0 / PIX)
    # row of wsum*PIX (to inject exact pp_b1 under the later winv_eff scaling)
    wsp_col = cpm.tile([C, 1], F32, tag="wsp_col")
    nc.scalar.mul(wsp_col[:], wclamp[:], float(PIX))
    wsp_ps = ps.tile([1, C], F32, tag="ps")
    nc.tensor.transpose(wsp_ps[:], wsp_col[:], id128[:C, :C])
    wsp_row = cpm.tile([1, C], F32, tag="wsp_row")
    nc.vector.tensor_copy(wsp_row[:], wsp_ps[:])

    ys_ps = ps.tile([C, 1], F32, tag="ps")
    nc.tensor.matmul(ys_ps[:], y_sb[:], ones_col[:B, :])
    present = cpm.tile([C, 1], F32, tag="present")
    nc.vector.tensor_single_scalar(present[:], ys_ps[:], 0.5, ALU.is_gt)

    # proto row norms via Square+accum (proto left unnormalized, scales folded)
    nrm_part = cpe.tile([C, 4], F32, tag="nrm_part")
    sq_scr = cpe.tile([C, 512], F32, tag="sq_scr")
    for j in range(4):
        pp_ = psacc.tile([C, 512], F32, tag="acc")
        nc.tensor.matmul(
            pp_[:], probs16[:],
            pooled16[:, 512 * j:512 * (j + 1)],
        )
        nc.scalar.activation(
            sq_scr[:], pp_[:], ACTF.Square, accum_out=nrm_part[:, j:j + 1]
        )
    nrm_sq = cpm.tile([C, 1], F32, tag="nrm_sq")
    nc.vector.tensor_reduce(nrm_sq[:], nrm_part[:], axis=AX.X, op=ALU.add)
    nrm = cpm.tile([C, 1], F32, tag="nrm")
    nc.scalar.activation(nrm[:], nrm_sq[:], ACTF.Ln)
    nc.scalar.activation(nrm[:], nrm[:], ACTF.Exp, scale=0.5)  # sqrt
    nc.vector.tensor_scalar_max(nrm[:], nrm[:], EPS)
    inv_u = cpm.tile([C, 1], F32, tag="inv_u")
    nc.vector.reciprocal(inv_u[:], nrm[:])

    # protoT (f-major), G, cos
    protoT = cpe.tile([128, KF, C], F16, tag="protoT")
    pooled16_q = pooled16[:].rearrange("b (p q) -> b q p", q=KF)
    for k in range(KF):
        pt_ = ps.tile([128, C], F32, tag="ps")
        nc.tensor.matmul(pt_[:], pooled16_q[:, k, :], probs16[:])
        if k % 2 == 0:
            nc.vector.tensor_copy(protoT[:, k, :], pt_[:])
        else:
            nc.scalar.copy(protoT[:, k, :], pt_[:])
    g_ps = psacc.tile([C, C], F32, tag="acc")
    for k in range(KF):
        nc.tensor.matmul(
            g_ps[:], protoT[:, k, :], protoT[:, k, :],
            start=(k == 0), stop=(k == KF - 1),
        )
    ir_ps = ps.tile([1, C], F32, tag="ps")
    nc.tensor.transpose(ir_ps[:], inv_u[:], id128[:C, :C])
    inv_row = cpm.tile([1, C], F32, tag="inv_row")
    nc.vector.tensor_copy(inv_row[:], ir_ps[:])
    s_ps = ps.tile([C, C], F32, tag="ps")
    nc.tensor.matmul(s_ps[:], inv_row[:], inv_row[:])
    cos_sb = cpm.tile([C, C], F32, tag="cos_sb")
    nc.scalar.copy(cos_sb[:], g_ps[:])
    nc.vector.tensor_tensor(cos_sb[:], cos_sb[:], s_ps[:], ALU.mult)
    nc.vector.tensor_scalar(cos_sb[:], cos_sb[:], 1.0, -1.0, ALU.min, ALU.max)

    # Hn1 = relu((proto_u @ pp_w1 + pp_b1*wsum*PIX) * winv_eff)
    h1_ps = psacc.tile([C, HID], F32, tag="acc")
    for k in range(KF):
        nc.tensor.matmul(
            h1_ps[:], protoT[:, k, :], pp_w1_sb[:, k, :],
            start=(k == 0), stop=False,
        )
    nc.tensor.matmul(h1_ps[:], wsp_row[:], b_pp1[:], start=False, stop=True)
    relu1 = cpe.tile([C, HID], F32, tag="relu1")
    nc.vector.tensor_scalar_mul(relu1[:], h1_ps[:], winv_eff[:])
    nc.scalar.activation(relu1[:], relu1[:], ACTF.Relu)

    r1T = cpm.tile([128, 2, C], F16, tag="r1T")
    for j in range(2):
        tp_ = ps.tile([128, C], F32, tag="ps")
        nc.tensor.transpose(tp_[:], relu1[:, 128 * j:128 * (j + 1)], id128[:C, :C])
        nc.scalar.copy(r1T[:, j, :], tp_[:])

    # release pp_w1 + early tensors (LIFO); open late pools
    cpe_cm.__exit__(None, None, None)
    pfull_cm.__exit__(None, None, None)
    w1pool_cm.__exit__(None, None, None)

    cpl_cm = tc.tile_pool(name="cpl", bufs=1)
    cpl = cpl_cm.__enter__()
    awpool_cm = tc.tile_pool(name="awpool", bufs=1)
    awpool = awpool_cm.__enter__()
    alpha_w_sb = awpool.tile([128, 2, F], F16, tag="alpha_w_sb")
    nc.sync.dma_start(alpha_w_sb[:], t_alpha_w.ap().rearrange("(k p) h -> p k h", p=128))

    def transpose_pair(src, tag="tpair"):
        # src [C, 256] -> dst [128, 2, C]
        dst = cpl.tile([128, 2, C], F16, tag=tag)
        for j in range(2):
            tp_ = ps.tile([128, C], F32, tag="ps")
            nc.tensor.transpose(tp_[:], src[:, 128 * j:128 * (j + 1)], id128[:C, :C])
            nc.scalar.copy(dst[:, j, :], tp_[:])
        return dst

    hn_ps = psacc.tile([C, HID], F32, tag="acc")
    for j in range(2):
        nc.tensor.matmul(
            hn_ps[:], r1T[:, j, :], pp_w2_sb[:, j, :],
            start=(j == 0), stop=False,
        )
    nc.tensor.matmul(hn_ps[:], ones_row[:1, :C], b_pp2[:], start=False, stop=True)
    hn_sb = cpl.tile([C, HID], F32, tag="hn_sb")
    nc.scalar.copy(hn_sb[:], hn_ps[:])

    hnT = transpose_pair(hn_sb)
    zd_ps = psacc.tile([C, EDD], F32, tag="acc")
    for j in range(2):
        nc.tensor.matmul(zd_ps[:], hnT[:, j, :], ed_w_sb[:, j, :], start=(j == 0), stop=False)
    nc.tensor.matmul(zd_ps[:], ones_row[:1, :C], b_ed[:], start=False, stop=True)
    zd_sb = cpl.tile([C, EDD], F32, tag="zd_sb")
    nc.scalar.activation(zd_sb[:], zd_ps[:], ACTF.Relu)

    zdT_ps = ps.tile([EDD, C], F32, tag="ps")
    nc.tensor.transpose(zdT_ps[:], zd_sb[:], id128[:C, :C])
    zdT = cpl.tile([EDD, C], F32, tag="zdT")
    nc.vector.tensor_copy(zdT[:], zdT_ps[:])

    # local (sharded) pieces via iselT
    zdl_ps = ps.tile([IL, EDD], F32, tag="ps")
    nc.tensor.matmul(zdl_ps[:], isel_sb[:], zd_sb[:])
    zdl = cpl.tile([IL, EDD], F32, tag="zdl")
    nc.vector.tensor_copy(zdl[:], zdl_ps[:])
    zdTl_ps = ps.tile([EDD, IL], F32, tag="ps")
    nc.tensor.transpose(zdTl_ps[:], zdl[:], id128[:IL, :IL])
    zdTl = cpl.tile([EDD, IL], F32, tag="zdTl")
    nc.vector.tensor_copy(zdTl[:], zdTl_ps[:])

    fl_ps = ps.tile([IL, 1], F32, tag="ps")
    nc.tensor.matmul(fl_ps[:], isel_sb[:], freq_col[:])
    fl_sb = cpl.tile([IL, 1], F32, tag="fl_sb")
    nc.vector.tensor_copy(fl_sb[:], fl_ps[:])
    flr_ps = ps.tile([1, IL], F32, tag="ps")
    nc.tensor.transpose(flr_ps[:], fl_sb[:], id128[:IL, :IL])
    flr = cpl.tile([1, IL], F32, tag="flr")
    nc.vector.tensor_copy(flr[:], flr_ps[:])
    fr_ps = ps.tile([1, C], F32, tag="ps")
    nc.tensor.transpose(fr_ps[:], freq_col[:], id128[:C, :C])
    fr_sb = cpl.tile([1, C], F32, tag="fr_sb")
    nc.vector.tensor_copy(fr_sb[:], fr_ps[:])

    # PT_loc (+ freq_i term), QT (+ freq_j term)
    ptl_ps = ps.tile([128, IL], F32, tag="ps")
    nc.tensor.matmul(ptl_ps[:], ew1_a[:], zdTl[:], start=True, stop=False)
    nc.tensor.matmul(ptl_ps[:], ew1_fi[:], flr[:], start=False, stop=True)
    ptl = cpl.tile([128, IL], F32, tag="ptl")
    nc.vector.tensor_copy(ptl[:], ptl_ps[:])
    qt_ps = ps.tile([128, C], F32, tag="ps")
    nc.tensor.matmul(qt_ps[:], ew1_b[:], zdT[:], start=True, stop=False)
    nc.tensor.matmul(qt_ps[:], ew1_fj[:], fr_sb[:], start=False, stop=True)
    qt = cpl.tile([128, C], F32, tag="qt")
    nc.scalar.copy(qt[:], qt_ps[:])
    ptq = cpl.tile([128, IL, C], F16, tag="ptq")
    nc.vector.tensor_tensor(
        ptq[:],
        ptl[:].unsqueeze(2).broadcast_to([128, IL, C]),
        qt[:].unsqueeze(1).broadcast_to([128, IL, C]),
        ALU.add,
    )

    cosl_ps = ps.tile([IL, C], F32, tag="ps")
    nc.tensor.matmul(cosl_ps[:], isel_sb[:], cos_sb[:])
    cosl16 = cpl.tile([IL, C], F16, tag="cosl16")
    nc.scalar.copy(cosl16[:], cosl_ps[:])
    cos_flat = cpl.tile([1, IL * C], F16, tag="cos_flat")
    nc.sync.dma_start(cos_flat[:], cosl16[:])

    # edge feature blocks: |zi-zj| (in place) and zi*zj, fp16
    absblk = cpl.tile([EDD, IL, C], F16, tag="absblk")
    nc.vector.tensor_tensor(
        absblk[:],
        zdTl[:].unsqueeze(2).broadcast_to([EDD, IL, C]),
        zdT[:].unsqueeze(1).broadcast_to([EDD, IL, C]),
        ALU.subtract,
    )
    nc.scalar.activation(absblk[:], absblk[:], ACTF.Abs)
    prodblk = cpl.tile([EDD, IL, C], F16, tag="prodblk")
    nc.vector.tensor_tensor(
        prodblk[:],
        zdTl[:].unsqueeze(2).broadcast_to([EDD, IL, C]),
        zdT[:].unsqueeze(1).broadcast_to([EDD, IL, C]),
        ALU.mult,
    )

    nedge = IL * C
    abs_flat = absblk[:].rearrange("p r j -> p (r j)")
    prod_flat = prodblk[:].rearrange("p r j -> p (r j)")
    ptqf = ptq[:].rearrange("p r j -> p (r j)")

    e1T = cpl.tile([128, nedge], F16, tag="e1T")
    for n0, n1 in ((0, 512), (512, nedge)):
        e1_ps = psacc.tile([128, 512], F32, tag="acc")
        nc.tensor.matmul(e1_ps[:, :n1 - n0], ew1_abs16[:], abs_flat[:, n0:n1], start=True, stop=False)
        nc.tensor.matmul(e1_ps[:, :n1 - n0], ew1_prod16[:], prod_flat[:, n0:n1], start=False, stop=False)
        nc.tensor.matmul(e1_ps[:, :n1 - n0], ew1_cos16[:], cos_flat[:, n0:n1], start=False, stop=True)
        nc.vector.tensor_tensor(e1_ps[:, :n1 - n0], e1_ps[:, :n1 - n0], ptqf[:, n0:n1], ALU.add)
        nc.scalar.activation(e1T[:, n0:n1], e1_ps[:, :n1 - n0], ACTF.Relu, bias=b_em1[:])

    e2T = cpl.tile([EDD, nedge], F16, tag="e2T")
    for n0, n1 in ((0, 512), (512, nedge)):
        e2_ps = psacc.tile([EDD, 512], F32, tag="acc")
        nc.tensor.matmul(e2_ps[:, :n1 - n0], em_w2_16[:], e1T[:, n0:n1])
        nc.scalar.activation(e2T[:, n0:n1], e2_ps[:, :n1 - n0], ACTF.Relu, bias=b_em2[:])

    r_sb = cpl.tile([1, nedge], F32, tag="r_sb")
    for n0, n1 in ((0, 512), (512, nedge)):
        r_ps = ps.tile([1, 512], F32, tag="ps")
        nc.tensor.matmul(r_ps[:, :n1 - n0], em_w3_16[:], e2T[:, n0:n1])
        nc.scalar.activation(r_sb[:, n0:n1], r_ps[:, :n1 - n0], ACTF.Identity, bias=b_em3[:])

    r_loc_d = dram.tile([IL, C], F32)
    r_full_d = dram.tile([C, C], F32)
    nc.sync.dma_start(r_loc_d[:], r_sb[:])
    nc.gpsimd.collective_compute(
        "AllGather", ALU.bypass,
        replica_groups=[list(range(N_CORES))],
        ins=[r_loc_d.opt()], outs=[r_full_d.opt()],
    )
    r_full = cpl.tile([C, C], F32, tag="r_full")
    nc.sync.dma_start(r_full[:], r_full_d[:])

    # W_adj
    h_sb = cpl.tile([C, C], F32, tag="h_sb")
    nc.vector.scalar_tensor_tensor(h_sb[:], prior_sb[:], 1.0 / TEMP, r_full[:], ALU.mult, ALU.add)
    w_sg = cpl.tile([C, C], F32, tag="w_sg")
    sigmoid(w_sg[:], h_sb[:])
    wt_ps = ps.tile([C, C], F32, tag="ps")
    nc.tensor.transpose(wt_ps[:], w_sg[:], id128[:C, :C])
    w_half = cpl.tile([C, C], F32, tag="w_half")
    nc.scalar.mul(w_half[:], wt_ps[:], 0.5)
    nc.vector.scalar_tensor_tensor(w_half[:], w_sg[:], 0.5, w_half[:], ALU.mult, ALU.add)
    wadj = cpl.tile([C, C], F32, tag="wadj")
    nc.vector.tensor_mul(wadj[:], w_half[:], offd[:])
    nc.sync.dma_start(o_wadj[:, :], wadj[:])

    rowsum = cpl.tile([C, 1], F32, tag="rowsum")
    nc.vector.tensor_reduce(rowsum[:], wadj[:], axis=AX.X, op=ALU.add)
    nc.vector.tensor_scalar_max(rowsum[:], rowsum[:], EPS)
    rinv = cpl.tile([C, 1], F32, tag="rinv")
    nc.vector.reciprocal(rinv[:], rowsum[:])
    rr_ps = ps.tile([1, C], F32, tag="ps")
    nc.tensor.transpose(rr_ps[:], rinv[:], id128[:C, :C])
    rinv_row = cpl.tile([1, C], F32, tag="rinv_row")
    nc.vector.tensor_copy(rinv_row[:], rr_ps[:])
    rb_ps = ps.tile([C, C], F32, tag="ps")
    nc.tensor.matmul(rb_ps[:], ones_row[:1, :C], rinv_row[:])
    at_sb = cpl.tile([C, C], F32, tag="at_sb")
    nc.vector.tensor_tensor(at_sb[:], wadj[:], rb_ps[:], ALU.mult)

    # message passing (1 step)
    m1T = cpl.tile([128, 2, C], F16, tag="tpair")
    for j in range(2):
        mp_ = ps.tile([128, C], F32, tag="ps")
        nc.tensor.matmul(mp_[:], hn_sb[:, 128 * j:128 * (j + 1)], at_sb[:])
        nc.scalar.copy(m1T[:, j, :], mp_[:])
    m1_ps = psacc.tile([C, HID], F32, tag="acc")
    for j in range(2):
        nc.tensor.matmul(
            m1_ps[:], m1T[:, j, :], msg_w1_sb[:, j, :],
            start=(j == 0), stop=False,
        )
    nc.tensor.matmul(m1_ps[:], ones_row[:1, :C], b_m1[:], start=False, stop=True)
    mr_sb = cpl.tile([C, HID], F32, tag="mr_sb")
    nc.scalar.activation(mr_sb[:], m1_ps[:], ACTF.Relu)
    mrT = transpose_pair(mr_sb)
    m2_ps = psacc.tile([C, HID], F32, tag="acc")
    for j in range(2):
        nc.tensor.matmul(
            m2_ps[:], mrT[:, j, :], msg_w2_sb[:, j, :],
            start=(j == 0), stop=False,
        )
    nc.tensor.matmul(m2_ps[:], ones_row[:1, :C], b_m2[:], start=False, stop=True)
    zn_sb = cpl.tile([C, HID], F32, tag="zn_sb")
    nc.vector.tensor_tensor(zn_sb[:], m2_ps[:], hn_sb[:], ALU.add)
    nc.scalar.activation(zn_sb[:], zn_sb[:], ACTF.Relu)
    znT = transpose_pair(zn_sb, tag="znT")

    # aw head
    aw_sb = cpl.tile([C, F], F32, tag="aw_sb")
    aws_part = cpl.tile([C, 4], F32, tag="aws_part")
    for n in range(4):
        a_ps = psacc.tile([C, 512], F32, tag="acc")
        for j in range(2):
            nc.tensor.matmul(
                a_ps[:], znT[:, j, :],
                alpha_w_sb[:, j, 512 * n:512 * (n + 1)],
                start=(j == 0), stop=False,
            )
        nc.tensor.matmul(a_ps[:], ones_row16[:1, :C], b_alpha[:, 512 * n:512 * (n + 1)], start=False, stop=True)
        # softplus(x) = ln(exp(x) + 1)
        nc.scalar.activation(aw_sb[:, 512 * n:512 * (n + 1)], a_ps[:], ACTF.Exp)
        nc.scalar.activation(
            aw_sb[:, 512 * n:512 * (n + 1)], aw_sb[:, 512 * n:512 * (n + 1)],
            ACTF.Ln, bias=1.0,
            accum_out=aws_part[:, n:n + 1],
        )
    aws = cpl.tile([C, 1], F32, tag="aws")
    nc.vector.tensor_reduce(aws[:], aws_part[:], axis=AX.X, op=ALU.add)
    nc.vector.tensor_scalar_max(aws[:], aws[:], EPS)
    sinv = cpl.tile([C, 1], F32, tag="sinv")
    nc.vector.reciprocal(sinv[:], aws[:])
    nc.vector.tensor_scalar_mul(aw_sb[:], aw_sb[:], sinv[:])
    nc.sync.dma_start(o_aw[:, :], aw_sb[:])

    awT16 = cpl.tile([128, KF, C], F16, tag="awT16")
    aw_q = aw_sb[:].rearrange("c (p q) -> c q p", q=KF)
    for k in range(KF):
        at_ps = ps.tile([128, C], F32, tag="ps")
        nc.tensor.transpose(at_ps[:], aw_q[:, k, :], id128[:C, :C])
        nc.scalar.copy(awT16[:, k, :], at_ps[:])

    awpool_cm.__exit__(None, None, None)

    # dlog
    dl_ps = ps.tile([C, 1], F32, tag="ps")
    for j in range(2):
        nc.tensor.matmul(dl_ps[:], znT[:, j, :], bias_w_sb[:, j, :], start=(j == 0), stop=False)
    nc.tensor.matmul(dl_ps[:], ones_row[:1, :C], b_biasb[:], start=False, stop=True)
    dlog_sb = cpl.tile([C, 1], F32, tag="dlog_sb")
    nc.scalar.copy(dlog_sb[:], dl_ps[:])
    nc.sync.dma_start(o_dlog.ap().rearrange("(p o) -> p o", o=1), dlog_sb[:])
    dlr_ps = ps.tile([1, C], F32, tag="ps")
    nc.tensor.transpose(dlr_ps[:], dlog_sb[:], id128[:C, :C])
    dlr = cpl.tile([1, C], F32, tag="dlr")
    nc.vector.tensor_copy(dlr[:], dlr_ps[:])

    # refined
    py = cpl.tile([B, C], F32, tag="py")
    nc.vector.tensor_mul(py[:], probs[:], y_sb[:])
    pyT_ps = ps.tile([C, B], F32, tag="ps")
    nc.tensor.transpose(pyT_ps[:], py[:], id128[:B, :B])
    pyT = cpl.tile([C, B], F32, tag="pyT")
    nc.vector.tensor_copy(pyT[:], pyT_ps[:])
    pa_ps = ps.tile([B, C], F32, tag="ps")
    nc.tensor.matmul(pa_ps[:], pyT[:], wadj[:])
    nc.vector.tensor_mul(py[:], y_sb[:], pa_ps[:])  # py := pos_agg
    dlb_ps = ps.tile([B, C], F32, tag="ps")
    nc.tensor.matmul(dlb_ps[:], ones_row[:1, :B], dlr[:])
    prow = cpl.tile([B, 1], F32, tag="prow")
    nc.vector.tensor_reduce(prow[:], probs[:], axis=AX.X, op=ALU.add)
    nc.scalar.mul(prow[:], prow[:], -GAMMA_NEG)
    refined = cpl.tile([B, C], F32, tag="refined")
    nc.vector.scalar_tensor_tensor(
        refined[:], py[:], BETA_POS + GAMMA_NEG, dlb_ps[:], ALU.mult, ALU.add
    )
    nc.vector.tensor_scalar_add(refined[:], refined[:], prow[:])
    nc.vector.tensor_add(refined[:], refined[:], cls_sb[:])
    nc.sync.dma_start(o_refined[:, :], refined[:])

    # ---------------- phase D: CAM ----------------
    dpool_cm = tc.tile_pool(name="dpool", bufs=2)
    dpool = dpool_cm.__enter__()
    for b in range(BL):
        cam = dpool.tile([C, PIX], F32, tag="cam")
        for hh in range(2):
            c_ps = pscam.tile([C, 512], F32, tag="cam_ps")
            for k in range(KF):
                nc.tensor.matmul(
                    c_ps[:], awT16[:, k, :], stash_slice(b, k, 512 * hh, 512 * (hh + 1)),
                    start=(k == 0), stop=(k == KF - 1),
                )
            nc.scalar.activation(cam[:, 512 * hh:512 * (hh + 1)], c_ps[:], ACTF.Relu)
        mn = dpool.tile([C, 1], F32, tag="mn")
        nc.vector.tensor_reduce(mn[:], cam[:], axis=AX.X, op=ALU.min)
        mx = dpool.tile([C, 1], F32, tag="mx")
        nc.vector.tensor_reduce(mx[:], cam[:], axis=AX.X, op=ALU.max)
        nc.vector.tensor_sub(mx[:], mx[:], mn[:])
        nc.vector.tensor_scalar_add(mx[:], mx[:], EPS)
        dinv = dpool.tile([C, 1], F32, tag="dinv")
        nc.vector.reciprocal(dinv[:], mx[:])
        nc.vector.tensor_scalar(cam[:], cam[:], mn[:], dinv[:], ALU.subtract, ALU.mult)
        nc.sync.dma_start(o_cam[b, :, :], cam[:])

    # ---------------- losses ----------------
    def colsum_1x1(src_col, nrows, tag):
        p_ = ps.tile([1, 1], F32, tag="ps")
        nc.tensor.matmul(p_[:], src_col[:], ones_col[:nrows, :])
        out = cpl.tile([1, 1], F32, tag=tag)
        nc.vector.tensor_copy(out[:], p_[:])
        return out

    # cls loss: bce = relu(h) - h*t + softplus(-|h|), masked mean
    m_sb = cpl.tile([B, C], F32, tag="m_sb")
    nc.vector.tensor_single_scalar(m_sb[:], tgt_sb[:], -1.0, ALU.not_equal)
    safe_t = cpl.tile([B, C], F32, tag="safe_t")
    nc.vector.tensor_mul(safe_t[:], tgt_sb[:], m_sb[:])
    nc.vector.tensor_mul(safe_t[:], refined[:], safe_t[:])  # safe_t := h*t
    rh = cpl.tile([B, C], F32, tag="rh")
    nc.scalar.activation(rh[:], refined[:], ACTF.Relu)
    ab = cpl.tile([B, C], F32, tag="ab")
    nc.scalar.activation(ab[:], refined[:], ACTF.Abs)
    softplus_neg(ab[:], ab[:])  # ab := softplus(-|h|)
    nc.vector.tensor_sub(rh[:], rh[:], safe_t[:])
    nc.vector.tensor_add(rh[:], rh[:], ab[:])
    nc.vector.tensor_mul(rh[:], rh[:], m_sb[:])  # rh := bce * m
    bm_rows = cpl.tile([B, 1], F32, tag="bm_rows")
    nc.vector.tensor_reduce(bm_rows[:], rh[:], axis=AX.X, op=ALU.add)
    bce_sum = colsum_1x1(bm_rows, B, "bce_sum")
    nc.vector.tensor_reduce(bm_rows[:], m_sb[:], axis=AX.X, op=ALU.add)
    m_sum = colsum_1x1(bm_rows, B, "m_sum")
    nc.vector.tensor_scalar_max(m_sum[:], m_sum[:], 1.0)
    m_inv = cpl.tile([1, 1], F32, tag="m_inv")
    nc.vector.reciprocal(m_inv[:], m_sum[:])
    cls_loss = cpl.tile([1, 1], F32, tag="cls_loss")
    nc.vector.tensor_mul(cls_loss[:], bce_sum[:], m_inv[:])

    # edge mask
    pr_ps = ps.tile([1, C], F32, tag="ps")
    nc.tensor.transpose(pr_ps[:], present[:], id128[:C, :C])
    pres_row = cpl.tile([1, C], F32, tag="pres_row")
    nc.vector.tensor_copy(pres_row[:], pr_ps[:])
    ppo_ps = ps.tile([C, C], F32, tag="ps")
    nc.tensor.matmul(ppo_ps[:], pres_row[:], pres_row[:])
    emf = cpl.tile([C, C], F32, tag="emf")
    nc.vector.tensor_tensor(emf[:], offd[:], ppo_ps[:], ALU.mult)

    # edge bce over h vs soft targets t = sigmoid(prior/TEMP)
    te_sb = cpl.tile([C, C], F32, tag="te_sb")
    sigmoid(te_sb[:], prior_sb[:], scale=1.0 / TEMP)
    nc.vector.tensor_mul(te_sb[:], h_sb[:], te_sb[:])  # te_sb := h*t
    rhe = cpl.tile([C, C], F32, tag="rhe")
    nc.scalar.activation(rhe[:], h_sb[:], ACTF.Relu)
    abe = cpl.tile([C, C], F32, tag="abe")
    nc.scalar.activation(abe[:], h_sb[:], ACTF.Abs)
    softplus_neg(abe[:], abe[:])
    nc.vector.tensor_sub(rhe[:], rhe[:], te_sb[:])
    nc.vector.tensor_add(rhe[:], rhe[:], abe[:])   # rhe := bce_e
    nc.vector.tensor_mul(rhe[:], rhe[:], emf[:])   # rhe := bce_e * emf

    posf = cpl.tile([C, C], F32, tag="posf")
    nc.vector.tensor_single_scalar(posf[:], prior_sb[:], 0.0, ALU.is_gt)

    rcol = cpl.tile([C, 1], F32, tag="rcol")
    nc.vector.tensor_reduce(rcol[:], emf[:], axis=AX.X, op=ALU.add)
    n_edges = colsum_1x1(rcol, C, "n_edges")
    nc.vector.tensor_mul(abe[:], posf[:], emf[:])  # abe := posf*emf
    nc.vector.tensor_reduce(rcol[:], abe[:], axis=AX.X, op=ALU.add)
    n_pos = colsum_1x1(rcol, C, "n_pos")
    nc.vector.tensor_reduce(rcol[:], rhe[:], axis=AX.X, op=ALU.add)
    s1 = colsum_1x1(rcol, C, "s1")
    nc.vector.tensor_mul(abe[:], rhe[:], posf[:])  # abe := bce_e*emf*posf
    nc.vector.tensor_reduce(rcol[:], abe[:], axis=AX.X, op=ALU.add)
    s2 = colsum_1x1(rcol, C, "s2")

    nc.scalar.activation(abe[:], r_full[:], ACTF.Abs)
    nc.vector.tensor_mul(abe[:], abe[:], emf[:])   # abe := |r|*emf
    nc.vector.tensor_reduce(rcol[:], abe[:], axis=AX.X, op=ALU.add)
    rr_sum = colsum_1x1(rcol, C, "rr_sum")
    nc.vector.tensor_reduce(rcol[:], wadj[:], axis=AX.X, op=ALU.add)
    wa_sum = colsum_1x1(rcol, C, "wa_sum")

    n_pos_c = cpl.tile([1, 1], F32, tag="n_pos_c")
    nc.vector.tensor_scalar_max(n_pos_c[:], n_pos[:], 1.0)
    n_neg = cpl.tile([1, 1], F32, tag="n_neg")
    nc.vector.tensor_sub(n_neg[:], n_edges[:], n_pos[:])
    nc.vector.tensor_scalar_max(n_neg[:], n_neg[:], 1.0)
    np_inv = cpl.tile([1, 1], F32, tag="np_inv")
    nc.vector.reciprocal(np_inv[:], n_pos_c[:])
    w_pos = cpl.tile([1, 1], F32, tag="w_pos")
    nc.vector.tensor_mul(w_pos[:], n_neg[:], np_inv[:])
    nc.vector.tensor_scalar(w_pos[:], w_pos[:], 1.0, 10.0, ALU.max, ALU.min)
    nc.vector.tensor_scalar_add(w_pos[:], w_pos[:], -1.0)  # w_pos := w_pos - 1

    nc.vector.tensor_scalar_max(n_edges[:], n_edges[:], 1.0)
    ne_inv = cpl.tile([1, 1], F32, tag="ne_inv")
    nc.vector.reciprocal(ne_inv[:], n_edges[:])

    # edge_loss = (s1 + (w_pos-1)*s2) / n_edges ; r_reg = 0.001*rr_sum/n_edges
    el_num = cpl.tile([1, 1], F32, tag="el_num")
    nc.vector.tensor_mul(el_num[:], w_pos[:], s2[:])
    nc.vector.tensor_add(el_num[:], el_num[:], s1[:])
    nc.vector.tensor_mul(el_num[:], el_num[:], ne_inv[:])  # el_num := edge_loss
    r_reg = cpl.tile([1, 1], F32, tag="r_reg")
    nc.vector.tensor_mul(r_reg[:], rr_sum[:], ne_inv[:])

    total = cpl.tile([1, 1], F32, tag="total")
    nc.vector.scalar_tensor_tensor(total[:], el_num[:], 0.1, cls_loss[:], ALU.mult, ALU.add)
    nc.vector.scalar_tensor_tensor(total[:], r_reg[:], 0.001, total[:], ALU.mult, ALU.add)
    nc.vector.scalar_tensor_tensor(total[:], wa_sum[:], 0.01 / (C * C), total[:], ALU.mult, ALU.add)
    nc.sync.dma_start(o_total.ap().rearrange("(p o) -> p o", o=1), total[:])

    dpool_cm.__exit__(None, None, None)
    cpl_cm.__exit__(None, None, None)
    cpm_cm.__exit__(None, None, None)
    wts_cm.__exit__(None, None, None)
    base_cm.__exit__(None, None, None)
    dram_cm.__exit__(None, None, None)
    pscam_cm.__exit__(None, None, None)
    psacc_cm.__exit__(None, None, None)
    ps_cm.__exit__(None, None, None)


_CACHE = {}


def _get_compiled():
    if "nc" in _CACHE:
        return _CACHE["nc"]
    nc = bacc.Bacc("TRN2", target_bir_lowering=False, debug=False, num_devices=N_CORES)
    with tile.TileContext(nc) as tc:
        _build_program(nc, tc)
    nc.compile()
    _CACHE["nc"] = nc
    return nc


def make_in_maps(inputs):
    feats = np.ascontiguousarray(np.asarray(inputs["feats"], np.float32)).reshape(B, F, PIX)
    tgt = np.asarray(inputs["img_labels"]).astype(np.float32)
    shared = {
        "cls_logits": np.asarray(inputs["cls_logits"], np.float32),
        "tgt": tgt,
        "prior_pmi": np.asarray(inputs["prior_pmi"], np.float32),
    }
    for k in ("em_w1", "em_w2", "em_w3", "pp_b1", "pp_b2", "msg_b1", "msg_b2",
              "ed_b", "em_b1", "em_b2", "em_b3", "alpha_b", "bias_b"):
        shared[k] = np.ascontiguousarray(np.asarray(inputs[k], np.float32))
    for k in ("pp_w1", "pp_w2", "msg_w1", "msg_w2", "ed_w", "alpha_w", "bias_w"):
        shared[k] = np.ascontiguousarray(np.asarray(inputs[k], np.float32).astype(np.float16))
    shared["em_w3"] = shared["em_w3"].reshape(64, 1)
    shared["bias_w"] = shared["bias_w"].reshape(HID, 1)
    in_maps = []
    for c in range(N_CORES):
        isel = np.zeros((C, IL), np.float32)
        for r in range(IL):
            isel[IL * c + r, r] = 1.0
        m = dict(shared)
        # strided batch shard: core c owns global images c, 8+c, 16+c, 24+c
        m["feats_l"] = np.ascontiguousarray(feats[c::N_CORES])
        m["iselT"] = isel
        in_maps.append(m)
    return in_maps


def run(inputs, trace=False):
    nc = _get_compiled()
    res = run_bass_kernel_spmd(
        nc, make_in_maps(inputs), core_ids=list(range(N_CORES)), trace=trace
    )
    r0 = res.results[0]
    cam = np.empty((B, C, PIX), np.float32)
    for c in range(N_CORES):
        cam[c::N_CORES] = res.results[c]["o_cam"]
    out = (
        r0["o_wadj"],
        r0["o_aw"],
        r0["o_dlog"],
        cam.reshape(B, C, H, W),
        r0["o_refined"],
        np.float32(r0["o_total"].reshape(())),
    )
    return out, res


def kernel(**inputs):
    out, _ = run(inputs, trace=False)
    return out


def bench(inputs, iters=12):
    """Time the NEFF with device-resident inputs (no donation, no re-transfer)."""
    import time

    import jax
    import numpy as np_
    from jax.experimental.shard_map import shard_map
    from jax.sharding import Mesh, NamedSharding, PartitionSpec

    from concourse import bass2jax as b2j
    from concourse import mybir as mb

    nc = _get_compiled()
    b2j.install_neuronx_cc_hook()
    partition_name = nc.partition_id_tensor.name if nc.partition_id_tensor else None
    in_names, out_names, out_avals, zero_outs = [], [], [], []
    for alloc in nc.m.functions[0].allocations:
        if not isinstance(alloc, mb.MemoryLocationSet):
            continue
        name = alloc.memorylocations[0].name
        if alloc.kind == "ExternalInput":
            if name != partition_name:
                in_names.append(name)
        elif alloc.kind == "ExternalOutput":
            out_names.append(name)
            shape = tuple(alloc.tensor_shape)
            dtype = mb.dt.np(alloc.dtype)
            out_avals.append(jax.core.ShapedArray(shape, dtype))
            zero_outs.append(np_.zeros(shape, dtype))
    n_params = len(in_names)
    all_in_names = list(in_names) + list(out_names)
    if partition_name is not None:
        all_in_names.append(partition_name)

    def _body(*args):
        operands = list(args)
        if partition_name is not None:
            operands.append(b2j.partition_id_tensor())
        outs = b2j._bass_exec_p.bind(
            *operands,
            out_avals=tuple(out_avals),
            in_names=tuple(all_in_names),
            out_names=tuple(out_names),
            lowering_input_output_aliases=(),
            sim_require_finite=True,
            sim_require_nnan=True,
            nc=nc,
        )
        return tuple(outs)

    devices = jax.devices()[:N_CORES]
    mesh = Mesh(np_.asarray(devices), ("core",))
    n_outs = len(out_names)
    in_specs = (PartitionSpec("core"),) * (n_params + n_outs)
    out_specs = (PartitionSpec("core"),) * n_outs
    sharded = jax.jit(
        shard_map(_body, mesh=mesh, in_specs=in_specs, out_specs=out_specs, check_rep=False),
        keep_unused=True,
    )
    in_maps = make_in_maps(inputs)
    sh = NamedSharding(mesh, PartitionSpec("core"))
    concat_in = [
        jax.device_put(
            np_.concatenate([np_.asarray(in_maps[c][n]) for c in range(N_CORES)], axis=0), sh
        )
        for n in in_names
    ]
    concat_zeros = [
        jax.device_put(np_.zeros((N_CORES * z.shape[0], *z.shape[1:]), z.dtype), sh)
        for z in zero_outs
    ]
    out = sharded(*concat_in, *concat_zeros)
    jax.block_until_ready(out)
    times = []
    for _ in range(iters):
        t0 = time.perf_counter()
        out = sharded(*concat_in, *concat_zeros)
        jax.block_until_ready(out)
        times.append(time.perf_counter() - t0)
    return times, out, out_names
